# revision 41
# baseline (speedup 1.0000x reference)
"""NeRF-style render kernel for TRN2 (8 NeuronCores, data-parallel over rays).

Self-contained: hardcodes all shapes. Coarse proposal MLP runs in fp32
(resampling is precision-critical), fine MLP in float32r.
"""
import os
import sys

sys.path.insert(0, '/opt/trn_rl_repo')
import numpy as np
import concourse.bass as bass
import concourse.bacc as bacc
import concourse.tile as tile
import concourse.mybir as mybir
from concourse.bass_utils import run_bass_kernel_spmd

F32 = mybir.dt.float32
F32R = mybir.dt.float32r
AF = mybir.ActivationFunctionType
OP = mybir.AluOpType

NCORES = 8
R = 128          # rays per core
S = 128          # samples per pass
CHUNK_RAYS = 16  # rays per chunk
NCHUNK = R // CHUNK_RAYS          # 8
CN = CHUNK_RAYS * S               # 2048 cols per chunk
TILE_N = 512                      # matmul moving size
NTILE = CN // TILE_N              # 4 point-tiles per chunk

MAGIC = np.float32(12582912.0)    # 1.5 * 2^23 (round-to-int trick)
INV2PI = np.float32(1.0 / (2.0 * np.pi))
C1 = np.float32(6.28125)          # 2*pi split, k*C1 exact for k < 2^13
C2 = np.float32(2.0 * np.pi - 6.28125)

BUILD_STAGE = int(os.environ.get("KERNEL_STAGE", "3"))
DEBUG_OUT = os.environ.get("KERNEL_DEBUG", "0") == "1"


# ---------------------------------------------------------------- host prep
def _posenc_rows(nf, span=None, minp=None):
    """A3 [6*nf,3] / const [6*nf] for rows f-major: per f: 3 sin, 3 cos."""
    rows = 6 * nf
    A3 = np.zeros((rows, 3), np.float64)
    ph = np.zeros((rows,), np.float64)
    for f in range(nf):
        for k in range(6):
            r = 6 * f + k
            d = k % 3
            sc = 2.0 ** f
            if span is not None:
                A3[r, d] = sc / span[d]
                ph[r] = -sc * minp[d] / span[d]
            else:
                A3[r, d] = sc
            if k >= 3:
                ph[r] += np.pi / 2.0
    return A3, ph


def host_prep(inp):
    c = {}
    f32 = np.float32

    # coarse enc: per-ray rank-1 matrices  arg[i,(r,s)] = m*(B@d) + (C@[o;1])
    # cC4T [4,63]: cols 0:60 sin rows (A3s row + phase), cols 60:63 linear xyz
    A3s, phs = _posenc_rows(10)
    cC4 = np.zeros((4, 63), np.float64)
    cC4[0:3, 0:60] = A3s.T
    cC4[3, 0:60] = phs
    cC4[0:3, 60:63] = np.eye(3)
    c['cC4T'] = cC4.astype(f32).copy()                           # [4,63]

    # fine posenc rows: [sinx60, sinapp36, xyz3, appx3]
    minp = inp['min_point'].astype(np.float64)
    span = (inp['max_point'] - inp['min_point']).astype(np.float64)
    A3a, pha = _posenc_rows(6, span=span, minp=minp)
    pad4 = np.zeros((4, 3))
    fA3 = np.concatenate([A3s, pad4, A3a, np.eye(3), np.diag(1.0 / span)], 0)
    fph = np.concatenate([phs, np.zeros(4), pha, np.zeros(3), -minp / span], 0)
    c['fA3T'] = fA3.T.astype(f32).copy()                         # [3,106]
    c['fA4T'] = np.concatenate([fA3, fph[:, None]], 1).T.astype(f32).copy()

    # per-ray enc matrices (lhsT)
    Ad = np.zeros((24, 4), np.float64)
    for f in range(4):
        for k in range(6):
            r = 6 * f + k
            Ad[r, k % 3] = 2.0 ** f
            if k >= 3:
                Ad[r, 3] = np.pi / 2.0
    c['AdT'] = Ad.T.astype(f32).copy()                           # [4,24]
    At = np.zeros((12, 2), np.float64)
    for f in range(6):
        At[2 * f, 0] = 2.0 ** f
        At[2 * f + 1, 0] = 2.0 ** f
        At[2 * f + 1, 1] = np.pi / 2.0
    c['AtT'] = At.T.astype(f32).copy()                           # [2,12]

    perm63 = list(range(3, 63)) + [0, 1, 2]
    c['pW0my'] = np.ascontiguousarray(inp['pW0'][perm63])        # [63,128]
    c['pW1'] = inp['pW1'].copy()
    c['pW2'] = inp['pW2'].copy()
    c['pWo'] = inp['pWo'].copy()                                 # [128,1]
    c['pb0col'] = inp['pb0'].reshape(-1, 1).copy()
    c['pb1col'] = inp['pb1'].reshape(-1, 1).copy()
    c['pb2col'] = inp['pb2'].reshape(-1, 1).copy()

    c['fW0my'] = np.ascontiguousarray(inp['fW0'][perm63])        # [63,256]

    def pack_km(Wm):  # [256, 256] -> [128, 4, 128], slot 2k+m
        out = np.zeros((128, 4, 128), f32)
        for k in range(2):
            for m in range(2):
                out[:, 2 * k + m, :] = Wm[k * 128:(k + 1) * 128,
                                          m * 128:(m + 1) * 128]
        return out

    for i in range(3):
        c[f'fWm{i}'] = pack_km(inp['fWm'][i])
        c[f'fWp{i}'] = pack_km(inp['fWp'][i])
    c['fWs_h'] = pack_km(inp['fWs'][0:256])
    c['fWs_e'] = np.ascontiguousarray(inp['fWs'][256:][perm63])  # [63,256]
    c['fb0col'] = inp['fb0'].reshape(2, 128).T.copy()            # [128,2]
    for i in range(3):
        c[f'fbm{i}col'] = inp['fbm'][i].reshape(2, 128).T.copy()
        c[f'fbp{i}col'] = inp['fbp'][i].reshape(2, 128).T.copy()
    c['fbscol'] = inp['fbs'].reshape(2, 128).T.copy()

    # view head: fold Wfeat into Wview
    Wv = inp['Wview']
    Wv_d, Wv_emb, Wv_t, Wv_app = (Wv[256:283], Wv[283:331],
                                  Wv[331:344], Wv[344:383])
    Wfc = (inp['Wfeat'].astype(np.float64) @ Wv[0:256].astype(np.float64)
           ).astype(f32)
    out = np.zeros((128, 2, 128), f32)
    out[:, 0, :] = Wfc[0:128]
    out[:, 1, :] = Wfc[128:256]
    c['Wfc'] = out
    c['bveffcol'] = (inp['bfeat'].astype(np.float64)
                     @ Wv[0:256].astype(np.float64)
                     + inp['bview'].astype(np.float64)
                     ).astype(f32).reshape(-1, 1)
    perm39 = list(range(3, 39)) + [0, 1, 2]
    c['Wv_app'] = np.ascontiguousarray(Wv_app[perm39])           # [39,128]
    c['Wv_d_lin'] = np.ascontiguousarray(Wv_d[0:3])
    c['Wv_d_sin'] = np.ascontiguousarray(Wv_d[3:27])
    c['Wv_emb'] = np.ascontiguousarray(Wv_emb)
    c['Wv_t_lin'] = np.ascontiguousarray(Wv_t[0:1])
    c['Wv_t_sin'] = np.ascontiguousarray(Wv_t[1:13])
    c['Wsig'] = np.stack([inp['Wsig'][0:128, 0],
                          inp['Wsig'][128:256, 0]], 1).copy()    # [128,2]
    c['Wrgb'] = inp['Wrgb'].copy()                               # [128,3]
    c['brgbcol'] = inp['brgb'].reshape(-1, 1).copy()             # [3,1]
    c['brgbcol2'] = (0.5 * inp['brgb']).reshape(-1, 1).astype(f32)
    c['emb_table'] = inp['emb_table'].copy()

    c['sgrid'] = np.broadcast_to(
        np.arange(129, dtype=f32) / 128.0, (128, 129)).copy()
    c['identity'] = np.eye(128, dtype=f32)
    E = np.zeros((4, 512), f32)
    for rl in range(4):
        E[rl, rl * 128:(rl + 1) * 128] = 1.0
    c['Etile'] = E
    c['iotacol'] = np.arange(100, dtype=f32).reshape(-1, 1)
    scalars = dict(pbo_f=float(inp['pbo'][0]), bsig_f=float(inp['bsig'][0]))
    return c, scalars


INPUT_SHAPES = {
    'rays': (R, 12),
    'cC4T': (4, 63),
    'fA3T': (3, 106), 'fA4T': (4, 106),
    'AdT': (4, 24), 'AtT': (2, 12),
    'pW0my': (63, 128),
    'pW1': (128, 128), 'pW2': (128, 128), 'pWo': (128, 1),
    'pb0col': (128, 1), 'pb1col': (128, 1), 'pb2col': (128, 1),
    'fW0my': (63, 256), 'fWm0': (128, 4, 128), 'fWm1': (128, 4, 128),
    'fWm2': (128, 4, 128), 'fWp0': (128, 4, 128), 'fWp1': (128, 4, 128),
    'fWp2': (128, 4, 128), 'fWs_h': (128, 4, 128), 'fWs_e': (63, 256),
    'fb0col': (128, 2), 'fbm0col': (128, 2), 'fbm1col': (128, 2),
    'fbm2col': (128, 2), 'fbp0col': (128, 2), 'fbp1col': (128, 2),
    'fbp2col': (128, 2), 'fbscol': (128, 2),
    'Wfc': (128, 2, 128), 'bveffcol': (128, 1), 'Wv_app': (39, 128),
    'Wv_d_lin': (3, 128), 'Wv_d_sin': (24, 128), 'Wv_emb': (48, 128),
    'Wv_t_lin': (1, 128), 'Wv_t_sin': (12, 128),
    'Wsig': (128, 2), 'Wrgb': (128, 3), 'brgbcol': (3, 1),
    'brgbcol2': (3, 1),
    'emb_table': (100, 48),
    'sgrid': (128, 129), 'identity': (128, 128),
    'Etile': (4, 512), 'iotacol': (100, 1),
}
F32R_WEIGHTS = {'fW0my', 'fWm0', 'fWm1', 'fWm2', 'fWp0', 'fWp1', 'fWp2',
                'fWs_h', 'fWs_e', 'Wfc', 'Wv_app', 'Wv_d_lin', 'Wv_d_sin',
                'Wv_emb', 'Wv_t_lin', 'Wv_t_sin', 'Wsig', 'Wrgb',
                'emb_table', 'Etile'}


# ---------------------------------------------------------------- bass build
def build_nc(pbo_f, bsig_f, stage=3, debug=False):
    nc = bacc.Bacc("TRN2", target_bir_lowering=False)
    D = {k: nc.dram_tensor(k, list(v), F32, kind="ExternalInput")
         for k, v in INPUT_SHAPES.items()}
    OUT = nc.dram_tensor("rgb_out", [R, 3], F32, kind="ExternalOutput")
    dbg = {}
    if debug:
        for nm, shp in [("d_sigc", (R, S)), ("d_zf", (R, S + 1)),
                        ("d_wc", (R, S)), ("d_sigf", (R, S)),
                        ("d_wf", (R, S)), ("d_hvray", (128, R)),
                        ("d_ec", (63, CN)), ("d_efa", (63, CN)),
                        ("d_efb", (39, CN)), ("d_h1", (128, 2 * TILE_N))]:
            dbg[nm] = nc.dram_tensor(nm, list(shp), F32, kind="ExternalOutput")
    with tile.TileContext(nc) as tc:
        _body(nc, tc, D, OUT, dbg, pbo_f, bsig_f, stage, debug)
    nc.compile()
    return nc


def _body(nc, tc, D, OUT, dbg, pbo_f, bsig_f, stage, debug):
    from contextlib import ExitStack
    ctx = ExitStack()
    wpool = ctx.enter_context(tc.tile_pool(name="w", bufs=1))
    per = ctx.enter_context(tc.tile_pool(name="per", bufs=1))
    pp2 = ctx.enter_context(tc.tile_pool(name="pp2", bufs=2))
    big = ctx.enter_context(tc.tile_pool(name="big", bufs=2))
    hp = ctx.enter_context(tc.tile_pool(name="h", bufs=3))
    dram = ctx.enter_context(tc.tile_pool(name="dr", bufs=2, space="DRAM"))
    psA = ctx.enter_context(tc.tile_pool(name="psA", bufs=4, space="PSUM"))
    psS = ctx.enter_context(tc.tile_pool(name="psS", bufs=1, space="PSUM"))
    psR = ctx.enter_context(tc.tile_pool(name="psR", bufs=1, space="PSUM"))
    psC = ctx.enter_context(tc.tile_pool(name="psC", bufs=1, space="PSUM"))

    W = {}
    for k, t in D.items():
        if k == 'rays':
            continue
        dt = F32R if k in F32R_WEIGHTS else F32
        tl = wpool.tile(list(t.shape), dt, tag="w_" + k)
        nc.sync.dma_start(tl[:], t[:].bitcast(F32R) if dt == F32R else t[:])
        W[k] = tl
    rays = wpool.tile([R, 12], F32, tag="w_rays")
    nc.sync.dma_start(rays[:], D['rays'][:])
    ident = W['identity']

    # ---------------- phase 0: per-ray prep (ray-major layout)
    nearc = per.tile([R, 1], F32)
    nc.vector.tensor_scalar(nearc[:], rays[:, 6:7], 1e-8, None, op0=OP.max)
    spanc = per.tile([R, 1], F32)
    nc.vector.tensor_tensor(spanc[:], rays[:, 7:8], nearc[:], op=OP.subtract)

    dsq = per.tile([R, 3], F32)
    nc.vector.tensor_tensor(dsq[:], rays[:, 3:6], rays[:, 3:6], op=OP.mult)
    ssum = per.tile([R, 1], F32)
    nc.vector.reduce_sum(ssum[:], dsq[:], axis=mybir.AxisListType.X)
    norm = per.tile([R, 1], F32)
    nc.scalar.activation(norm[:], ssum[:], AF.Sqrt)
    for it in range(2):
        t1 = per.tile([R, 1], F32, tag="nwt")
        nc.vector.reciprocal(t1[:], norm[:])
        nc.vector.scalar_tensor_tensor(t1[:], ssum[:], 1.0, t1[:],
                                       op0=OP.mult, op1=OP.mult)
        nc.vector.tensor_tensor(t1[:], t1[:], norm[:], op=OP.add)
        nc.vector.tensor_scalar(norm[:], t1[:], 0.5, None, op0=OP.mult)
    invn = per.tile([R, 1], F32)
    nc.vector.reciprocal(invn[:], norm[:])

    # bundle: 0:3 oc, 3 ones | 4:7 dc | 8:11 o, 11 ones | 12:15 dir |
    #         16:19 viewdir, 19 ones | 20 t, 21 ones | 22 embid
    bundle = per.tile([R, 28], F32)
    nc.gpsimd.memset(bundle[:], 0.0)
    nc.vector.scalar_tensor_tensor(bundle[:, 0:3], rays[:, 3:6], nearc[:],
                                   rays[:, 0:3], op0=OP.mult, op1=OP.add)
    nc.vector.memset(bundle[:, 3:4], 1.0)
    nc.vector.tensor_scalar(bundle[:, 4:7], rays[:, 3:6], spanc[:], None,
                            op0=OP.mult)
    nc.vector.tensor_copy(bundle[:, 8:11], rays[:, 0:3])
    nc.vector.memset(bundle[:, 11:12], 1.0)
    nc.vector.tensor_copy(bundle[:, 12:15], rays[:, 3:6])
    nc.vector.tensor_scalar(bundle[:, 16:19], rays[:, 3:6], invn[:], None,
                            op0=OP.mult)
    nc.vector.memset(bundle[:, 19:20], 1.0)
    nc.vector.tensor_copy(bundle[:, 20:21], rays[:, 8:9])
    nc.vector.memset(bundle[:, 21:22], 1.0)
    nc.vector.tensor_copy(bundle[:, 22:23], rays[:, 9:10])

    def transp(col):
        p = psC.tile([4, 128], F32, tag="ptp")
        nc.tensor.transpose(p[:], bundle[:, col:col + 4], ident[:])
        sb = per.tile([4, 128], F32, tag="tp%d" % col)
        nc.scalar.copy(sb[:], p[:])
        return sb

    ocT = transp(0)      # [ocT;ones]
    dcT = transp(4)      # [dcT;..]
    oT = transp(8)       # [oT;ones]
    dirT = transp(12)
    vdT = transp(16)     # [viewdirT;ones]
    tT = transp(20)      # [t;ones;embid]
    eiT = transp(22)     # row0 = embid (base 0 for partition_broadcast)

    def mm_copy(lhsT, rhs, shape, nm, dst_dtype=F32):
        p = psC.tile(shape, F32, tag="pmc")
        nc.tensor.matmul(p[:], lhsT, rhs, start=True, stop=True)
        sb = per.tile(shape, dst_dtype, tag="mc_" + nm)
        nc.scalar.copy(sb[:], p[:])
        return sb

    Bf = mm_copy(W['fA3T'][:], dirT[0:3, :], [106, 128], "Bf")
    Cf = mm_copy(W['fA4T'][:], oT[:], [106, 128], "Cf")
    Bc = mm_copy(W['cC4T'][0:3, :], dirT[0:3, :], [63, 128], "Bc")
    Cc = mm_copy(W['cC4T'][:], oT[:], [63, 128], "Cc")

    def rangered_v(ap, shape, tag):
        sc = per.tile(shape, F32, tag=tag)
        nc.vector.tensor_scalar(sc[:], ap, float(INV2PI), float(MAGIC),
                                op0=OP.mult, op1=OP.add)
        nc.vector.tensor_scalar(sc[:], sc[:], float(MAGIC), None,
                                op0=OP.subtract)
        nc.vector.scalar_tensor_tensor(ap, sc[:], -float(C1), ap,
                                       op0=OP.mult, op1=OP.add)
        nc.vector.scalar_tensor_tensor(ap, sc[:], -float(C2), ap,
                                       op0=OP.mult, op1=OP.add)

    # per-ray view features
    argd = mm_copy(W['AdT'][:], vdT[:], [24, 128], 'argd')
    rangered_v(argd[:], [24, 128], "rrd")
    sind = per.tile([24, 128], F32R)
    nc.scalar.activation(sind[:], argd[:], AF.Sin)
    vd_r = per.tile([4, 128], F32R)
    nc.vector.tensor_copy(vd_r[:], vdT[:])

    argt = mm_copy(W['AtT'][:], tT[0:2, :], [12, 128], 'argt')
    rangered_v(argt[:], [12, 128], "rrt")
    sint = per.tile([12, 128], F32R)
    nc.scalar.activation(sint[:], argt[:], AF.Sin)
    t_r = per.tile([4, 128], F32R)
    nc.vector.tensor_copy(t_r[:], tT[:])

    embBC = per.tile([100, 128], F32)
    nc.gpsimd.partition_broadcast(embBC[:], eiT[0:1, :], channels=100)
    onehot = per.tile([100, 128], F32R)
    nc.vector.tensor_scalar(onehot[:], embBC[:], W['iotacol'][:], None,
                            op0=OP.is_equal)
    embT = mm_copy(W['emb_table'][:], onehot[:], [48, 128], 'embT', dst_dtype=F32R)

    phv = psC.tile([128, 128], F32, tag="pmc")
    nc.tensor.matmul(phv[:], W['Wv_d_lin'][:], vd_r[0:3, :],
                     start=True, stop=False)
    nc.tensor.matmul(phv[:], W['Wv_d_sin'][:], sind[:], start=False, stop=False)
    nc.tensor.matmul(phv[:], W['Wv_emb'][:], embT[:], start=False, stop=False)
    nc.tensor.matmul(phv[:], W['Wv_t_lin'][:], t_r[0:1, :],
                     start=False, stop=False)
    nc.tensor.matmul(phv[:], W['Wv_t_sin'][:], sint[:], start=False, stop=True)
    hvray = per.tile([128, 128], F32)
    nc.vector.tensor_scalar(hvray[:], phv[:], W['bveffcol'][:], None,
                            op0=OP.add)
    if debug:
        nc.sync.dma_start(dbg["d_hvray"][:], hvray[:])
    phvT = psC.tile([128, 128], F32, tag="pmc")
    nc.tensor.transpose(phvT[:], hvray[:], ident[:])
    hvrayT = per.tile([128, 128], F32R)
    nc.scalar.copy(hvrayT[:], phvT[:])
    hvb = dram.tile([128, 128], F32R, tag="hvb")
    nc.sync.dma_start(hvb[:], hvrayT[:])
    hvre = wpool.tile([4, 32, 128], F32R, tag="hvre")
    nc.sync.dma_start(hvre[:], hvb[:].rearrange("(t rl) m -> rl t m", rl=4))

    # coarse z edges
    zc = per.tile([R, S + 1], F32)
    nc.vector.tensor_scalar(zc[:], W['sgrid'][:], spanc[:], None, op0=OP.mult)
    nc.vector.tensor_scalar(zc[:], zc[:], nearc[:], None, op0=OP.add)
    midc = per.tile([R, S], F32)
    nc.vector.tensor_tensor(midc[:], zc[:, 0:S], zc[:, 1:S + 1], op=OP.add)
    nc.vector.tensor_scalar(midc[:], midc[:], 0.5, None, op0=OP.mult)

    # ======================= COARSE PASS =======================
    # midc bounce to DRAM once; per chunk DMA-replicate flat mids to 63 rows
    mc_dram = dram.tile([R, S], F32, tag="mcd", bufs=1)
    nc.scalar.dma_start(mc_dram[:], midc[:])
    sigcT = per.tile([R, S], F32, tag="sigcT")
    for ci in range(NCHUNK):
        r0 = ci * CHUNK_RAYS
        argc = big.tile([63, CN], F32, tag="arg")
        msrc = (mc_dram[r0:r0 + CHUNK_RAYS, :]
                .rearrange("p f -> (p f)").unsqueeze(0)
                .broadcast_to([63, CN]))
        nc.scalar.dma_start(argc[:], msrc)
        B3 = Bc[:, r0:r0 + CHUNK_RAYS].unsqueeze(2).broadcast_to(
            [63, CHUNK_RAYS, S])
        C3 = Cc[:, r0:r0 + CHUNK_RAYS].unsqueeze(2).broadcast_to(
            [63, CHUNK_RAYS, S])
        a3 = argc[:].rearrange("p (r s) -> p r s", r=CHUNK_RAYS)
        nc.vector.tensor_tensor(a3, a3, B3, op=OP.mult)
        nc.gpsimd.tensor_tensor(a3, a3, C3, op=OP.add)
        sc = big.tile([100, CN], F32, tag="mbcrr", bufs=2)
        nc.gpsimd.tensor_scalar(sc[0:60, :], argc[0:60, :], float(INV2PI),
                                float(MAGIC), op0=OP.mult, op1=OP.add)
        nc.gpsimd.tensor_scalar(sc[0:60, :], sc[0:60, :], float(MAGIC), None,
                                op0=OP.subtract)
        nc.vector.scalar_tensor_tensor(argc[0:60, :], sc[0:60, :], -float(C1),
                                       argc[0:60, :], op0=OP.mult, op1=OP.add)
        nc.vector.scalar_tensor_tensor(argc[0:60, :], sc[0:60, :], -float(C2),
                                       argc[0:60, :], op0=OP.mult, op1=OP.add)
        sb_ = dram.tile([1, CN], F32, tag="sigb")
        sigflat = pp2.tile([1, CN], F32, tag="sigflat", bufs=1)
        nc.scalar.activation(argc[0:60, :], argc[0:60, :], AF.Sin)
        # layer-major over tile pairs: PE works tile t+1 while relu(t) lands
        for tp in range(NTILE // 2):
            pair = (2 * tp, 2 * tp + 1)
            colsv = [slice(t * TILE_N, (t + 1) * TILE_N) for t in pair]
            hh = []
            for i, t in enumerate(pair):
                p1 = psA.tile([128, TILE_N], F32, tag="mmps")
                nc.tensor.matmul(p1[:], W['pW0my'][:], argc[:, colsv[i]],
                                 start=True, stop=True)
                h1 = hp.tile([128, TILE_N], F32, tag="ch", bufs=4)
                nc.scalar.activation(h1[:], p1[:], AF.Relu,
                                     bias=W['pb0col'][:])
                hh.append(h1)
            for i, t in enumerate(pair):
                p2 = psA.tile([128, TILE_N], F32, tag="mmps")
                nc.tensor.matmul(p2[:], W['pW1'][:], hh[i][:],
                                 start=True, stop=True)
                h2 = hp.tile([128, TILE_N], F32, tag="ch", bufs=4)
                if i == 0:
                    nc.vector.tensor_scalar(h2[:], p2[:], W['pb1col'][:], 0.0,
                                            op0=OP.add, op1=OP.max)
                else:
                    nc.scalar.activation(h2[:], p2[:], AF.Relu,
                                         bias=W['pb1col'][:])
                hh[i] = h2
            for i, t in enumerate(pair):
                p3 = psA.tile([128, TILE_N], F32, tag="mmps")
                nc.tensor.matmul(p3[:], W['pW2'][:], hh[i][:],
                                 start=True, stop=True)
                h3 = hp.tile([128, TILE_N], F32, tag="ch", bufs=4)
                if i == 0:
                    nc.scalar.activation(h3[:], p3[:], AF.Relu,
                                         bias=W['pb2col'][:])
                else:
                    nc.vector.tensor_scalar(h3[:], p3[:], W['pb2col'][:], 0.0,
                                            op0=OP.add, op1=OP.max)
                hh[i] = h3
            for i, t in enumerate(pair):
                ps_ = psS.tile([1, TILE_N], F32, tag="sigps")
                nc.tensor.matmul(ps_[:], W['pWo'][:], hh[i][:],
                                 start=True, stop=True)
                if t % 2 == 0:
                    nc.scalar.copy(sigflat[0:1, colsv[i]], ps_[:])
                else:
                    nc.vector.tensor_copy(sigflat[0:1, colsv[i]], ps_[:])
        nc.sync.dma_start(sb_[:], sigflat[:])
        nc.sync.dma_start(sigcT[r0:r0 + CHUNK_RAYS, :],
                          sb_[:].rearrange("a (p f) -> (a p) f", p=CHUNK_RAYS))

    if debug:
        nc.sync.dma_start(dbg["d_sigc"][:], sigcT[:])
    if stage < 2:
        ctx.close()
        return

    # ======================= raw2weights helper =======================
    def raw2w(sigT_ap, z_lo, z_hi, norm_ap, bias_f, nrows, tag):
        """w = alpha * exclusive-cumprod(1-alpha+1e-10); returns (w, dz)."""
        P = nrows
        dz = per.tile([P, S], F32, tag=tag + "dz")
        nc.vector.tensor_tensor(dz[:], z_hi, z_lo, op=OP.subtract)
        di = per.tile([P, S], F32, tag=tag + "di")
        nc.vector.tensor_scalar(di[:], dz[:], norm_ap, None, op0=OP.mult)
        s1 = per.tile([P, S], F32, tag=tag + "s1")
        nc.vector.tensor_scalar(s1[:], sigT_ap, bias_f, 0.0,
                                op0=OP.add, op1=OP.max)
        ea = per.tile([P, S], F32, tag=tag + "ea")
        nc.vector.tensor_tensor(ea[:], s1[:], di[:], op=OP.mult)
        e = per.tile([P, S], F32, tag=tag + "e")
        nc.scalar.activation(e[:], ea[:], AF.Exp, scale=-1.0)
        al = per.tile([P, S], F32, tag=tag + "al")
        nc.vector.tensor_scalar(al[:], e[:], -1.0, 1.0, op0=OP.mult, op1=OP.add)
        om = per.tile([P, S], F32, tag=tag + "om")
        nc.vector.tensor_scalar(om[:], e[:], 1e-10, None, op0=OP.add)
        tr = per.tile([P, S], F32, tag=tag + "tr")
        nc.vector.tensor_tensor_scan(tr[:], om[:], om[:], 1.0,
                                     op0=OP.mult, op1=OP.bypass)
        w = per.tile([P, S], F32, tag=tag + "w")
        nc.vector.tensor_copy(w[:, 0:1], al[:, 0:1])
        nc.vector.tensor_tensor(w[:, 1:S], al[:, 1:S], tr[:, 0:S - 1],
                                op=OP.mult)
        return w, dz

    zf = per.tile([R, S + 1], F32)
    wc, dzc = raw2w(sigcT[:], zc[:, 0:S], zc[:, 1:S + 1],
                    norm[:], pbo_f, R, "c")
    Wt = per.tile([R, S], F32, tag="Wt")
    nc.vector.tensor_scalar(Wt[:], wc[:], 1e-5, None, op0=OP.add)
    Sx = per.tile([R, S], F32, tag="Sx")
    nc.vector.memset(Sx[:, 0:1], 0.0)
    nc.vector.tensor_tensor_scan(Sx[:, 1:S], Wt[:, 0:S - 1],
                                 Wt[:, 0:S - 1], 0.0,
                                 op0=OP.add, op1=OP.bypass)
    Tt = per.tile([R, 1], F32, tag="Tt")
    nc.vector.tensor_tensor(Tt[:], Sx[:, S - 1:S], Wt[:, S - 1:S],
                            op=OP.add)
    P2 = per.tile([R, S], F32, tag="P2")
    nc.vector.reciprocal(P2[:], Wt[:])
    nc.vector.tensor_tensor(P2[:], P2[:], dzc[:], op=OP.mult)
    JB = 16
    Sx_b = Sx[:].unsqueeze(1).broadcast_to([R, JB, S])
    P2_b = P2[:].unsqueeze(1).broadcast_to([R, JB, S])
    dz_b = dzc[:].unsqueeze(1).broadcast_to([R, JB, S])
    for jb in range(0, S, JB):
        rs_ = pp2.tile([R, JB * S], F32, tag="rsx", name="rs_", bufs=1)
        x3 = rs_[:].rearrange("p (j s) -> p j s", j=JB)
        g_b = W['sgrid'][:, jb:jb + JB].unsqueeze(2).broadcast_to([R, JB, S])
        nc.vector.scalar_tensor_tensor(x3, g_b, Tt[:], Sx_b,
                                       op0=OP.mult, op1=OP.subtract)
        nc.vector.scalar_tensor_tensor(x3, x3, 0.0, P2_b,
                                       op0=OP.max, op1=OP.mult)
        nc.vector.tensor_tensor(x3, x3, dz_b, op=OP.min)
        nc.vector.tensor_reduce(zf[:, jb:jb + JB], x3,
                                axis=mybir.AxisListType.X, op=OP.add)
    # last edge j=S: all bins saturate -> sum(dz) == zc[:,S] - zc[:,0]
    nc.vector.tensor_tensor(zf[:, S:S + 1], zc[:, S:S + 1], zc[:, 0:1],
                            op=OP.subtract)
    nc.vector.tensor_scalar(zf[:], zf[:], zc[:, 0:1], None, op0=OP.add)
    if debug:
        nc.sync.dma_start(dbg["d_zf"][:], zf[:])
        nc.sync.dma_start(dbg["d_wc"][:], wc[:])
    if stage < 3:
        ctx.close()
        return

    midf = per.tile([R, S], F32)
    nc.vector.tensor_tensor(midf[:], zf[:, 0:S], zf[:, 1:S + 1], op=OP.add)
    nc.vector.tensor_scalar(midf[:], midf[:], 0.5, None, op0=OP.mult)

    # ======================= FINE PASS =======================
    rgbmT = per.tile([3, 128], F32)
    nc.vector.memset(rgbmT[:], 0.0)

    mf_dram = dram.tile([R, S], F32, tag="mfd", bufs=1)
    nc.scalar.dma_start(mf_dram[:], midf[:])
    for ci in range(NCHUNK):
        r0 = ci * CHUNK_RAYS
        argf = big.tile([106, CN], F32, tag="arg")
        msrc = (mf_dram[r0:r0 + CHUNK_RAYS, :]
                .rearrange("p f -> (p f)").unsqueeze(0)
                .broadcast_to([106, CN]))
        nc.scalar.dma_start(argf[:], msrc)
        b3 = Bf[:, r0:r0 + CHUNK_RAYS].unsqueeze(2).broadcast_to(
            [106, CHUNK_RAYS, S])
        c3 = Cf[:, r0:r0 + CHUNK_RAYS].unsqueeze(2).broadcast_to(
            [106, CHUNK_RAYS, S])
        a3 = argf[:].rearrange("p (r s) -> p r s", r=CHUNK_RAYS)
        nc.vector.tensor_tensor(a3, a3, b3, op=OP.mult)
        nc.gpsimd.tensor_tensor(a3, a3, c3, op=OP.add)
        sc = big.tile([100, CN], F32, tag="mbcrr", bufs=2)
        TWOPI = float(np.float32(2.0 * np.pi))
        for lo, hi in ((0, 60), (64, 100)):
            nc.gpsimd.tensor_scalar(sc[lo:hi, :], argf[lo:hi, :], float(INV2PI),
                                    float(MAGIC), op0=OP.mult, op1=OP.add)
            nc.gpsimd.tensor_scalar(sc[lo:hi, :], sc[lo:hi, :], float(MAGIC),
                                    None, op0=OP.subtract)
            nc.vector.scalar_tensor_tensor(argf[lo:hi, :], sc[lo:hi, :],
                                           -TWOPI, argf[lo:hi, :],
                                           op0=OP.mult, op1=OP.add)
        efa = big.tile([63, CN], F32R, tag="efa")
        efb = big.tile([39, CN], F32R, tag="efb")
        nc.scalar.activation(efa[0:60, :], argf[0:60, :], AF.Sin)
        nc.scalar.activation(efb[0:36, :], argf[64:100, :], AF.Sin)
        nc.sync.dma_start(efa[60:63, :], argf[100:103, :].bitcast(F32R))
        nc.sync.dma_start(efb[36:39, :], argf[103:106, :].bitcast(F32R))
        if debug and ci == 0:
            nc.sync.dma_start(dbg["d_efa"][:], efa[:].bitcast(F32))
            nc.sync.dma_start(dbg["d_efb"][:], efb[:].bitcast(F32))

        rgbS = big.tile([3, CN], F32, tag="rgbS")
        sb_ = dram.tile([1, CN], F32, tag="sigb")
        sigflat = pp2.tile([1, CN], F32, tag="sigflat", bufs=1)

        def relu2(pmm, bname, i):
            """bias+relu both halves; engines alternate per tile parity."""
            hout = hp.tile([128, 2 * TILE_N], F32R, tag="fh", bufs=4)
            if i == 0:
                nc.scalar.activation(hout[:, 0:TILE_N], pmm[0][:], AF.Relu,
                                     bias=W[bname][:, 0:1])
                nc.vector.tensor_scalar(hout[:, TILE_N:], pmm[1][:],
                                        W[bname][:, 1:2], 0.0,
                                        op0=OP.add, op1=OP.max)
            else:
                nc.vector.tensor_scalar(hout[:, 0:TILE_N], pmm[0][:],
                                        W[bname][:, 0:1], 0.0,
                                        op0=OP.add, op1=OP.max)
                nc.scalar.activation(hout[:, TILE_N:], pmm[1][:], AF.Relu,
                                     bias=W[bname][:, 1:2])
            return hout

        for tp in range(NTILE // 2):
            pair = (2 * tp, 2 * tp + 1)
            colsv = [slice(t * TILE_N, (t + 1) * TILE_N) for t in pair]
            hh = []
            for i, t in enumerate(pair):
                pm = [psA.tile([128, TILE_N], F32, tag="mmps",
                               name="pm%d" % _m) for _m in range(2)]
                for m in range(2):
                    nc.tensor.matmul(pm[m][:],
                                     W['fW0my'][:, m * 128:(m + 1) * 128],
                                     efa[:, colsv[i]], start=True, stop=True)
                hh.append(relu2(pm, 'fb0col', i))
            if debug and ci == 0:
                nc.sync.dma_start(dbg["d_h1"][:], hh[0][:].bitcast(F32))

            for wname, bname, skip in (
                    ('fWm0', 'fbm0col', False), ('fWm1', 'fbm1col', False),
                    ('fWm2', 'fbm2col', False), ('fWs_h', 'fbscol', True),
                    ('fWp0', 'fbp0col', False), ('fWp1', 'fbp1col', False),
                    ('fWp2', 'fbp2col', False)):
                for i, t in enumerate(pair):
                    hin = hh[i]
                    pmm = [psA.tile([128, TILE_N], F32, tag="mmps",
                                    name="pmm%d" % _m) for _m in range(2)]
                    for m in range(2):
                        nc.tensor.matmul(pmm[m][:], W[wname][:, m, :],
                                         hin[:, 0:TILE_N],
                                         start=True, stop=False)
                        nc.tensor.matmul(pmm[m][:], W[wname][:, 2 + m, :],
                                         hin[:, TILE_N:],
                                         start=False, stop=not skip)
                        if skip:
                            nc.tensor.matmul(
                                pmm[m][:],
                                W['fWs_e'][:, m * 128:(m + 1) * 128],
                                efa[:, colsv[i]], start=False, stop=True)
                    hh[i] = relu2(pmm, bname, i)

            for i, t in enumerate(pair):
                h = hh[i]
                gtile = ci * NTILE + t
                ps_ = psS.tile([1, TILE_N], F32, tag="sigps")
                nc.tensor.matmul(ps_[:], W['Wsig'][:, 0:1], h[:, 0:TILE_N],
                                 start=True, stop=False)
                nc.tensor.matmul(ps_[:], W['Wsig'][:, 1:2], h[:, TILE_N:],
                                 start=False, stop=True)
                if t % 2 == 0:
                    nc.scalar.copy(sigflat[0:1, colsv[i]], ps_[:])
                else:
                    nc.vector.tensor_copy(sigflat[0:1, colsv[i]], ps_[:])

                pv = psA.tile([128, TILE_N], F32, tag="mmps")
                nc.tensor.matmul(pv[:], W['Wfc'][:, 0, :], h[:, 0:TILE_N],
                                 start=True, stop=False)
                nc.tensor.matmul(pv[:], W['Wfc'][:, 1, :], h[:, TILE_N:],
                                 start=False, stop=False)
                nc.tensor.matmul(pv[:], W['Wv_app'][:], efb[:, colsv[i]],
                                 start=False, stop=False)
                nc.tensor.matmul(pv[:], hvre[:, gtile, :], W['Etile'][:],
                                 start=False, stop=True)
                hv = hp.tile([128, TILE_N], F32R, tag="fhv", bufs=2)
                nc.scalar.activation(hv[:], pv[:], AF.Relu)

                prgb = psR.tile([3, TILE_N], F32, tag="rgbps")
                nc.tensor.matmul(prgb[:], W['Wrgb'][:], hv[:],
                                 start=True, stop=True)
                nc.scalar.activation(rgbS[0:3, colsv[i]], prgb[:],
                                     AF.Sigmoid, bias=W['brgbcol'][:])

        nc.sync.dma_start(sb_[:], sigflat[:])
        sigch = pp2.tile([CHUNK_RAYS, S], F32, tag="sigch")
        nc.sync.dma_start(sigch[:],
                          sb_[:].rearrange("a (p f) -> (a p) f", p=CHUNK_RAYS))
        zfc = pp2.tile([CHUNK_RAYS, S + 1], F32, tag="zfc")
        nc.sync.dma_start(zfc[:], zf[r0:r0 + CHUNK_RAYS, :])
        normc = pp2.tile([CHUNK_RAYS, 1], F32, tag="normc")
        nc.sync.dma_start(normc[:], norm[r0:r0 + CHUNK_RAYS, :])

        wf, _dzf = raw2w(sigch[:], zfc[:, 0:S], zfc[:, 1:S + 1],
                         normc[:], bsig_f, CHUNK_RAYS, "f")
        if debug:
            nc.sync.dma_start(dbg["d_sigf"][r0:r0 + CHUNK_RAYS, :], sigch[:])
            nc.sync.dma_start(dbg["d_wf"][r0:r0 + CHUNK_RAYS, :], wf[:])

        wb = dram.tile([CHUNK_RAYS, S], F32, tag="wb")
        nc.sync.dma_start(wb[:], wf[:])
        wBC = pp2.tile([3, CN], F32, tag="wbc", bufs=1)
        nc.sync.dma_start(
            wBC[:],
            wb[:].rearrange("p f -> (p f)").unsqueeze(0).broadcast_to([3, CN]))
        nc.gpsimd.tensor_tensor(rgbS[0:3, :], rgbS[0:3, :], wBC[0:3, :],
                                op=OP.mult)
        nc.vector.tensor_reduce(
            rgbmT[0:3, r0:r0 + CHUNK_RAYS],
            rgbS[0:3, :].rearrange("p (r s) -> p r s", r=CHUNK_RAYS),
            axis=mybir.AxisListType.X, op=OP.add)

    # out: transpose [3,128] -> [128,3] via DRAM bounce
    rb = dram.tile([3, 128], F32, tag="rb")
    nc.sync.dma_start(rb[:], rgbmT[:])
    rgbout = per.tile([128, 3], F32)
    nc.sync.dma_start(rgbout[:], rb[:].rearrange("c r -> r c"))
    nc.sync.dma_start(OUT[:], rgbout[:])
    ctx.close()


# ---------------------------------------------------------------- entry
_CACHE = {}


def kernel(**inputs):
    inp = {k: np.asarray(v) for k, v in inputs.items()}
    consts, scal = host_prep(inp)
    key = (BUILD_STAGE, DEBUG_OUT, scal['pbo_f'], scal['bsig_f'])
    if key not in _CACHE:
        _CACHE[key] = build_nc(scal['pbo_f'], scal['bsig_f'],
                               stage=BUILD_STAGE, debug=DEBUG_OUT)
    nc = _CACHE[key]
    rays = np.asarray(inp['rays'], np.float32)
    in_maps = []
    for core in range(NCORES):
        m = {k: np.ascontiguousarray(v, dtype=np.float32)
             for k, v in consts.items()}
        m['rays'] = np.ascontiguousarray(rays[core * R:(core + 1) * R])
        in_maps.append(m)
    res = run_bass_kernel_spmd(nc, in_maps, core_ids=list(range(NCORES)))
    globals()['_LAST_RESULTS'] = res
    return np.concatenate([r['rgb_out'] for r in res.results], 0)



# revision 42
# speedup vs baseline: 1.0158x; 1.0158x over previous
"""NeRF-style render kernel for TRN2 (8 NeuronCores, data-parallel over rays).

Self-contained: hardcodes all shapes. Coarse proposal MLP runs in fp32
(resampling is precision-critical), fine MLP in float32r.
"""
import os
import sys

sys.path.insert(0, '/opt/trn_rl_repo')
import numpy as np
import concourse.bass as bass
import concourse.bacc as bacc
import concourse.tile as tile
import concourse.mybir as mybir
from concourse.bass_utils import run_bass_kernel_spmd

F32 = mybir.dt.float32
F32R = mybir.dt.float32r
AF = mybir.ActivationFunctionType
OP = mybir.AluOpType

NCORES = 8
R = 128          # rays per core
S = 128          # samples per pass
CHUNK_RAYS = 16  # rays per chunk
NCHUNK = R // CHUNK_RAYS          # 8
CN = CHUNK_RAYS * S               # 2048 cols per chunk
TILE_N = 512                      # matmul moving size
NTILE = CN // TILE_N              # 4 point-tiles per chunk

MAGIC = np.float32(12582912.0)    # 1.5 * 2^23 (round-to-int trick)
INV2PI = np.float32(1.0 / (2.0 * np.pi))
C1 = np.float32(6.28125)          # 2*pi split, k*C1 exact for k < 2^13
C2 = np.float32(2.0 * np.pi - 6.28125)

BUILD_STAGE = int(os.environ.get("KERNEL_STAGE", "3"))
DEBUG_OUT = os.environ.get("KERNEL_DEBUG", "0") == "1"


# ---------------------------------------------------------------- host prep
def _posenc_rows(nf, span=None, minp=None):
    """A3 [6*nf,3] / const [6*nf] for rows f-major: per f: 3 sin, 3 cos."""
    rows = 6 * nf
    A3 = np.zeros((rows, 3), np.float64)
    ph = np.zeros((rows,), np.float64)
    for f in range(nf):
        for k in range(6):
            r = 6 * f + k
            d = k % 3
            sc = 2.0 ** f
            if span is not None:
                A3[r, d] = sc / span[d]
                ph[r] = -sc * minp[d] / span[d]
            else:
                A3[r, d] = sc
            if k >= 3:
                ph[r] += np.pi / 2.0
    return A3, ph


def host_prep(inp):
    c = {}
    f32 = np.float32

    # coarse enc: per-ray rank-1 matrices  arg[i,(r,s)] = m*(B@d) + (C@[o;1])
    # cC4T [4,63]: cols 0:60 sin rows (A3s row + phase), cols 60:63 linear xyz
    A3s, phs = _posenc_rows(10)
    cC4 = np.zeros((4, 63), np.float64)
    cC4[0:3, 0:60] = A3s.T
    cC4[3, 0:60] = phs
    cC4[0:3, 60:63] = np.eye(3)
    c['cC4T'] = cC4.astype(f32).copy()                           # [4,63]

    # fine posenc rows: [sinx60, sinapp36, xyz3, appx3]
    minp = inp['min_point'].astype(np.float64)
    span = (inp['max_point'] - inp['min_point']).astype(np.float64)
    A3a, pha = _posenc_rows(6, span=span, minp=minp)
    pad4 = np.zeros((4, 3))
    fA3 = np.concatenate([A3s, pad4, A3a, np.eye(3), np.diag(1.0 / span)], 0)
    fph = np.concatenate([phs, np.zeros(4), pha, np.zeros(3), -minp / span], 0)
    c['fA3T'] = fA3.T.astype(f32).copy()                         # [3,106]
    c['fA4T'] = np.concatenate([fA3, fph[:, None]], 1).T.astype(f32).copy()

    # per-ray enc matrices (lhsT)
    Ad = np.zeros((24, 4), np.float64)
    for f in range(4):
        for k in range(6):
            r = 6 * f + k
            Ad[r, k % 3] = 2.0 ** f
            if k >= 3:
                Ad[r, 3] = np.pi / 2.0
    c['AdT'] = Ad.T.astype(f32).copy()                           # [4,24]
    At = np.zeros((12, 2), np.float64)
    for f in range(6):
        At[2 * f, 0] = 2.0 ** f
        At[2 * f + 1, 0] = 2.0 ** f
        At[2 * f + 1, 1] = np.pi / 2.0
    c['AtT'] = At.T.astype(f32).copy()                           # [2,12]

    perm63 = list(range(3, 63)) + [0, 1, 2]
    c['pW0my'] = np.ascontiguousarray(inp['pW0'][perm63])        # [63,128]
    c['pW1'] = inp['pW1'].copy()
    c['pW2'] = inp['pW2'].copy()
    c['pWo'] = inp['pWo'].copy()                                 # [128,1]
    c['pb0col'] = inp['pb0'].reshape(-1, 1).copy()
    c['pb1col'] = inp['pb1'].reshape(-1, 1).copy()
    c['pb2col'] = inp['pb2'].reshape(-1, 1).copy()

    c['fW0my'] = np.ascontiguousarray(inp['fW0'][perm63])        # [63,256]

    def pack_km(Wm):  # [256, 256] -> [128, 4, 128], slot 2k+m
        out = np.zeros((128, 4, 128), f32)
        for k in range(2):
            for m in range(2):
                out[:, 2 * k + m, :] = Wm[k * 128:(k + 1) * 128,
                                          m * 128:(m + 1) * 128]
        return out

    for i in range(3):
        c[f'fWm{i}'] = pack_km(inp['fWm'][i])
        c[f'fWp{i}'] = pack_km(inp['fWp'][i])
    c['fWs_h'] = pack_km(inp['fWs'][0:256])
    c['fWs_e'] = np.ascontiguousarray(inp['fWs'][256:][perm63])  # [63,256]
    c['fb0col'] = inp['fb0'].reshape(2, 128).T.copy()            # [128,2]
    for i in range(3):
        c[f'fbm{i}col'] = inp['fbm'][i].reshape(2, 128).T.copy()
        c[f'fbp{i}col'] = inp['fbp'][i].reshape(2, 128).T.copy()
    c['fbscol'] = inp['fbs'].reshape(2, 128).T.copy()

    # view head: fold Wfeat into Wview
    Wv = inp['Wview']
    Wv_d, Wv_emb, Wv_t, Wv_app = (Wv[256:283], Wv[283:331],
                                  Wv[331:344], Wv[344:383])
    Wfc = (inp['Wfeat'].astype(np.float64) @ Wv[0:256].astype(np.float64)
           ).astype(f32)
    out = np.zeros((128, 2, 128), f32)
    out[:, 0, :] = Wfc[0:128]
    out[:, 1, :] = Wfc[128:256]
    c['Wfc'] = out
    c['bveffcol'] = (inp['bfeat'].astype(np.float64)
                     @ Wv[0:256].astype(np.float64)
                     + inp['bview'].astype(np.float64)
                     ).astype(f32).reshape(-1, 1)
    perm39 = list(range(3, 39)) + [0, 1, 2]
    c['Wv_app'] = np.ascontiguousarray(Wv_app[perm39])           # [39,128]
    c['Wv_d_lin'] = np.ascontiguousarray(Wv_d[0:3])
    c['Wv_d_sin'] = np.ascontiguousarray(Wv_d[3:27])
    c['Wv_emb'] = np.ascontiguousarray(Wv_emb)
    c['Wv_t_lin'] = np.ascontiguousarray(Wv_t[0:1])
    c['Wv_t_sin'] = np.ascontiguousarray(Wv_t[1:13])
    c['Wsig'] = np.stack([inp['Wsig'][0:128, 0],
                          inp['Wsig'][128:256, 0]], 1).copy()    # [128,2]
    c['Wrgb'] = inp['Wrgb'].copy()                               # [128,3]
    c['brgbcol'] = inp['brgb'].reshape(-1, 1).copy()             # [3,1]
    c['brgbcol2'] = (0.5 * inp['brgb']).reshape(-1, 1).astype(f32)
    c['emb_table'] = inp['emb_table'].copy()

    c['sgrid'] = np.broadcast_to(
        np.arange(129, dtype=f32) / 128.0, (128, 129)).copy()
    c['identity'] = np.eye(128, dtype=f32)
    E = np.zeros((4, 512), f32)
    for rl in range(4):
        E[rl, rl * 128:(rl + 1) * 128] = 1.0
    c['Etile'] = E
    c['iotacol'] = np.arange(100, dtype=f32).reshape(-1, 1)
    scalars = dict(pbo_f=float(inp['pbo'][0]), bsig_f=float(inp['bsig'][0]))
    return c, scalars


INPUT_SHAPES = {
    'rays': (R, 12),
    'cC4T': (4, 63),
    'fA3T': (3, 106), 'fA4T': (4, 106),
    'AdT': (4, 24), 'AtT': (2, 12),
    'pW0my': (63, 128),
    'pW1': (128, 128), 'pW2': (128, 128), 'pWo': (128, 1),
    'pb0col': (128, 1), 'pb1col': (128, 1), 'pb2col': (128, 1),
    'fW0my': (63, 256), 'fWm0': (128, 4, 128), 'fWm1': (128, 4, 128),
    'fWm2': (128, 4, 128), 'fWp0': (128, 4, 128), 'fWp1': (128, 4, 128),
    'fWp2': (128, 4, 128), 'fWs_h': (128, 4, 128), 'fWs_e': (63, 256),
    'fb0col': (128, 2), 'fbm0col': (128, 2), 'fbm1col': (128, 2),
    'fbm2col': (128, 2), 'fbp0col': (128, 2), 'fbp1col': (128, 2),
    'fbp2col': (128, 2), 'fbscol': (128, 2),
    'Wfc': (128, 2, 128), 'bveffcol': (128, 1), 'Wv_app': (39, 128),
    'Wv_d_lin': (3, 128), 'Wv_d_sin': (24, 128), 'Wv_emb': (48, 128),
    'Wv_t_lin': (1, 128), 'Wv_t_sin': (12, 128),
    'Wsig': (128, 2), 'Wrgb': (128, 3), 'brgbcol': (3, 1),
    'brgbcol2': (3, 1),
    'emb_table': (100, 48),
    'sgrid': (128, 129), 'identity': (128, 128),
    'Etile': (4, 512), 'iotacol': (100, 1),
}
F32R_WEIGHTS = {'fW0my', 'fWm0', 'fWm1', 'fWm2', 'fWp0', 'fWp1', 'fWp2',
                'fWs_h', 'fWs_e', 'Wfc', 'Wv_app', 'Wv_d_lin', 'Wv_d_sin',
                'Wv_emb', 'Wv_t_lin', 'Wv_t_sin', 'Wsig', 'Wrgb',
                'emb_table', 'Etile'}


# ---------------------------------------------------------------- bass build
def build_nc(pbo_f, bsig_f, stage=3, debug=False):
    nc = bacc.Bacc("TRN2", target_bir_lowering=False)
    D = {k: nc.dram_tensor(k, list(v), F32, kind="ExternalInput")
         for k, v in INPUT_SHAPES.items()}
    OUT = nc.dram_tensor("rgb_out", [R, 3], F32, kind="ExternalOutput")
    dbg = {}
    if debug:
        for nm, shp in [("d_sigc", (R, S)), ("d_zf", (R, S + 1)),
                        ("d_wc", (R, S)), ("d_sigf", (R, S)),
                        ("d_wf", (R, S)), ("d_hvray", (128, R)),
                        ("d_ec", (63, CN)), ("d_efa", (63, CN)),
                        ("d_efb", (39, CN)), ("d_h1", (128, 2 * TILE_N))]:
            dbg[nm] = nc.dram_tensor(nm, list(shp), F32, kind="ExternalOutput")
    with tile.TileContext(nc) as tc:
        _body(nc, tc, D, OUT, dbg, pbo_f, bsig_f, stage, debug)
    nc.compile()
    return nc


def _body(nc, tc, D, OUT, dbg, pbo_f, bsig_f, stage, debug):
    from contextlib import ExitStack
    ctx = ExitStack()
    wpool = ctx.enter_context(tc.tile_pool(name="w", bufs=1))
    per = ctx.enter_context(tc.tile_pool(name="per", bufs=1))
    pp2 = ctx.enter_context(tc.tile_pool(name="pp2", bufs=2))
    big = ctx.enter_context(tc.tile_pool(name="big", bufs=2))
    hp = ctx.enter_context(tc.tile_pool(name="h", bufs=3))
    dram = ctx.enter_context(tc.tile_pool(name="dr", bufs=2, space="DRAM"))
    psA = ctx.enter_context(tc.tile_pool(name="psA", bufs=4, space="PSUM"))
    psS = ctx.enter_context(tc.tile_pool(name="psS", bufs=1, space="PSUM"))
    psR = ctx.enter_context(tc.tile_pool(name="psR", bufs=1, space="PSUM"))
    psC = ctx.enter_context(tc.tile_pool(name="psC", bufs=1, space="PSUM"))

    W = {}
    for k, t in D.items():
        if k == 'rays':
            continue
        dt = F32R if k in F32R_WEIGHTS else F32
        tl = wpool.tile(list(t.shape), dt, tag="w_" + k)
        nc.sync.dma_start(tl[:], t[:].bitcast(F32R) if dt == F32R else t[:])
        W[k] = tl
    rays = wpool.tile([R, 12], F32, tag="w_rays")
    nc.sync.dma_start(rays[:], D['rays'][:])
    ident = W['identity']

    # ---------------- phase 0: per-ray prep (ray-major layout)
    nearc = per.tile([R, 1], F32)
    nc.vector.tensor_scalar(nearc[:], rays[:, 6:7], 1e-8, None, op0=OP.max)
    spanc = per.tile([R, 1], F32)
    nc.vector.tensor_tensor(spanc[:], rays[:, 7:8], nearc[:], op=OP.subtract)

    dsq = per.tile([R, 3], F32)
    nc.vector.tensor_tensor(dsq[:], rays[:, 3:6], rays[:, 3:6], op=OP.mult)
    ssum = per.tile([R, 1], F32)
    nc.vector.reduce_sum(ssum[:], dsq[:], axis=mybir.AxisListType.X)
    norm = per.tile([R, 1], F32)
    nc.scalar.activation(norm[:], ssum[:], AF.Sqrt)
    for it in range(2):
        t1 = per.tile([R, 1], F32, tag="nwt")
        nc.vector.reciprocal(t1[:], norm[:])
        nc.vector.scalar_tensor_tensor(t1[:], ssum[:], 1.0, t1[:],
                                       op0=OP.mult, op1=OP.mult)
        nc.vector.tensor_tensor(t1[:], t1[:], norm[:], op=OP.add)
        nc.vector.tensor_scalar(norm[:], t1[:], 0.5, None, op0=OP.mult)
    invn = per.tile([R, 1], F32)
    nc.vector.reciprocal(invn[:], norm[:])

    # bundle: 0:3 oc, 3 ones | 4:7 dc | 8:11 o, 11 ones | 12:15 dir |
    #         16:19 viewdir, 19 ones | 20 t, 21 ones | 22 embid
    bundle = per.tile([R, 28], F32)
    nc.gpsimd.memset(bundle[:], 0.0)
    nc.vector.scalar_tensor_tensor(bundle[:, 0:3], rays[:, 3:6], nearc[:],
                                   rays[:, 0:3], op0=OP.mult, op1=OP.add)
    nc.vector.memset(bundle[:, 3:4], 1.0)
    nc.vector.tensor_scalar(bundle[:, 4:7], rays[:, 3:6], spanc[:], None,
                            op0=OP.mult)
    nc.vector.tensor_copy(bundle[:, 8:11], rays[:, 0:3])
    nc.vector.memset(bundle[:, 11:12], 1.0)
    nc.vector.tensor_copy(bundle[:, 12:15], rays[:, 3:6])
    nc.vector.tensor_scalar(bundle[:, 16:19], rays[:, 3:6], invn[:], None,
                            op0=OP.mult)
    nc.vector.memset(bundle[:, 19:20], 1.0)
    nc.vector.tensor_copy(bundle[:, 20:21], rays[:, 8:9])
    nc.vector.memset(bundle[:, 21:22], 1.0)
    nc.vector.tensor_copy(bundle[:, 22:23], rays[:, 9:10])

    def transp(col):
        p = psC.tile([4, 128], F32, tag="ptp")
        nc.tensor.transpose(p[:], bundle[:, col:col + 4], ident[:])
        sb = per.tile([4, 128], F32, tag="tp%d" % col)
        nc.scalar.copy(sb[:], p[:])
        return sb

    ocT = transp(0)      # [ocT;ones]
    dcT = transp(4)      # [dcT;..]
    oT = transp(8)       # [oT;ones]
    dirT = transp(12)
    vdT = transp(16)     # [viewdirT;ones]
    tT = transp(20)      # [t;ones;embid]
    eiT = transp(22)     # row0 = embid (base 0 for partition_broadcast)

    def mm_copy(lhsT, rhs, shape, nm, dst_dtype=F32):
        p = psC.tile(shape, F32, tag="pmc")
        nc.tensor.matmul(p[:], lhsT, rhs, start=True, stop=True)
        sb = per.tile(shape, dst_dtype, tag="mc_" + nm)
        nc.scalar.copy(sb[:], p[:])
        return sb

    Bf = mm_copy(W['fA3T'][:], dirT[0:3, :], [106, 128], "Bf")
    Cf = mm_copy(W['fA4T'][:], oT[:], [106, 128], "Cf")
    Bc = mm_copy(W['cC4T'][0:3, :], dirT[0:3, :], [63, 128], "Bc")
    Cc = mm_copy(W['cC4T'][:], oT[:], [63, 128], "Cc")

    def rangered_v(ap, shape, tag):
        sc = per.tile(shape, F32, tag=tag)
        nc.vector.tensor_scalar(sc[:], ap, float(INV2PI), float(MAGIC),
                                op0=OP.mult, op1=OP.add)
        nc.vector.tensor_scalar(sc[:], sc[:], float(MAGIC), None,
                                op0=OP.subtract)
        nc.vector.scalar_tensor_tensor(ap, sc[:], -float(C1), ap,
                                       op0=OP.mult, op1=OP.add)
        nc.vector.scalar_tensor_tensor(ap, sc[:], -float(C2), ap,
                                       op0=OP.mult, op1=OP.add)

    # per-ray view features
    argd = mm_copy(W['AdT'][:], vdT[:], [24, 128], 'argd')
    rangered_v(argd[:], [24, 128], "rrd")
    sind = per.tile([24, 128], F32R)
    nc.scalar.activation(sind[:], argd[:], AF.Sin)
    vd_r = per.tile([4, 128], F32R)
    nc.vector.tensor_copy(vd_r[:], vdT[:])

    argt = mm_copy(W['AtT'][:], tT[0:2, :], [12, 128], 'argt')
    rangered_v(argt[:], [12, 128], "rrt")
    sint = per.tile([12, 128], F32R)
    nc.scalar.activation(sint[:], argt[:], AF.Sin)
    t_r = per.tile([4, 128], F32R)
    nc.vector.tensor_copy(t_r[:], tT[:])

    embBC = per.tile([100, 128], F32)
    nc.gpsimd.partition_broadcast(embBC[:], eiT[0:1, :], channels=100)
    onehot = per.tile([100, 128], F32R)
    nc.vector.tensor_scalar(onehot[:], embBC[:], W['iotacol'][:], None,
                            op0=OP.is_equal)
    embT = mm_copy(W['emb_table'][:], onehot[:], [48, 128], 'embT', dst_dtype=F32R)

    phv = psC.tile([128, 128], F32, tag="pmc")
    nc.tensor.matmul(phv[:], W['Wv_d_lin'][:], vd_r[0:3, :],
                     start=True, stop=False)
    nc.tensor.matmul(phv[:], W['Wv_d_sin'][:], sind[:], start=False, stop=False)
    nc.tensor.matmul(phv[:], W['Wv_emb'][:], embT[:], start=False, stop=False)
    nc.tensor.matmul(phv[:], W['Wv_t_lin'][:], t_r[0:1, :],
                     start=False, stop=False)
    nc.tensor.matmul(phv[:], W['Wv_t_sin'][:], sint[:], start=False, stop=True)
    hvray = per.tile([128, 128], F32)
    nc.vector.tensor_scalar(hvray[:], phv[:], W['bveffcol'][:], None,
                            op0=OP.add)
    if debug:
        nc.sync.dma_start(dbg["d_hvray"][:], hvray[:])
    phvT = psC.tile([128, 128], F32, tag="pmc")
    nc.tensor.transpose(phvT[:], hvray[:], ident[:])
    hvrayT = per.tile([128, 128], F32R)
    nc.scalar.copy(hvrayT[:], phvT[:])
    hvb = dram.tile([128, 128], F32R, tag="hvb")
    nc.sync.dma_start(hvb[:], hvrayT[:])
    hvre = wpool.tile([4, 32, 128], F32R, tag="hvre")
    nc.sync.dma_start(hvre[:], hvb[:].rearrange("(t rl) m -> rl t m", rl=4))

    # coarse z edges
    zc = per.tile([R, S + 1], F32)
    nc.vector.tensor_scalar(zc[:], W['sgrid'][:], spanc[:], None, op0=OP.mult)
    nc.vector.tensor_scalar(zc[:], zc[:], nearc[:], None, op0=OP.add)
    midc = per.tile([R, S], F32)
    nc.vector.tensor_tensor(midc[:], zc[:, 0:S], zc[:, 1:S + 1], op=OP.add)
    nc.vector.tensor_scalar(midc[:], midc[:], 0.5, None, op0=OP.mult)

    # ======================= COARSE PASS =======================
    # midc bounce to DRAM once; per chunk DMA-replicate flat mids to 63 rows
    mc_dram = dram.tile([R, S], F32, tag="mcd", bufs=1)
    nc.scalar.dma_start(mc_dram[:], midc[:])
    sigcT = per.tile([R, S], F32, tag="sigcT")
    for ci in range(NCHUNK):
        r0 = ci * CHUNK_RAYS
        argc = big.tile([63, CN], F32, tag="arg")
        msrc = (mc_dram[r0:r0 + CHUNK_RAYS, :]
                .rearrange("p f -> (p f)").unsqueeze(0)
                .broadcast_to([63, CN]))
        nc.scalar.dma_start(argc[:], msrc)
        B3 = Bc[:, r0:r0 + CHUNK_RAYS].unsqueeze(2).broadcast_to(
            [63, CHUNK_RAYS, S])
        C3 = Cc[:, r0:r0 + CHUNK_RAYS].unsqueeze(2).broadcast_to(
            [63, CHUNK_RAYS, S])
        a3 = argc[:].rearrange("p (r s) -> p r s", r=CHUNK_RAYS)
        nc.vector.tensor_tensor(a3, a3, B3, op=OP.mult)
        nc.gpsimd.tensor_tensor(a3, a3, C3, op=OP.add)
        sc = big.tile([100, CN], F32, tag="mbcrr", bufs=2)
        nc.gpsimd.tensor_scalar(sc[0:60, :], argc[0:60, :], float(INV2PI),
                                float(MAGIC), op0=OP.mult, op1=OP.add)
        nc.gpsimd.tensor_scalar(sc[0:60, :], sc[0:60, :], float(MAGIC), None,
                                op0=OP.subtract)
        nc.vector.scalar_tensor_tensor(argc[0:60, :], sc[0:60, :], -float(C1),
                                       argc[0:60, :], op0=OP.mult, op1=OP.add)
        nc.vector.scalar_tensor_tensor(argc[0:60, :], sc[0:60, :], -float(C2),
                                       argc[0:60, :], op0=OP.mult, op1=OP.add)
        sb_ = dram.tile([1, CN], F32, tag="sigb")
        sigflat = pp2.tile([1, CN], F32, tag="sigflat", bufs=1)
        nc.scalar.activation(argc[0:60, :], argc[0:60, :], AF.Sin)
        # layer-major over tile pairs: PE works tile t+1 while relu(t) lands
        for tp in range(NTILE // 2):
            pair = (2 * tp, 2 * tp + 1)
            colsv = [slice(t * TILE_N, (t + 1) * TILE_N) for t in pair]
            hh = []
            for i, t in enumerate(pair):
                p1 = psA.tile([128, TILE_N], F32, tag="mmps")
                nc.tensor.matmul(p1[:], W['pW0my'][:], argc[:, colsv[i]],
                                 start=True, stop=True)
                h1 = hp.tile([128, TILE_N], F32, tag="ch", bufs=4)
                nc.scalar.activation(h1[:], p1[:], AF.Relu,
                                     bias=W['pb0col'][:])
                hh.append(h1)
            for i, t in enumerate(pair):
                p2 = psA.tile([128, TILE_N], F32, tag="mmps")
                nc.tensor.matmul(p2[:], W['pW1'][:], hh[i][:],
                                 start=True, stop=True)
                h2 = hp.tile([128, TILE_N], F32, tag="ch", bufs=4)
                if i == 0:
                    nc.vector.tensor_scalar(h2[:], p2[:], W['pb1col'][:], 0.0,
                                            op0=OP.add, op1=OP.max)
                else:
                    nc.scalar.activation(h2[:], p2[:], AF.Relu,
                                         bias=W['pb1col'][:])
                hh[i] = h2
            for i, t in enumerate(pair):
                p3 = psA.tile([128, TILE_N], F32, tag="mmps")
                nc.tensor.matmul(p3[:], W['pW2'][:], hh[i][:],
                                 start=True, stop=True)
                h3 = hp.tile([128, TILE_N], F32, tag="ch", bufs=4)
                if i == 0:
                    nc.scalar.activation(h3[:], p3[:], AF.Relu,
                                         bias=W['pb2col'][:])
                else:
                    nc.vector.tensor_scalar(h3[:], p3[:], W['pb2col'][:], 0.0,
                                            op0=OP.add, op1=OP.max)
                hh[i] = h3
            for i, t in enumerate(pair):
                ps_ = psS.tile([1, TILE_N], F32, tag="sigps")
                nc.tensor.matmul(ps_[:], W['pWo'][:], hh[i][:],
                                 start=True, stop=True)
                if t % 2 == 0:
                    nc.scalar.copy(sigflat[0:1, colsv[i]], ps_[:])
                else:
                    nc.vector.tensor_copy(sigflat[0:1, colsv[i]], ps_[:])
        nc.sync.dma_start(sb_[:], sigflat[:])
        nc.sync.dma_start(sigcT[r0:r0 + CHUNK_RAYS, :],
                          sb_[:].rearrange("a (p f) -> (a p) f", p=CHUNK_RAYS))

    if debug:
        nc.sync.dma_start(dbg["d_sigc"][:], sigcT[:])
    if stage < 2:
        ctx.close()
        return

    # ======================= raw2weights helper =======================
    def raw2w(sigT_ap, z_lo, z_hi, norm_ap, bias_f, nrows, tag):
        """w = alpha * exclusive-cumprod(1-alpha+1e-10); returns (w, dz)."""
        P = nrows
        dz = per.tile([P, S], F32, tag=tag + "dz")
        nc.vector.tensor_tensor(dz[:], z_hi, z_lo, op=OP.subtract)
        di = per.tile([P, S], F32, tag=tag + "di")
        nc.vector.tensor_scalar(di[:], dz[:], norm_ap, None, op0=OP.mult)
        s1 = per.tile([P, S], F32, tag=tag + "s1")
        nc.vector.tensor_scalar(s1[:], sigT_ap, bias_f, 0.0,
                                op0=OP.add, op1=OP.max)
        ea = per.tile([P, S], F32, tag=tag + "ea")
        nc.vector.tensor_tensor(ea[:], s1[:], di[:], op=OP.mult)
        e = per.tile([P, S], F32, tag=tag + "e")
        nc.scalar.activation(e[:], ea[:], AF.Exp, scale=-1.0)
        al = per.tile([P, S], F32, tag=tag + "al")
        nc.vector.tensor_scalar(al[:], e[:], -1.0, 1.0, op0=OP.mult, op1=OP.add)
        om = per.tile([P, S], F32, tag=tag + "om")
        nc.vector.tensor_scalar(om[:], e[:], 1e-10, None, op0=OP.add)
        tr = per.tile([P, S], F32, tag=tag + "tr")
        nc.vector.tensor_tensor_scan(tr[:], om[:], om[:], 1.0,
                                     op0=OP.mult, op1=OP.bypass)
        w = per.tile([P, S], F32, tag=tag + "w")
        nc.vector.tensor_copy(w[:, 0:1], al[:, 0:1])
        nc.vector.tensor_tensor(w[:, 1:S], al[:, 1:S], tr[:, 0:S - 1],
                                op=OP.mult)
        return w, dz

    zf = per.tile([R, S + 1], F32)
    wc, dzc = raw2w(sigcT[:], zc[:, 0:S], zc[:, 1:S + 1],
                    norm[:], pbo_f, R, "c")
    Wt = per.tile([R, S], F32, tag="Wt")
    nc.vector.tensor_scalar(Wt[:], wc[:], 1e-5, None, op0=OP.add)
    Sx = per.tile([R, S], F32, tag="Sx")
    nc.vector.memset(Sx[:, 0:1], 0.0)
    nc.vector.tensor_tensor_scan(Sx[:, 1:S], Wt[:, 0:S - 1],
                                 Wt[:, 0:S - 1], 0.0,
                                 op0=OP.add, op1=OP.bypass)
    Tt = per.tile([R, 1], F32, tag="Tt")
    nc.vector.tensor_tensor(Tt[:], Sx[:, S - 1:S], Wt[:, S - 1:S],
                            op=OP.add)
    P2 = per.tile([R, S], F32, tag="P2")
    nc.vector.reciprocal(P2[:], Wt[:])
    nc.vector.tensor_tensor(P2[:], P2[:], dzc[:], op=OP.mult)
    JB = 16
    Sx_b = Sx[:].unsqueeze(1).broadcast_to([R, JB, S])
    P2_b = P2[:].unsqueeze(1).broadcast_to([R, JB, S])
    dz_b = dzc[:].unsqueeze(1).broadcast_to([R, JB, S])
    for jb in range(0, S, JB):
        rs_ = pp2.tile([R, JB * S], F32, tag="rsx", name="rs_", bufs=1)
        x3 = rs_[:].rearrange("p (j s) -> p j s", j=JB)
        g_b = W['sgrid'][:, jb:jb + JB].unsqueeze(2).broadcast_to([R, JB, S])
        nc.vector.scalar_tensor_tensor(x3, g_b, Tt[:], Sx_b,
                                       op0=OP.mult, op1=OP.subtract)
        nc.vector.scalar_tensor_tensor(x3, x3, 0.0, P2_b,
                                       op0=OP.max, op1=OP.mult)
        nc.vector.tensor_tensor(x3, x3, dz_b, op=OP.min)
        nc.vector.tensor_reduce(zf[:, jb:jb + JB], x3,
                                axis=mybir.AxisListType.X, op=OP.add)
    # last edge j=S: all bins saturate -> sum(dz) == zc[:,S] - zc[:,0]
    nc.vector.tensor_tensor(zf[:, S:S + 1], zc[:, S:S + 1], zc[:, 0:1],
                            op=OP.subtract)
    nc.vector.tensor_scalar(zf[:], zf[:], zc[:, 0:1], None, op0=OP.add)
    if debug:
        nc.sync.dma_start(dbg["d_zf"][:], zf[:])
        nc.sync.dma_start(dbg["d_wc"][:], wc[:])
    if stage < 3:
        ctx.close()
        return

    midf = per.tile([R, S], F32)
    nc.vector.tensor_tensor(midf[:], zf[:, 0:S], zf[:, 1:S + 1], op=OP.add)
    nc.vector.tensor_scalar(midf[:], midf[:], 0.5, None, op0=OP.mult)

    # ======================= FINE PASS =======================
    rgbmT = per.tile([3, 128], F32)
    nc.vector.memset(rgbmT[:], 0.0)

    mf_dram = dram.tile([R, S], F32, tag="mfd", bufs=1)
    nc.scalar.dma_start(mf_dram[:], midf[:])
    for ci in range(NCHUNK):
        r0 = ci * CHUNK_RAYS
        argf = big.tile([106, CN], F32, tag="arg")
        msrc = (mf_dram[r0:r0 + CHUNK_RAYS, :]
                .rearrange("p f -> (p f)").unsqueeze(0)
                .broadcast_to([106, CN]))
        nc.scalar.dma_start(argf[:], msrc)
        b3 = Bf[:, r0:r0 + CHUNK_RAYS].unsqueeze(2).broadcast_to(
            [106, CHUNK_RAYS, S])
        c3 = Cf[:, r0:r0 + CHUNK_RAYS].unsqueeze(2).broadcast_to(
            [106, CHUNK_RAYS, S])
        a3 = argf[:].rearrange("p (r s) -> p r s", r=CHUNK_RAYS)
        nc.vector.tensor_tensor(a3, a3, b3, op=OP.mult)
        nc.gpsimd.tensor_tensor(a3, a3, c3, op=OP.add)
        sc = big.tile([100, CN], F32, tag="mbcrr", bufs=2)
        TWOPI = float(np.float32(2.0 * np.pi))
        for lo, hi in ((0, 60), (64, 100)):
            nc.gpsimd.tensor_scalar(sc[lo:hi, :], argf[lo:hi, :], float(INV2PI),
                                    float(MAGIC), op0=OP.mult, op1=OP.add)
            nc.gpsimd.tensor_scalar(sc[lo:hi, :], sc[lo:hi, :], float(MAGIC),
                                    None, op0=OP.subtract)
            nc.vector.scalar_tensor_tensor(argf[lo:hi, :], sc[lo:hi, :],
                                           -TWOPI, argf[lo:hi, :],
                                           op0=OP.mult, op1=OP.add)
        efa = big.tile([63, CN], F32R, tag="efa")
        efb = big.tile([39, CN], F32R, tag="efb")
        nc.scalar.activation(efa[0:60, :], argf[0:60, :], AF.Sin)
        nc.scalar.activation(efb[0:36, :], argf[64:100, :], AF.Sin)
        nc.sync.dma_start(efa[60:63, :], argf[100:103, :].bitcast(F32R))
        nc.sync.dma_start(efb[36:39, :], argf[103:106, :].bitcast(F32R))
        if debug and ci == 0:
            nc.sync.dma_start(dbg["d_efa"][:], efa[:].bitcast(F32))
            nc.sync.dma_start(dbg["d_efb"][:], efb[:].bitcast(F32))

        rgbS = big.tile([3, CN], F32, tag="rgbS")
        sb_ = dram.tile([1, CN], F32, tag="sigb")
        sigflat = pp2.tile([1, CN], F32, tag="sigflat", bufs=1)

        def relu2(pmm, bname, i):
            """bias+relu both halves; engines alternate per tile parity."""
            hout = hp.tile([128, 2 * TILE_N], F32R, tag="fh", bufs=4)
            if i == 0:
                nc.scalar.activation(hout[:, 0:TILE_N], pmm[0][:], AF.Relu,
                                     bias=W[bname][:, 0:1])
                nc.vector.tensor_scalar(hout[:, TILE_N:], pmm[1][:],
                                        W[bname][:, 1:2], 0.0,
                                        op0=OP.add, op1=OP.max)
            else:
                nc.vector.tensor_scalar(hout[:, 0:TILE_N], pmm[0][:],
                                        W[bname][:, 0:1], 0.0,
                                        op0=OP.add, op1=OP.max)
                nc.scalar.activation(hout[:, TILE_N:], pmm[1][:], AF.Relu,
                                     bias=W[bname][:, 1:2])
            return hout

        for tp in range(NTILE // 2):
            pair = (2 * tp, 2 * tp + 1)
            colsv = [slice(t * TILE_N, (t + 1) * TILE_N) for t in pair]
            hh = []
            for i, t in enumerate(pair):
                pm = [psA.tile([128, TILE_N], F32, tag="mmps",
                               name="pm%d" % _m) for _m in range(2)]
                for m in range(2):
                    nc.tensor.matmul(pm[m][:],
                                     W['fW0my'][:, m * 128:(m + 1) * 128],
                                     efa[:, colsv[i]], start=True, stop=True)
                hh.append(relu2(pm, 'fb0col', i))
            if debug and ci == 0:
                nc.sync.dma_start(dbg["d_h1"][:], hh[0][:].bitcast(F32))

            for wname, bname, skip in (
                    ('fWm0', 'fbm0col', False), ('fWm1', 'fbm1col', False),
                    ('fWm2', 'fbm2col', False), ('fWs_h', 'fbscol', True),
                    ('fWp0', 'fbp0col', False), ('fWp1', 'fbp1col', False),
                    ('fWp2', 'fbp2col', False)):
                for i, t in enumerate(pair):
                    hin = hh[i]
                    pmm = [psA.tile([128, TILE_N], F32, tag="mmps",
                                    name="pmm%d" % _m) for _m in range(2)]
                    for m in range(2):
                        nc.tensor.matmul(pmm[m][:], W[wname][:, m, :],
                                         hin[:, 0:TILE_N],
                                         start=True, stop=False)
                        nc.tensor.matmul(pmm[m][:], W[wname][:, 2 + m, :],
                                         hin[:, TILE_N:],
                                         start=False, stop=not skip)
                        if skip:
                            nc.tensor.matmul(
                                pmm[m][:],
                                W['fWs_e'][:, m * 128:(m + 1) * 128],
                                efa[:, colsv[i]], start=False, stop=True)
                    hh[i] = relu2(pmm, bname, i)

            for i, t in enumerate(pair):
                h = hh[i]
                gtile = ci * NTILE + t
                ps_ = psS.tile([1, TILE_N], F32, tag="sigps")
                nc.tensor.matmul(ps_[:], W['Wsig'][:, 0:1], h[:, 0:TILE_N],
                                 start=True, stop=False)
                nc.tensor.matmul(ps_[:], W['Wsig'][:, 1:2], h[:, TILE_N:],
                                 start=False, stop=True)
                if t % 2 == 0:
                    nc.scalar.copy(sigflat[0:1, colsv[i]], ps_[:])
                else:
                    nc.vector.tensor_copy(sigflat[0:1, colsv[i]], ps_[:])

                pv = psA.tile([128, TILE_N], F32, tag="mmps")
                nc.tensor.matmul(pv[:], W['Wfc'][:, 0, :], h[:, 0:TILE_N],
                                 start=True, stop=False)
                nc.tensor.matmul(pv[:], W['Wfc'][:, 1, :], h[:, TILE_N:],
                                 start=False, stop=False)
                nc.tensor.matmul(pv[:], W['Wv_app'][:], efb[:, colsv[i]],
                                 start=False, stop=False)
                nc.tensor.matmul(pv[:], hvre[:, gtile, :], W['Etile'][:],
                                 start=False, stop=True)
                hv = hp.tile([128, TILE_N], F32R, tag="fhv", bufs=2)
                nc.vector.tensor_scalar(hv[:], pv[:], 0.0, None, op0=OP.max)

                prgb = psR.tile([3, TILE_N], F32, tag="rgbps")
                nc.tensor.matmul(prgb[:], W['Wrgb'][:], hv[:],
                                 start=True, stop=True)
                nc.scalar.activation(rgbS[0:3, colsv[i]], prgb[:],
                                     AF.Sigmoid, bias=W['brgbcol'][:])

        nc.sync.dma_start(sb_[:], sigflat[:])
        sigch = pp2.tile([CHUNK_RAYS, S], F32, tag="sigch")
        nc.sync.dma_start(sigch[:],
                          sb_[:].rearrange("a (p f) -> (a p) f", p=CHUNK_RAYS))
        zfc = pp2.tile([CHUNK_RAYS, S + 1], F32, tag="zfc")
        nc.sync.dma_start(zfc[:], zf[r0:r0 + CHUNK_RAYS, :])
        normc = pp2.tile([CHUNK_RAYS, 1], F32, tag="normc")
        nc.sync.dma_start(normc[:], norm[r0:r0 + CHUNK_RAYS, :])

        wf, _dzf = raw2w(sigch[:], zfc[:, 0:S], zfc[:, 1:S + 1],
                         normc[:], bsig_f, CHUNK_RAYS, "f")
        if debug:
            nc.sync.dma_start(dbg["d_sigf"][r0:r0 + CHUNK_RAYS, :], sigch[:])
            nc.sync.dma_start(dbg["d_wf"][r0:r0 + CHUNK_RAYS, :], wf[:])

        wb = dram.tile([CHUNK_RAYS, S], F32, tag="wb")
        nc.sync.dma_start(wb[:], wf[:])
        wBC = pp2.tile([3, CN], F32, tag="wbc", bufs=1)
        nc.sync.dma_start(
            wBC[:],
            wb[:].rearrange("p f -> (p f)").unsqueeze(0).broadcast_to([3, CN]))
        nc.gpsimd.tensor_tensor(rgbS[0:3, :], rgbS[0:3, :], wBC[0:3, :],
                                op=OP.mult)
        nc.vector.tensor_reduce(
            rgbmT[0:3, r0:r0 + CHUNK_RAYS],
            rgbS[0:3, :].rearrange("p (r s) -> p r s", r=CHUNK_RAYS),
            axis=mybir.AxisListType.X, op=OP.add)

    # out: transpose [3,128] -> [128,3] via DRAM bounce
    rb = dram.tile([3, 128], F32, tag="rb")
    nc.sync.dma_start(rb[:], rgbmT[:])
    rgbout = per.tile([128, 3], F32)
    nc.sync.dma_start(rgbout[:], rb[:].rearrange("c r -> r c"))
    nc.sync.dma_start(OUT[:], rgbout[:])
    ctx.close()


# ---------------------------------------------------------------- entry
_CACHE = {}


def kernel(**inputs):
    inp = {k: np.asarray(v) for k, v in inputs.items()}
    consts, scal = host_prep(inp)
    key = (BUILD_STAGE, DEBUG_OUT, scal['pbo_f'], scal['bsig_f'])
    if key not in _CACHE:
        _CACHE[key] = build_nc(scal['pbo_f'], scal['bsig_f'],
                               stage=BUILD_STAGE, debug=DEBUG_OUT)
    nc = _CACHE[key]
    rays = np.asarray(inp['rays'], np.float32)
    in_maps = []
    for core in range(NCORES):
        m = {k: np.ascontiguousarray(v, dtype=np.float32)
             for k, v in consts.items()}
        m['rays'] = np.ascontiguousarray(rays[core * R:(core + 1) * R])
        in_maps.append(m)
    res = run_bass_kernel_spmd(nc, in_maps, core_ids=list(range(NCORES)))
    globals()['_LAST_RESULTS'] = res
    return np.concatenate([r['rgb_out'] for r in res.results], 0)



# revision 43
# speedup vs baseline: 1.0337x; 1.0176x over previous
"""NeRF-style render kernel for TRN2 (8 NeuronCores, data-parallel over rays).

Self-contained: hardcodes all shapes. Coarse proposal MLP runs in fp32
(resampling is precision-critical), fine MLP in float32r.
"""
import os
import sys

sys.path.insert(0, '/opt/trn_rl_repo')
import numpy as np
import concourse.bass as bass
import concourse.bacc as bacc
import concourse.tile as tile
import concourse.mybir as mybir
from concourse.bass_utils import run_bass_kernel_spmd

F32 = mybir.dt.float32
F32R = mybir.dt.float32r
AF = mybir.ActivationFunctionType
OP = mybir.AluOpType

NCORES = 8
R = 128          # rays per core
S = 128          # samples per pass
CHUNK_RAYS = 16  # rays per chunk
NCHUNK = R // CHUNK_RAYS          # 8
CN = CHUNK_RAYS * S               # 2048 cols per chunk
TILE_N = 512                      # matmul moving size
NTILE = CN // TILE_N              # 4 point-tiles per chunk

MAGIC = np.float32(12582912.0)    # 1.5 * 2^23 (round-to-int trick)
INV2PI = np.float32(1.0 / (2.0 * np.pi))
C1 = np.float32(6.28125)          # 2*pi split, k*C1 exact for k < 2^13
C2 = np.float32(2.0 * np.pi - 6.28125)

BUILD_STAGE = int(os.environ.get("KERNEL_STAGE", "3"))
DEBUG_OUT = os.environ.get("KERNEL_DEBUG", "0") == "1"


# ---------------------------------------------------------------- host prep
def _posenc_rows(nf, span=None, minp=None):
    """A3 [6*nf,3] / const [6*nf] for rows f-major: per f: 3 sin, 3 cos."""
    rows = 6 * nf
    A3 = np.zeros((rows, 3), np.float64)
    ph = np.zeros((rows,), np.float64)
    for f in range(nf):
        for k in range(6):
            r = 6 * f + k
            d = k % 3
            sc = 2.0 ** f
            if span is not None:
                A3[r, d] = sc / span[d]
                ph[r] = -sc * minp[d] / span[d]
            else:
                A3[r, d] = sc
            if k >= 3:
                ph[r] += np.pi / 2.0
    return A3, ph


def host_prep(inp):
    c = {}
    f32 = np.float32

    # coarse enc: per-ray rank-1 matrices  arg[i,(r,s)] = m*(B@d) + (C@[o;1])
    # cC4T [4,63]: cols 0:60 sin rows (A3s row + phase), cols 60:63 linear xyz
    A3s, phs = _posenc_rows(10)
    cC4 = np.zeros((4, 63), np.float64)
    cC4[0:3, 0:60] = A3s.T
    cC4[3, 0:60] = phs
    cC4[0:3, 60:63] = np.eye(3)
    c['cC4T'] = cC4.astype(f32).copy()                           # [4,63]

    # fine posenc rows: [sinx60, sinapp36, xyz3, appx3]
    minp = inp['min_point'].astype(np.float64)
    span = (inp['max_point'] - inp['min_point']).astype(np.float64)
    A3a, pha = _posenc_rows(6, span=span, minp=minp)
    pad4 = np.zeros((4, 3))
    fA3 = np.concatenate([A3s, pad4, A3a, np.eye(3), np.diag(1.0 / span)], 0)
    fph = np.concatenate([phs, np.zeros(4), pha, np.zeros(3), -minp / span], 0)
    c['fA3T'] = fA3.T.astype(f32).copy()                         # [3,106]
    c['fA4T'] = np.concatenate([fA3, fph[:, None]], 1).T.astype(f32).copy()

    # per-ray enc matrices (lhsT)
    Ad = np.zeros((24, 4), np.float64)
    for f in range(4):
        for k in range(6):
            r = 6 * f + k
            Ad[r, k % 3] = 2.0 ** f
            if k >= 3:
                Ad[r, 3] = np.pi / 2.0
    c['AdT'] = Ad.T.astype(f32).copy()                           # [4,24]
    At = np.zeros((12, 2), np.float64)
    for f in range(6):
        At[2 * f, 0] = 2.0 ** f
        At[2 * f + 1, 0] = 2.0 ** f
        At[2 * f + 1, 1] = np.pi / 2.0
    c['AtT'] = At.T.astype(f32).copy()                           # [2,12]

    perm63 = list(range(3, 63)) + [0, 1, 2]
    c['pW0my'] = np.ascontiguousarray(inp['pW0'][perm63])        # [63,128]
    c['pW1'] = inp['pW1'].copy()
    c['pW2'] = inp['pW2'].copy()
    c['pWo'] = inp['pWo'].copy()                                 # [128,1]
    c['pb0col'] = inp['pb0'].reshape(-1, 1).copy()
    c['pb1col'] = inp['pb1'].reshape(-1, 1).copy()
    c['pb2col'] = inp['pb2'].reshape(-1, 1).copy()

    c['fW0my'] = np.ascontiguousarray(inp['fW0'][perm63])        # [63,256]

    def pack_km(Wm):  # [256, 256] -> [128, 4, 128], slot 2k+m
        out = np.zeros((128, 4, 128), f32)
        for k in range(2):
            for m in range(2):
                out[:, 2 * k + m, :] = Wm[k * 128:(k + 1) * 128,
                                          m * 128:(m + 1) * 128]
        return out

    for i in range(3):
        c[f'fWm{i}'] = pack_km(inp['fWm'][i])
        c[f'fWp{i}'] = pack_km(inp['fWp'][i])
    c['fWs_h'] = pack_km(inp['fWs'][0:256])
    c['fWs_e'] = np.ascontiguousarray(inp['fWs'][256:][perm63])  # [63,256]
    c['fb0col'] = inp['fb0'].reshape(2, 128).T.copy()            # [128,2]
    for i in range(3):
        c[f'fbm{i}col'] = inp['fbm'][i].reshape(2, 128).T.copy()
        c[f'fbp{i}col'] = inp['fbp'][i].reshape(2, 128).T.copy()
    c['fbscol'] = inp['fbs'].reshape(2, 128).T.copy()

    # view head: fold Wfeat into Wview
    Wv = inp['Wview']
    Wv_d, Wv_emb, Wv_t, Wv_app = (Wv[256:283], Wv[283:331],
                                  Wv[331:344], Wv[344:383])
    Wfc = (inp['Wfeat'].astype(np.float64) @ Wv[0:256].astype(np.float64)
           ).astype(f32)
    out = np.zeros((128, 2, 128), f32)
    out[:, 0, :] = Wfc[0:128]
    out[:, 1, :] = Wfc[128:256]
    c['Wfc'] = out
    c['bveffcol'] = (inp['bfeat'].astype(np.float64)
                     @ Wv[0:256].astype(np.float64)
                     + inp['bview'].astype(np.float64)
                     ).astype(f32).reshape(-1, 1)
    perm39 = list(range(3, 39)) + [0, 1, 2]
    c['Wv_app'] = np.ascontiguousarray(Wv_app[perm39])           # [39,128]
    c['Wv_d_lin'] = np.ascontiguousarray(Wv_d[0:3])
    c['Wv_d_sin'] = np.ascontiguousarray(Wv_d[3:27])
    c['Wv_emb'] = np.ascontiguousarray(Wv_emb)
    c['Wv_t_lin'] = np.ascontiguousarray(Wv_t[0:1])
    c['Wv_t_sin'] = np.ascontiguousarray(Wv_t[1:13])
    c['Wsig'] = np.stack([inp['Wsig'][0:128, 0],
                          inp['Wsig'][128:256, 0]], 1).copy()    # [128,2]
    c['Wrgb'] = inp['Wrgb'].copy()                               # [128,3]
    c['brgbcol'] = inp['brgb'].reshape(-1, 1).copy()             # [3,1]
    c['brgbcol2'] = (0.5 * inp['brgb']).reshape(-1, 1).astype(f32)
    c['emb_table'] = inp['emb_table'].copy()

    c['sgrid'] = np.broadcast_to(
        np.arange(129, dtype=f32) / 128.0, (128, 129)).copy()
    c['identity'] = np.eye(128, dtype=f32)
    E = np.zeros((4, 512), f32)
    for rl in range(4):
        E[rl, rl * 128:(rl + 1) * 128] = 1.0
    c['Etile'] = E
    c['iotacol'] = np.arange(100, dtype=f32).reshape(-1, 1)
    scalars = dict(pbo_f=float(inp['pbo'][0]), bsig_f=float(inp['bsig'][0]))
    return c, scalars


INPUT_SHAPES = {
    'rays': (R, 12),
    'cC4T': (4, 63),
    'fA3T': (3, 106), 'fA4T': (4, 106),
    'AdT': (4, 24), 'AtT': (2, 12),
    'pW0my': (63, 128),
    'pW1': (128, 128), 'pW2': (128, 128), 'pWo': (128, 1),
    'pb0col': (128, 1), 'pb1col': (128, 1), 'pb2col': (128, 1),
    'fW0my': (63, 256), 'fWm0': (128, 4, 128), 'fWm1': (128, 4, 128),
    'fWm2': (128, 4, 128), 'fWp0': (128, 4, 128), 'fWp1': (128, 4, 128),
    'fWp2': (128, 4, 128), 'fWs_h': (128, 4, 128), 'fWs_e': (63, 256),
    'fb0col': (128, 2), 'fbm0col': (128, 2), 'fbm1col': (128, 2),
    'fbm2col': (128, 2), 'fbp0col': (128, 2), 'fbp1col': (128, 2),
    'fbp2col': (128, 2), 'fbscol': (128, 2),
    'Wfc': (128, 2, 128), 'bveffcol': (128, 1), 'Wv_app': (39, 128),
    'Wv_d_lin': (3, 128), 'Wv_d_sin': (24, 128), 'Wv_emb': (48, 128),
    'Wv_t_lin': (1, 128), 'Wv_t_sin': (12, 128),
    'Wsig': (128, 2), 'Wrgb': (128, 3), 'brgbcol': (3, 1),
    'brgbcol2': (3, 1),
    'emb_table': (100, 48),
    'sgrid': (128, 129), 'identity': (128, 128),
    'Etile': (4, 512), 'iotacol': (100, 1),
}
F32R_WEIGHTS = {'fW0my', 'fWm0', 'fWm1', 'fWm2', 'fWp0', 'fWp1', 'fWp2',
                'fWs_h', 'fWs_e', 'Wfc', 'Wv_app', 'Wv_d_lin', 'Wv_d_sin',
                'Wv_emb', 'Wv_t_lin', 'Wv_t_sin', 'Wsig', 'Wrgb',
                'emb_table', 'Etile'}


# ---------------------------------------------------------------- bass build
def build_nc(pbo_f, bsig_f, stage=3, debug=False):
    nc = bacc.Bacc("TRN2", target_bir_lowering=False)
    D = {k: nc.dram_tensor(k, list(v), F32, kind="ExternalInput")
         for k, v in INPUT_SHAPES.items()}
    OUT = nc.dram_tensor("rgb_out", [R, 3], F32, kind="ExternalOutput")
    dbg = {}
    if debug:
        for nm, shp in [("d_sigc", (R, S)), ("d_zf", (R, S + 1)),
                        ("d_wc", (R, S)), ("d_sigf", (R, S)),
                        ("d_wf", (R, S)), ("d_hvray", (128, R)),
                        ("d_ec", (63, CN)), ("d_efa", (63, CN)),
                        ("d_efb", (39, CN)), ("d_h1", (128, 2 * TILE_N))]:
            dbg[nm] = nc.dram_tensor(nm, list(shp), F32, kind="ExternalOutput")
    with tile.TileContext(nc) as tc:
        _body(nc, tc, D, OUT, dbg, pbo_f, bsig_f, stage, debug)
    nc.compile()
    return nc


def _body(nc, tc, D, OUT, dbg, pbo_f, bsig_f, stage, debug):
    from contextlib import ExitStack
    ctx = ExitStack()
    wpool = ctx.enter_context(tc.tile_pool(name="w", bufs=1))
    per = ctx.enter_context(tc.tile_pool(name="per", bufs=1))
    pp2 = ctx.enter_context(tc.tile_pool(name="pp2", bufs=2))
    big = ctx.enter_context(tc.tile_pool(name="big", bufs=2))
    hp = ctx.enter_context(tc.tile_pool(name="h", bufs=3))
    dram = ctx.enter_context(tc.tile_pool(name="dr", bufs=2, space="DRAM"))
    psA = ctx.enter_context(tc.tile_pool(name="psA", bufs=4, space="PSUM"))
    psS = ctx.enter_context(tc.tile_pool(name="psS", bufs=1, space="PSUM"))
    psR = ctx.enter_context(tc.tile_pool(name="psR", bufs=1, space="PSUM"))
    psC = ctx.enter_context(tc.tile_pool(name="psC", bufs=1, space="PSUM"))

    W = {}
    for k, t in D.items():
        if k == 'rays':
            continue
        dt = F32R if k in F32R_WEIGHTS else F32
        tl = wpool.tile(list(t.shape), dt, tag="w_" + k)
        nc.sync.dma_start(tl[:], t[:].bitcast(F32R) if dt == F32R else t[:])
        W[k] = tl
    rays = wpool.tile([R, 12], F32, tag="w_rays")
    nc.sync.dma_start(rays[:], D['rays'][:])
    ident = W['identity']

    # ---------------- phase 0: per-ray prep (ray-major layout)
    nearc = per.tile([R, 1], F32)
    nc.vector.tensor_scalar(nearc[:], rays[:, 6:7], 1e-8, None, op0=OP.max)
    spanc = per.tile([R, 1], F32)
    nc.vector.tensor_tensor(spanc[:], rays[:, 7:8], nearc[:], op=OP.subtract)

    dsq = per.tile([R, 3], F32)
    nc.vector.tensor_tensor(dsq[:], rays[:, 3:6], rays[:, 3:6], op=OP.mult)
    ssum = per.tile([R, 1], F32)
    nc.vector.reduce_sum(ssum[:], dsq[:], axis=mybir.AxisListType.X)
    norm = per.tile([R, 1], F32)
    nc.scalar.activation(norm[:], ssum[:], AF.Sqrt)
    for it in range(2):
        t1 = per.tile([R, 1], F32, tag="nwt")
        nc.vector.reciprocal(t1[:], norm[:])
        nc.vector.scalar_tensor_tensor(t1[:], ssum[:], 1.0, t1[:],
                                       op0=OP.mult, op1=OP.mult)
        nc.vector.tensor_tensor(t1[:], t1[:], norm[:], op=OP.add)
        nc.vector.tensor_scalar(norm[:], t1[:], 0.5, None, op0=OP.mult)
    invn = per.tile([R, 1], F32)
    nc.vector.reciprocal(invn[:], norm[:])

    # bundle: 0:3 oc, 3 ones | 4:7 dc | 8:11 o, 11 ones | 12:15 dir |
    #         16:19 viewdir, 19 ones | 20 t, 21 ones | 22 embid
    bundle = per.tile([R, 28], F32)
    nc.gpsimd.memset(bundle[:], 0.0)
    nc.vector.scalar_tensor_tensor(bundle[:, 0:3], rays[:, 3:6], nearc[:],
                                   rays[:, 0:3], op0=OP.mult, op1=OP.add)
    nc.vector.memset(bundle[:, 3:4], 1.0)
    nc.vector.tensor_scalar(bundle[:, 4:7], rays[:, 3:6], spanc[:], None,
                            op0=OP.mult)
    nc.vector.tensor_copy(bundle[:, 8:11], rays[:, 0:3])
    nc.vector.memset(bundle[:, 11:12], 1.0)
    nc.vector.tensor_copy(bundle[:, 12:15], rays[:, 3:6])
    nc.vector.tensor_scalar(bundle[:, 16:19], rays[:, 3:6], invn[:], None,
                            op0=OP.mult)
    nc.vector.memset(bundle[:, 19:20], 1.0)
    nc.vector.tensor_copy(bundle[:, 20:21], rays[:, 8:9])
    nc.vector.memset(bundle[:, 21:22], 1.0)
    nc.vector.tensor_copy(bundle[:, 22:23], rays[:, 9:10])

    def transp(col):
        p = psC.tile([4, 128], F32, tag="ptp")
        nc.tensor.transpose(p[:], bundle[:, col:col + 4], ident[:])
        sb = per.tile([4, 128], F32, tag="tp%d" % col)
        nc.scalar.copy(sb[:], p[:])
        return sb

    ocT = transp(0)      # [ocT;ones]
    dcT = transp(4)      # [dcT;..]
    oT = transp(8)       # [oT;ones]
    dirT = transp(12)
    vdT = transp(16)     # [viewdirT;ones]
    tT = transp(20)      # [t;ones;embid]
    eiT = transp(22)     # row0 = embid (base 0 for partition_broadcast)

    def mm_copy(lhsT, rhs, shape, nm, dst_dtype=F32):
        p = psC.tile(shape, F32, tag="pmc")
        nc.tensor.matmul(p[:], lhsT, rhs, start=True, stop=True)
        sb = per.tile(shape, dst_dtype, tag="mc_" + nm)
        nc.scalar.copy(sb[:], p[:])
        return sb

    Bf = mm_copy(W['fA3T'][:], dirT[0:3, :], [106, 128], "Bf")
    Cf = mm_copy(W['fA4T'][:], oT[:], [106, 128], "Cf")
    Bc = mm_copy(W['cC4T'][0:3, :], dirT[0:3, :], [63, 128], "Bc")
    Cc = mm_copy(W['cC4T'][:], oT[:], [63, 128], "Cc")

    def rangered_v(ap, shape, tag):
        sc = per.tile(shape, F32, tag=tag)
        nc.vector.tensor_scalar(sc[:], ap, float(INV2PI), float(MAGIC),
                                op0=OP.mult, op1=OP.add)
        nc.vector.tensor_scalar(sc[:], sc[:], float(MAGIC), None,
                                op0=OP.subtract)
        nc.vector.scalar_tensor_tensor(ap, sc[:], -float(C1), ap,
                                       op0=OP.mult, op1=OP.add)
        nc.vector.scalar_tensor_tensor(ap, sc[:], -float(C2), ap,
                                       op0=OP.mult, op1=OP.add)

    # per-ray view features
    argd = mm_copy(W['AdT'][:], vdT[:], [24, 128], 'argd')
    rangered_v(argd[:], [24, 128], "rrd")
    sind = per.tile([24, 128], F32R)
    nc.scalar.activation(sind[:], argd[:], AF.Sin)
    vd_r = per.tile([4, 128], F32R)
    nc.vector.tensor_copy(vd_r[:], vdT[:])

    argt = mm_copy(W['AtT'][:], tT[0:2, :], [12, 128], 'argt')
    rangered_v(argt[:], [12, 128], "rrt")
    sint = per.tile([12, 128], F32R)
    nc.scalar.activation(sint[:], argt[:], AF.Sin)
    t_r = per.tile([4, 128], F32R)
    nc.vector.tensor_copy(t_r[:], tT[:])

    embBC = per.tile([100, 128], F32)
    nc.gpsimd.partition_broadcast(embBC[:], eiT[0:1, :], channels=100)
    onehot = per.tile([100, 128], F32R)
    nc.vector.tensor_scalar(onehot[:], embBC[:], W['iotacol'][:], None,
                            op0=OP.is_equal)
    embT = mm_copy(W['emb_table'][:], onehot[:], [48, 128], 'embT', dst_dtype=F32R)

    phv = psC.tile([128, 128], F32, tag="pmc")
    nc.tensor.matmul(phv[:], W['Wv_d_lin'][:], vd_r[0:3, :],
                     start=True, stop=False)
    nc.tensor.matmul(phv[:], W['Wv_d_sin'][:], sind[:], start=False, stop=False)
    nc.tensor.matmul(phv[:], W['Wv_emb'][:], embT[:], start=False, stop=False)
    nc.tensor.matmul(phv[:], W['Wv_t_lin'][:], t_r[0:1, :],
                     start=False, stop=False)
    nc.tensor.matmul(phv[:], W['Wv_t_sin'][:], sint[:], start=False, stop=True)
    hvray = per.tile([128, 128], F32)
    nc.vector.tensor_scalar(hvray[:], phv[:], W['bveffcol'][:], None,
                            op0=OP.add)
    if debug:
        nc.sync.dma_start(dbg["d_hvray"][:], hvray[:])
    phvT = psC.tile([128, 128], F32, tag="pmc")
    nc.tensor.transpose(phvT[:], hvray[:], ident[:])
    hvrayT = per.tile([128, 128], F32R)
    nc.scalar.copy(hvrayT[:], phvT[:])
    hvb = dram.tile([128, 128], F32R, tag="hvb")
    nc.sync.dma_start(hvb[:], hvrayT[:])
    hvre = wpool.tile([4, 32, 128], F32R, tag="hvre")
    nc.sync.dma_start(hvre[:], hvb[:].rearrange("(t rl) m -> rl t m", rl=4))

    # coarse z edges
    zc = per.tile([R, S + 1], F32)
    nc.vector.tensor_scalar(zc[:], W['sgrid'][:], spanc[:], None, op0=OP.mult)
    nc.vector.tensor_scalar(zc[:], zc[:], nearc[:], None, op0=OP.add)
    midc = per.tile([R, S], F32)
    nc.vector.tensor_tensor(midc[:], zc[:, 0:S], zc[:, 1:S + 1], op=OP.add)
    nc.vector.tensor_scalar(midc[:], midc[:], 0.5, None, op0=OP.mult)

    # ======================= COARSE PASS =======================
    # midc bounce to DRAM once; per chunk DMA-replicate flat mids to 63 rows
    mc_dram = dram.tile([R, S], F32, tag="mcd", bufs=1)
    nc.scalar.dma_start(mc_dram[:], midc[:])
    sigcT = per.tile([R, S], F32, tag="sigcT")
    for ci in range(NCHUNK):
        r0 = ci * CHUNK_RAYS
        argc = big.tile([63, CN], F32, tag="arg")
        msrc = (mc_dram[r0:r0 + CHUNK_RAYS, :]
                .rearrange("p f -> (p f)").unsqueeze(0)
                .broadcast_to([63, CN]))
        nc.scalar.dma_start(argc[:], msrc)
        B3 = Bc[:, r0:r0 + CHUNK_RAYS].unsqueeze(2).broadcast_to(
            [63, CHUNK_RAYS, S])
        C3 = Cc[:, r0:r0 + CHUNK_RAYS].unsqueeze(2).broadcast_to(
            [63, CHUNK_RAYS, S])
        a3 = argc[:].rearrange("p (r s) -> p r s", r=CHUNK_RAYS)
        nc.vector.tensor_tensor(a3, a3, B3, op=OP.mult)
        nc.gpsimd.tensor_tensor(a3, a3, C3, op=OP.add)
        sc = big.tile([100, CN], F32, tag="mbcrr", bufs=1)
        nc.gpsimd.tensor_scalar(sc[0:60, :], argc[0:60, :], float(INV2PI),
                                float(MAGIC), op0=OP.mult, op1=OP.add)
        nc.gpsimd.tensor_scalar(sc[0:60, :], sc[0:60, :], float(MAGIC), None,
                                op0=OP.subtract)
        nc.vector.scalar_tensor_tensor(argc[0:60, :], sc[0:60, :], -float(C1),
                                       argc[0:60, :], op0=OP.mult, op1=OP.add)
        nc.vector.scalar_tensor_tensor(argc[0:60, :], sc[0:60, :], -float(C2),
                                       argc[0:60, :], op0=OP.mult, op1=OP.add)
        sb_ = dram.tile([1, CN], F32, tag="sigb")
        sigflat = pp2.tile([1, CN], F32, tag="sigflat", bufs=1)
        nc.scalar.activation(argc[0:60, :], argc[0:60, :], AF.Sin)
        # layer-major over tile pairs: PE works tile t+1 while relu(t) lands
        for tp in range(NTILE // 2):
            pair = (2 * tp, 2 * tp + 1)
            colsv = [slice(t * TILE_N, (t + 1) * TILE_N) for t in pair]
            hh = []
            for i, t in enumerate(pair):
                p1 = psA.tile([128, TILE_N], F32, tag="mmps")
                nc.tensor.matmul(p1[:], W['pW0my'][:], argc[:, colsv[i]],
                                 start=True, stop=True)
                h1 = hp.tile([128, TILE_N], F32, tag="ch", bufs=4)
                nc.scalar.activation(h1[:], p1[:], AF.Relu,
                                     bias=W['pb0col'][:])
                hh.append(h1)
            for i, t in enumerate(pair):
                p2 = psA.tile([128, TILE_N], F32, tag="mmps")
                nc.tensor.matmul(p2[:], W['pW1'][:], hh[i][:],
                                 start=True, stop=True)
                h2 = hp.tile([128, TILE_N], F32, tag="ch", bufs=4)
                if i == 0:
                    nc.vector.tensor_scalar(h2[:], p2[:], W['pb1col'][:], 0.0,
                                            op0=OP.add, op1=OP.max)
                else:
                    nc.scalar.activation(h2[:], p2[:], AF.Relu,
                                         bias=W['pb1col'][:])
                hh[i] = h2
            for i, t in enumerate(pair):
                p3 = psA.tile([128, TILE_N], F32, tag="mmps")
                nc.tensor.matmul(p3[:], W['pW2'][:], hh[i][:],
                                 start=True, stop=True)
                h3 = hp.tile([128, TILE_N], F32, tag="ch", bufs=4)
                if i == 0:
                    nc.scalar.activation(h3[:], p3[:], AF.Relu,
                                         bias=W['pb2col'][:])
                else:
                    nc.vector.tensor_scalar(h3[:], p3[:], W['pb2col'][:], 0.0,
                                            op0=OP.add, op1=OP.max)
                hh[i] = h3
            for i, t in enumerate(pair):
                ps_ = psS.tile([1, TILE_N], F32, tag="sigps")
                nc.tensor.matmul(ps_[:], W['pWo'][:], hh[i][:],
                                 start=True, stop=True)
                if t % 2 == 0:
                    nc.scalar.copy(sigflat[0:1, colsv[i]], ps_[:])
                else:
                    nc.vector.tensor_copy(sigflat[0:1, colsv[i]], ps_[:])
        nc.sync.dma_start(sb_[:], sigflat[:])
        nc.sync.dma_start(sigcT[r0:r0 + CHUNK_RAYS, :],
                          sb_[:].rearrange("a (p f) -> (a p) f", p=CHUNK_RAYS))

    if debug:
        nc.sync.dma_start(dbg["d_sigc"][:], sigcT[:])
    if stage < 2:
        ctx.close()
        return

    # ======================= raw2weights helper =======================
    def raw2w(sigT_ap, z_lo, z_hi, norm_ap, bias_f, nrows, tag):
        """w = alpha * exclusive-cumprod(1-alpha+1e-10); returns (w, dz)."""
        P = nrows
        dz = per.tile([P, S], F32, tag=tag + "dz")
        nc.vector.tensor_tensor(dz[:], z_hi, z_lo, op=OP.subtract)
        di = per.tile([P, S], F32, tag=tag + "di")
        nc.vector.tensor_scalar(di[:], dz[:], norm_ap, None, op0=OP.mult)
        s1 = per.tile([P, S], F32, tag=tag + "s1")
        nc.vector.tensor_scalar(s1[:], sigT_ap, bias_f, 0.0,
                                op0=OP.add, op1=OP.max)
        ea = per.tile([P, S], F32, tag=tag + "ea")
        nc.vector.tensor_tensor(ea[:], s1[:], di[:], op=OP.mult)
        e = per.tile([P, S], F32, tag=tag + "e")
        nc.scalar.activation(e[:], ea[:], AF.Exp, scale=-1.0)
        al = per.tile([P, S], F32, tag=tag + "al")
        nc.vector.tensor_scalar(al[:], e[:], -1.0, 1.0, op0=OP.mult, op1=OP.add)
        om = per.tile([P, S], F32, tag=tag + "om")
        nc.vector.tensor_scalar(om[:], e[:], 1e-10, None, op0=OP.add)
        tr = per.tile([P, S], F32, tag=tag + "tr")
        nc.vector.tensor_tensor_scan(tr[:], om[:], om[:], 1.0,
                                     op0=OP.mult, op1=OP.bypass)
        w = per.tile([P, S], F32, tag=tag + "w")
        nc.vector.tensor_copy(w[:, 0:1], al[:, 0:1])
        nc.vector.tensor_tensor(w[:, 1:S], al[:, 1:S], tr[:, 0:S - 1],
                                op=OP.mult)
        return w, dz

    zf = per.tile([R, S + 1], F32)
    wc, dzc = raw2w(sigcT[:], zc[:, 0:S], zc[:, 1:S + 1],
                    norm[:], pbo_f, R, "c")
    Wt = per.tile([R, S], F32, tag="Wt")
    nc.vector.tensor_scalar(Wt[:], wc[:], 1e-5, None, op0=OP.add)
    Sx = per.tile([R, S], F32, tag="Sx")
    nc.vector.memset(Sx[:, 0:1], 0.0)
    nc.vector.tensor_tensor_scan(Sx[:, 1:S], Wt[:, 0:S - 1],
                                 Wt[:, 0:S - 1], 0.0,
                                 op0=OP.add, op1=OP.bypass)
    Tt = per.tile([R, 1], F32, tag="Tt")
    nc.vector.tensor_tensor(Tt[:], Sx[:, S - 1:S], Wt[:, S - 1:S],
                            op=OP.add)
    P2 = per.tile([R, S], F32, tag="P2")
    nc.vector.reciprocal(P2[:], Wt[:])
    nc.vector.tensor_tensor(P2[:], P2[:], dzc[:], op=OP.mult)
    JB = 16
    Sx_b = Sx[:].unsqueeze(1).broadcast_to([R, JB, S])
    P2_b = P2[:].unsqueeze(1).broadcast_to([R, JB, S])
    dz_b = dzc[:].unsqueeze(1).broadcast_to([R, JB, S])
    for jb in range(0, S, JB):
        rs_ = pp2.tile([R, JB * S], F32, tag="rsx", name="rs_", bufs=1)
        x3 = rs_[:].rearrange("p (j s) -> p j s", j=JB)
        g_b = W['sgrid'][:, jb:jb + JB].unsqueeze(2).broadcast_to([R, JB, S])
        nc.vector.scalar_tensor_tensor(x3, g_b, Tt[:], Sx_b,
                                       op0=OP.mult, op1=OP.subtract)
        nc.vector.scalar_tensor_tensor(x3, x3, 0.0, P2_b,
                                       op0=OP.max, op1=OP.mult)
        nc.vector.tensor_tensor(x3, x3, dz_b, op=OP.min)
        nc.vector.tensor_reduce(zf[:, jb:jb + JB], x3,
                                axis=mybir.AxisListType.X, op=OP.add)
    # last edge j=S: all bins saturate -> sum(dz) == zc[:,S] - zc[:,0]
    nc.vector.tensor_tensor(zf[:, S:S + 1], zc[:, S:S + 1], zc[:, 0:1],
                            op=OP.subtract)
    nc.vector.tensor_scalar(zf[:], zf[:], zc[:, 0:1], None, op0=OP.add)
    if debug:
        nc.sync.dma_start(dbg["d_zf"][:], zf[:])
        nc.sync.dma_start(dbg["d_wc"][:], wc[:])
    if stage < 3:
        ctx.close()
        return

    midf = per.tile([R, S], F32)
    nc.vector.tensor_tensor(midf[:], zf[:, 0:S], zf[:, 1:S + 1], op=OP.add)
    nc.vector.tensor_scalar(midf[:], midf[:], 0.5, None, op0=OP.mult)

    # ======================= FINE PASS =======================
    rgbmT = per.tile([3, 128], F32)
    nc.vector.memset(rgbmT[:], 0.0)

    mf_dram = dram.tile([R, S], F32, tag="mfd", bufs=1)
    nc.scalar.dma_start(mf_dram[:], midf[:])
    for ci in range(NCHUNK):
        r0 = ci * CHUNK_RAYS
        argf = big.tile([106, CN], F32, tag="arg")
        msrc = (mf_dram[r0:r0 + CHUNK_RAYS, :]
                .rearrange("p f -> (p f)").unsqueeze(0)
                .broadcast_to([106, CN]))
        nc.scalar.dma_start(argf[:], msrc)
        b3 = Bf[:, r0:r0 + CHUNK_RAYS].unsqueeze(2).broadcast_to(
            [106, CHUNK_RAYS, S])
        c3 = Cf[:, r0:r0 + CHUNK_RAYS].unsqueeze(2).broadcast_to(
            [106, CHUNK_RAYS, S])
        a3 = argf[:].rearrange("p (r s) -> p r s", r=CHUNK_RAYS)
        nc.vector.tensor_tensor(a3, a3, b3, op=OP.mult)
        nc.gpsimd.tensor_tensor(a3, a3, c3, op=OP.add)
        sc = big.tile([100, CN], F32, tag="mbcrr", bufs=1)
        TWOPI = float(np.float32(2.0 * np.pi))
        for lo, hi in ((0, 60), (64, 100)):
            nc.gpsimd.tensor_scalar(sc[lo:hi, :], argf[lo:hi, :], float(INV2PI),
                                    float(MAGIC), op0=OP.mult, op1=OP.add)
            nc.gpsimd.tensor_scalar(sc[lo:hi, :], sc[lo:hi, :], float(MAGIC),
                                    None, op0=OP.subtract)
            nc.vector.scalar_tensor_tensor(argf[lo:hi, :], sc[lo:hi, :],
                                           -TWOPI, argf[lo:hi, :],
                                           op0=OP.mult, op1=OP.add)
        efa = big.tile([63, CN], F32R, tag="efa")
        efb = big.tile([39, CN], F32R, tag="efb")
        nc.scalar.activation(efa[0:60, :], argf[0:60, :], AF.Sin)
        nc.scalar.activation(efb[0:36, :], argf[64:100, :], AF.Sin)
        nc.sync.dma_start(efa[60:63, :], argf[100:103, :].bitcast(F32R))
        nc.sync.dma_start(efb[36:39, :], argf[103:106, :].bitcast(F32R))
        if debug and ci == 0:
            nc.sync.dma_start(dbg["d_efa"][:], efa[:].bitcast(F32))
            nc.sync.dma_start(dbg["d_efb"][:], efb[:].bitcast(F32))

        rgbS = big.tile([3, CN], F32, tag="rgbS")
        sb_ = dram.tile([1, CN], F32, tag="sigb")
        sigflat = pp2.tile([1, CN], F32, tag="sigflat", bufs=1)

        def relu2(pmm, bname, i):
            """bias+relu both halves; engines alternate per tile parity."""
            hout = hp.tile([128, 2 * TILE_N], F32R, tag="fh", bufs=4)
            if i == 0:
                nc.scalar.activation(hout[:, 0:TILE_N], pmm[0][:], AF.Relu,
                                     bias=W[bname][:, 0:1])
                nc.vector.tensor_scalar(hout[:, TILE_N:], pmm[1][:],
                                        W[bname][:, 1:2], 0.0,
                                        op0=OP.add, op1=OP.max)
            else:
                nc.vector.tensor_scalar(hout[:, 0:TILE_N], pmm[0][:],
                                        W[bname][:, 0:1], 0.0,
                                        op0=OP.add, op1=OP.max)
                nc.scalar.activation(hout[:, TILE_N:], pmm[1][:], AF.Relu,
                                     bias=W[bname][:, 1:2])
            return hout

        for tp in range(NTILE // 2):
            pair = (2 * tp, 2 * tp + 1)
            colsv = [slice(t * TILE_N, (t + 1) * TILE_N) for t in pair]
            hh = []
            for i, t in enumerate(pair):
                pm = [psA.tile([128, TILE_N], F32, tag="mmps",
                               name="pm%d" % _m) for _m in range(2)]
                for m in range(2):
                    nc.tensor.matmul(pm[m][:],
                                     W['fW0my'][:, m * 128:(m + 1) * 128],
                                     efa[:, colsv[i]], start=True, stop=True)
                hh.append(relu2(pm, 'fb0col', i))
            if debug and ci == 0:
                nc.sync.dma_start(dbg["d_h1"][:], hh[0][:].bitcast(F32))

            for wname, bname, skip in (
                    ('fWm0', 'fbm0col', False), ('fWm1', 'fbm1col', False),
                    ('fWm2', 'fbm2col', False), ('fWs_h', 'fbscol', True),
                    ('fWp0', 'fbp0col', False), ('fWp1', 'fbp1col', False),
                    ('fWp2', 'fbp2col', False)):
                for i, t in enumerate(pair):
                    hin = hh[i]
                    pmm = [psA.tile([128, TILE_N], F32, tag="mmps",
                                    name="pmm%d" % _m) for _m in range(2)]
                    for m in range(2):
                        nc.tensor.matmul(pmm[m][:], W[wname][:, m, :],
                                         hin[:, 0:TILE_N],
                                         start=True, stop=False)
                        nc.tensor.matmul(pmm[m][:], W[wname][:, 2 + m, :],
                                         hin[:, TILE_N:],
                                         start=False, stop=not skip)
                        if skip:
                            nc.tensor.matmul(
                                pmm[m][:],
                                W['fWs_e'][:, m * 128:(m + 1) * 128],
                                efa[:, colsv[i]], start=False, stop=True)
                    hh[i] = relu2(pmm, bname, i)

            for i, t in enumerate(pair):
                h = hh[i]
                gtile = ci * NTILE + t
                ps_ = psS.tile([1, TILE_N], F32, tag="sigps")
                nc.tensor.matmul(ps_[:], W['Wsig'][:, 0:1], h[:, 0:TILE_N],
                                 start=True, stop=False)
                nc.tensor.matmul(ps_[:], W['Wsig'][:, 1:2], h[:, TILE_N:],
                                 start=False, stop=True)
                if t % 2 == 0:
                    nc.scalar.copy(sigflat[0:1, colsv[i]], ps_[:])
                else:
                    nc.vector.tensor_copy(sigflat[0:1, colsv[i]], ps_[:])

                pv = psA.tile([128, TILE_N], F32, tag="mmps")
                nc.tensor.matmul(pv[:], W['Wfc'][:, 0, :], h[:, 0:TILE_N],
                                 start=True, stop=False)
                nc.tensor.matmul(pv[:], W['Wfc'][:, 1, :], h[:, TILE_N:],
                                 start=False, stop=False)
                nc.tensor.matmul(pv[:], W['Wv_app'][:], efb[:, colsv[i]],
                                 start=False, stop=False)
                nc.tensor.matmul(pv[:], hvre[:, gtile, :], W['Etile'][:],
                                 start=False, stop=True)
                hv = hp.tile([128, TILE_N], F32R, tag="fhv", bufs=2)
                nc.vector.tensor_scalar(hv[:], pv[:], 0.0, None, op0=OP.max)

                prgb = psR.tile([3, TILE_N], F32, tag="rgbps")
                nc.tensor.matmul(prgb[:], W['Wrgb'][:], hv[:],
                                 start=True, stop=True)
                nc.scalar.activation(rgbS[0:3, colsv[i]], prgb[:],
                                     AF.Sigmoid, bias=W['brgbcol'][:])

        nc.sync.dma_start(sb_[:], sigflat[:])
        sigch = pp2.tile([CHUNK_RAYS, S], F32, tag="sigch")
        nc.sync.dma_start(sigch[:],
                          sb_[:].rearrange("a (p f) -> (a p) f", p=CHUNK_RAYS))
        zfc = pp2.tile([CHUNK_RAYS, S + 1], F32, tag="zfc")
        nc.sync.dma_start(zfc[:], zf[r0:r0 + CHUNK_RAYS, :])
        normc = pp2.tile([CHUNK_RAYS, 1], F32, tag="normc")
        nc.sync.dma_start(normc[:], norm[r0:r0 + CHUNK_RAYS, :])

        wf, _dzf = raw2w(sigch[:], zfc[:, 0:S], zfc[:, 1:S + 1],
                         normc[:], bsig_f, CHUNK_RAYS, "f")
        if debug:
            nc.sync.dma_start(dbg["d_sigf"][r0:r0 + CHUNK_RAYS, :], sigch[:])
            nc.sync.dma_start(dbg["d_wf"][r0:r0 + CHUNK_RAYS, :], wf[:])

        wb = dram.tile([CHUNK_RAYS, S], F32, tag="wb")
        nc.sync.dma_start(wb[:], wf[:])
        wBC = pp2.tile([3, CN], F32, tag="wbc", bufs=1)
        nc.sync.dma_start(
            wBC[:],
            wb[:].rearrange("p f -> (p f)").unsqueeze(0).broadcast_to([3, CN]))
        nc.gpsimd.tensor_tensor(rgbS[0:3, :], rgbS[0:3, :], wBC[0:3, :],
                                op=OP.mult)
        nc.vector.tensor_reduce(
            rgbmT[0:3, r0:r0 + CHUNK_RAYS],
            rgbS[0:3, :].rearrange("p (r s) -> p r s", r=CHUNK_RAYS),
            axis=mybir.AxisListType.X, op=OP.add)

    # out: transpose [3,128] -> [128,3] via DRAM bounce
    rb = dram.tile([3, 128], F32, tag="rb")
    nc.sync.dma_start(rb[:], rgbmT[:])
    rgbout = per.tile([128, 3], F32)
    nc.sync.dma_start(rgbout[:], rb[:].rearrange("c r -> r c"))
    nc.sync.dma_start(OUT[:], rgbout[:])
    ctx.close()


# ---------------------------------------------------------------- entry
_CACHE = {}


def kernel(**inputs):
    inp = {k: np.asarray(v) for k, v in inputs.items()}
    consts, scal = host_prep(inp)
    key = (BUILD_STAGE, DEBUG_OUT, scal['pbo_f'], scal['bsig_f'])
    if key not in _CACHE:
        _CACHE[key] = build_nc(scal['pbo_f'], scal['bsig_f'],
                               stage=BUILD_STAGE, debug=DEBUG_OUT)
    nc = _CACHE[key]
    rays = np.asarray(inp['rays'], np.float32)
    in_maps = []
    for core in range(NCORES):
        m = {k: np.ascontiguousarray(v, dtype=np.float32)
             for k, v in consts.items()}
        m['rays'] = np.ascontiguousarray(rays[core * R:(core + 1) * R])
        in_maps.append(m)
    res = run_bass_kernel_spmd(nc, in_maps, core_ids=list(range(NCORES)))
    globals()['_LAST_RESULTS'] = res
    return np.concatenate([r['rgb_out'] for r in res.results], 0)



# revision 44
# speedup vs baseline: 1.0376x; 1.0038x over previous
"""NeRF-style render kernel for TRN2 (8 NeuronCores, data-parallel over rays).

Self-contained: hardcodes all shapes. Coarse proposal MLP runs in fp32
(resampling is precision-critical), fine MLP in float32r.
"""
import os
import sys

sys.path.insert(0, '/opt/trn_rl_repo')
import numpy as np
import concourse.bass as bass
import concourse.bacc as bacc
import concourse.tile as tile
import concourse.mybir as mybir
from concourse.bass_utils import run_bass_kernel_spmd

F32 = mybir.dt.float32
F32R = mybir.dt.float32r
AF = mybir.ActivationFunctionType
OP = mybir.AluOpType

NCORES = 8
R = 128          # rays per core
S = 128          # samples per pass
CHUNK_RAYS = 16  # rays per chunk
NCHUNK = R // CHUNK_RAYS          # 8
CN = CHUNK_RAYS * S               # 2048 cols per chunk
TILE_N = 512                      # matmul moving size
NTILE = CN // TILE_N              # 4 point-tiles per chunk

MAGIC = np.float32(12582912.0)    # 1.5 * 2^23 (round-to-int trick)
INV2PI = np.float32(1.0 / (2.0 * np.pi))
C1 = np.float32(6.28125)          # 2*pi split, k*C1 exact for k < 2^13
C2 = np.float32(2.0 * np.pi - 6.28125)

BUILD_STAGE = int(os.environ.get("KERNEL_STAGE", "3"))
DEBUG_OUT = os.environ.get("KERNEL_DEBUG", "0") == "1"


# ---------------------------------------------------------------- host prep
def _posenc_rows(nf, span=None, minp=None):
    """A3 [6*nf,3] / const [6*nf] for rows f-major: per f: 3 sin, 3 cos."""
    rows = 6 * nf
    A3 = np.zeros((rows, 3), np.float64)
    ph = np.zeros((rows,), np.float64)
    for f in range(nf):
        for k in range(6):
            r = 6 * f + k
            d = k % 3
            sc = 2.0 ** f
            if span is not None:
                A3[r, d] = sc / span[d]
                ph[r] = -sc * minp[d] / span[d]
            else:
                A3[r, d] = sc
            if k >= 3:
                ph[r] += np.pi / 2.0
    return A3, ph


def host_prep(inp):
    c = {}
    f32 = np.float32

    # coarse enc: per-ray rank-1 matrices  arg[i,(r,s)] = m*(B@d) + (C@[o;1])
    # cC4T [4,63]: cols 0:60 sin rows (A3s row + phase), cols 60:63 linear xyz
    A3s, phs = _posenc_rows(10)
    cC4 = np.zeros((4, 63), np.float64)
    cC4[0:3, 0:60] = A3s.T
    cC4[3, 0:60] = phs
    cC4[0:3, 60:63] = np.eye(3)
    c['cC4T'] = cC4.astype(f32).copy()                           # [4,63]

    # fine posenc rows: [sinx60, sinapp36, xyz3, appx3]
    minp = inp['min_point'].astype(np.float64)
    span = (inp['max_point'] - inp['min_point']).astype(np.float64)
    A3a, pha = _posenc_rows(6, span=span, minp=minp)
    pad4 = np.zeros((4, 3))
    fA3 = np.concatenate([A3s, pad4, A3a, np.eye(3), np.diag(1.0 / span)], 0)
    fph = np.concatenate([phs, np.zeros(4), pha, np.zeros(3), -minp / span], 0)
    c['fA3T'] = fA3.T.astype(f32).copy()                         # [3,106]
    c['fA4T'] = np.concatenate([fA3, fph[:, None]], 1).T.astype(f32).copy()

    # per-ray enc matrices (lhsT)
    Ad = np.zeros((24, 4), np.float64)
    for f in range(4):
        for k in range(6):
            r = 6 * f + k
            Ad[r, k % 3] = 2.0 ** f
            if k >= 3:
                Ad[r, 3] = np.pi / 2.0
    c['AdT'] = Ad.T.astype(f32).copy()                           # [4,24]
    At = np.zeros((12, 2), np.float64)
    for f in range(6):
        At[2 * f, 0] = 2.0 ** f
        At[2 * f + 1, 0] = 2.0 ** f
        At[2 * f + 1, 1] = np.pi / 2.0
    c['AtT'] = At.T.astype(f32).copy()                           # [2,12]

    perm63 = list(range(3, 63)) + [0, 1, 2]
    c['pW0my'] = np.ascontiguousarray(inp['pW0'][perm63])        # [63,128]
    c['pW1'] = inp['pW1'].copy()
    c['pW2'] = inp['pW2'].copy()
    c['pWo'] = inp['pWo'].copy()                                 # [128,1]
    c['pb0col'] = inp['pb0'].reshape(-1, 1).copy()
    c['pb1col'] = inp['pb1'].reshape(-1, 1).copy()
    c['pb2col'] = inp['pb2'].reshape(-1, 1).copy()

    c['fW0my'] = np.ascontiguousarray(inp['fW0'][perm63])        # [63,256]

    def pack_km(Wm):  # [256, 256] -> [128, 4, 128], slot 2k+m
        out = np.zeros((128, 4, 128), f32)
        for k in range(2):
            for m in range(2):
                out[:, 2 * k + m, :] = Wm[k * 128:(k + 1) * 128,
                                          m * 128:(m + 1) * 128]
        return out

    for i in range(3):
        c[f'fWm{i}'] = pack_km(inp['fWm'][i])
        c[f'fWp{i}'] = pack_km(inp['fWp'][i])
    c['fWs_h'] = pack_km(inp['fWs'][0:256])
    c['fWs_e'] = np.ascontiguousarray(inp['fWs'][256:][perm63])  # [63,256]
    c['fb0col'] = inp['fb0'].reshape(2, 128).T.copy()            # [128,2]
    for i in range(3):
        c[f'fbm{i}col'] = inp['fbm'][i].reshape(2, 128).T.copy()
        c[f'fbp{i}col'] = inp['fbp'][i].reshape(2, 128).T.copy()
    c['fbscol'] = inp['fbs'].reshape(2, 128).T.copy()

    # view head: fold Wfeat into Wview
    Wv = inp['Wview']
    Wv_d, Wv_emb, Wv_t, Wv_app = (Wv[256:283], Wv[283:331],
                                  Wv[331:344], Wv[344:383])
    Wfc = (inp['Wfeat'].astype(np.float64) @ Wv[0:256].astype(np.float64)
           ).astype(f32)
    out = np.zeros((128, 2, 128), f32)
    out[:, 0, :] = Wfc[0:128]
    out[:, 1, :] = Wfc[128:256]
    c['Wfc'] = out
    c['bveffcol'] = (inp['bfeat'].astype(np.float64)
                     @ Wv[0:256].astype(np.float64)
                     + inp['bview'].astype(np.float64)
                     ).astype(f32).reshape(-1, 1)
    perm39 = list(range(3, 39)) + [0, 1, 2]
    c['Wv_app'] = np.ascontiguousarray(Wv_app[perm39])           # [39,128]
    c['Wv_d_lin'] = np.ascontiguousarray(Wv_d[0:3])
    c['Wv_d_sin'] = np.ascontiguousarray(Wv_d[3:27])
    c['Wv_emb'] = np.ascontiguousarray(Wv_emb)
    c['Wv_t_lin'] = np.ascontiguousarray(Wv_t[0:1])
    c['Wv_t_sin'] = np.ascontiguousarray(Wv_t[1:13])
    c['Wsig'] = np.stack([inp['Wsig'][0:128, 0],
                          inp['Wsig'][128:256, 0]], 1).copy()    # [128,2]
    c['Wrgb'] = inp['Wrgb'].copy()                               # [128,3]
    c['brgbcol'] = inp['brgb'].reshape(-1, 1).copy()             # [3,1]
    c['brgbcol2'] = (0.5 * inp['brgb']).reshape(-1, 1).astype(f32)
    c['emb_table'] = inp['emb_table'].copy()

    c['sgrid'] = np.broadcast_to(
        np.arange(129, dtype=f32) / 128.0, (128, 129)).copy()
    c['identity'] = np.eye(128, dtype=f32)
    E = np.zeros((4, 512), f32)
    for rl in range(4):
        E[rl, rl * 128:(rl + 1) * 128] = 1.0
    c['Etile'] = E
    c['iotacol'] = np.arange(100, dtype=f32).reshape(-1, 1)
    scalars = dict(pbo_f=float(inp['pbo'][0]), bsig_f=float(inp['bsig'][0]))
    return c, scalars


INPUT_SHAPES = {
    'rays': (R, 12),
    'cC4T': (4, 63),
    'fA3T': (3, 106), 'fA4T': (4, 106),
    'AdT': (4, 24), 'AtT': (2, 12),
    'pW0my': (63, 128),
    'pW1': (128, 128), 'pW2': (128, 128), 'pWo': (128, 1),
    'pb0col': (128, 1), 'pb1col': (128, 1), 'pb2col': (128, 1),
    'fW0my': (63, 256), 'fWm0': (128, 4, 128), 'fWm1': (128, 4, 128),
    'fWm2': (128, 4, 128), 'fWp0': (128, 4, 128), 'fWp1': (128, 4, 128),
    'fWp2': (128, 4, 128), 'fWs_h': (128, 4, 128), 'fWs_e': (63, 256),
    'fb0col': (128, 2), 'fbm0col': (128, 2), 'fbm1col': (128, 2),
    'fbm2col': (128, 2), 'fbp0col': (128, 2), 'fbp1col': (128, 2),
    'fbp2col': (128, 2), 'fbscol': (128, 2),
    'Wfc': (128, 2, 128), 'bveffcol': (128, 1), 'Wv_app': (39, 128),
    'Wv_d_lin': (3, 128), 'Wv_d_sin': (24, 128), 'Wv_emb': (48, 128),
    'Wv_t_lin': (1, 128), 'Wv_t_sin': (12, 128),
    'Wsig': (128, 2), 'Wrgb': (128, 3), 'brgbcol': (3, 1),
    'brgbcol2': (3, 1),
    'emb_table': (100, 48),
    'sgrid': (128, 129), 'identity': (128, 128),
    'Etile': (4, 512), 'iotacol': (100, 1),
}
F32R_WEIGHTS = {'fW0my', 'fWm0', 'fWm1', 'fWm2', 'fWp0', 'fWp1', 'fWp2',
                'fWs_h', 'fWs_e', 'Wfc', 'Wv_app', 'Wv_d_lin', 'Wv_d_sin',
                'Wv_emb', 'Wv_t_lin', 'Wv_t_sin', 'Wsig', 'Wrgb',
                'emb_table', 'Etile'}


# ---------------------------------------------------------------- bass build
def build_nc(pbo_f, bsig_f, stage=3, debug=False):
    nc = bacc.Bacc("TRN2", target_bir_lowering=False)
    D = {k: nc.dram_tensor(k, list(v), F32, kind="ExternalInput")
         for k, v in INPUT_SHAPES.items()}
    OUT = nc.dram_tensor("rgb_out", [R, 3], F32, kind="ExternalOutput")
    dbg = {}
    if debug:
        for nm, shp in [("d_sigc", (R, S)), ("d_zf", (R, S + 1)),
                        ("d_wc", (R, S)), ("d_sigf", (R, S)),
                        ("d_wf", (R, S)), ("d_hvray", (128, R)),
                        ("d_ec", (63, CN)), ("d_efa", (63, CN)),
                        ("d_efb", (39, CN)), ("d_h1", (128, 2 * TILE_N))]:
            dbg[nm] = nc.dram_tensor(nm, list(shp), F32, kind="ExternalOutput")
    with tile.TileContext(nc) as tc:
        _body(nc, tc, D, OUT, dbg, pbo_f, bsig_f, stage, debug)
    nc.compile()
    return nc


def _body(nc, tc, D, OUT, dbg, pbo_f, bsig_f, stage, debug):
    from contextlib import ExitStack
    ctx = ExitStack()
    wpool = ctx.enter_context(tc.tile_pool(name="w", bufs=1))
    per = ctx.enter_context(tc.tile_pool(name="per", bufs=1))
    pp2 = ctx.enter_context(tc.tile_pool(name="pp2", bufs=2))
    big = ctx.enter_context(tc.tile_pool(name="big", bufs=2))
    hp = ctx.enter_context(tc.tile_pool(name="h", bufs=3))
    dram = ctx.enter_context(tc.tile_pool(name="dr", bufs=2, space="DRAM"))
    psA = ctx.enter_context(tc.tile_pool(name="psA", bufs=4, space="PSUM"))
    psS = ctx.enter_context(tc.tile_pool(name="psS", bufs=1, space="PSUM"))
    psR = ctx.enter_context(tc.tile_pool(name="psR", bufs=1, space="PSUM"))
    psC = ctx.enter_context(tc.tile_pool(name="psC", bufs=1, space="PSUM"))

    W = {}
    for k, t in D.items():
        if k == 'rays':
            continue
        dt = F32R if k in F32R_WEIGHTS else F32
        tl = wpool.tile(list(t.shape), dt, tag="w_" + k)
        nc.sync.dma_start(tl[:], t[:].bitcast(F32R) if dt == F32R else t[:])
        W[k] = tl
    rays = wpool.tile([R, 12], F32, tag="w_rays")
    nc.sync.dma_start(rays[:], D['rays'][:])
    ident = W['identity']

    # ---------------- phase 0: per-ray prep (ray-major layout)
    nearc = per.tile([R, 1], F32)
    nc.vector.tensor_scalar(nearc[:], rays[:, 6:7], 1e-8, None, op0=OP.max)
    spanc = per.tile([R, 1], F32)
    nc.vector.tensor_tensor(spanc[:], rays[:, 7:8], nearc[:], op=OP.subtract)

    dsq = per.tile([R, 3], F32)
    nc.vector.tensor_tensor(dsq[:], rays[:, 3:6], rays[:, 3:6], op=OP.mult)
    ssum = per.tile([R, 1], F32)
    nc.vector.reduce_sum(ssum[:], dsq[:], axis=mybir.AxisListType.X)
    norm = per.tile([R, 1], F32)
    nc.scalar.activation(norm[:], ssum[:], AF.Sqrt)
    for it in range(2):
        t1 = per.tile([R, 1], F32, tag="nwt")
        nc.vector.reciprocal(t1[:], norm[:])
        nc.vector.scalar_tensor_tensor(t1[:], ssum[:], 1.0, t1[:],
                                       op0=OP.mult, op1=OP.mult)
        nc.vector.tensor_tensor(t1[:], t1[:], norm[:], op=OP.add)
        nc.vector.tensor_scalar(norm[:], t1[:], 0.5, None, op0=OP.mult)
    invn = per.tile([R, 1], F32)
    nc.vector.reciprocal(invn[:], norm[:])

    # bundle: 0:3 oc, 3 ones | 4:7 dc | 8:11 o, 11 ones | 12:15 dir |
    #         16:19 viewdir, 19 ones | 20 t, 21 ones | 22 embid
    bundle = per.tile([R, 28], F32)
    nc.gpsimd.memset(bundle[:], 0.0)
    nc.vector.scalar_tensor_tensor(bundle[:, 0:3], rays[:, 3:6], nearc[:],
                                   rays[:, 0:3], op0=OP.mult, op1=OP.add)
    nc.vector.memset(bundle[:, 3:4], 1.0)
    nc.vector.tensor_scalar(bundle[:, 4:7], rays[:, 3:6], spanc[:], None,
                            op0=OP.mult)
    nc.vector.tensor_copy(bundle[:, 8:11], rays[:, 0:3])
    nc.vector.memset(bundle[:, 11:12], 1.0)
    nc.vector.tensor_copy(bundle[:, 12:15], rays[:, 3:6])
    nc.vector.tensor_scalar(bundle[:, 16:19], rays[:, 3:6], invn[:], None,
                            op0=OP.mult)
    nc.vector.memset(bundle[:, 19:20], 1.0)
    nc.vector.tensor_copy(bundle[:, 20:21], rays[:, 8:9])
    nc.vector.memset(bundle[:, 21:22], 1.0)
    nc.vector.tensor_copy(bundle[:, 22:23], rays[:, 9:10])

    def transp(col):
        p = psC.tile([4, 128], F32, tag="ptp")
        nc.tensor.transpose(p[:], bundle[:, col:col + 4], ident[:])
        sb = per.tile([4, 128], F32, tag="tp%d" % col)
        nc.scalar.copy(sb[:], p[:])
        return sb

    ocT = transp(0)      # [ocT;ones]
    dcT = transp(4)      # [dcT;..]
    oT = transp(8)       # [oT;ones]
    dirT = transp(12)
    vdT = transp(16)     # [viewdirT;ones]
    tT = transp(20)      # [t;ones;embid]
    eiT = transp(22)     # row0 = embid (base 0 for partition_broadcast)

    def mm_copy(lhsT, rhs, shape, nm, dst_dtype=F32):
        p = psC.tile(shape, F32, tag="pmc")
        nc.tensor.matmul(p[:], lhsT, rhs, start=True, stop=True)
        sb = per.tile(shape, dst_dtype, tag="mc_" + nm)
        nc.scalar.copy(sb[:], p[:])
        return sb

    Bf = mm_copy(W['fA3T'][:], dirT[0:3, :], [106, 128], "Bf")
    Cf = mm_copy(W['fA4T'][:], oT[:], [106, 128], "Cf")
    Bc = mm_copy(W['cC4T'][0:3, :], dirT[0:3, :], [63, 128], "Bc")
    Cc = mm_copy(W['cC4T'][:], oT[:], [63, 128], "Cc")

    def rangered_v(ap, shape, tag):
        sc = per.tile(shape, F32, tag=tag)
        nc.vector.tensor_scalar(sc[:], ap, float(INV2PI), float(MAGIC),
                                op0=OP.mult, op1=OP.add)
        nc.vector.tensor_scalar(sc[:], sc[:], float(MAGIC), None,
                                op0=OP.subtract)
        nc.vector.scalar_tensor_tensor(ap, sc[:], -float(C1), ap,
                                       op0=OP.mult, op1=OP.add)
        nc.vector.scalar_tensor_tensor(ap, sc[:], -float(C2), ap,
                                       op0=OP.mult, op1=OP.add)

    # per-ray view features
    argd = mm_copy(W['AdT'][:], vdT[:], [24, 128], 'argd')
    rangered_v(argd[:], [24, 128], "rrd")
    sind = per.tile([24, 128], F32R)
    nc.scalar.activation(sind[:], argd[:], AF.Sin)
    vd_r = per.tile([4, 128], F32R)
    nc.vector.tensor_copy(vd_r[:], vdT[:])

    argt = mm_copy(W['AtT'][:], tT[0:2, :], [12, 128], 'argt')
    rangered_v(argt[:], [12, 128], "rrt")
    sint = per.tile([12, 128], F32R)
    nc.scalar.activation(sint[:], argt[:], AF.Sin)
    t_r = per.tile([4, 128], F32R)
    nc.vector.tensor_copy(t_r[:], tT[:])

    embBC = per.tile([100, 128], F32)
    nc.gpsimd.partition_broadcast(embBC[:], eiT[0:1, :], channels=100)
    onehot = per.tile([100, 128], F32R)
    nc.vector.tensor_scalar(onehot[:], embBC[:], W['iotacol'][:], None,
                            op0=OP.is_equal)
    embT = mm_copy(W['emb_table'][:], onehot[:], [48, 128], 'embT', dst_dtype=F32R)

    phv = psC.tile([128, 128], F32, tag="pmc")
    nc.tensor.matmul(phv[:], W['Wv_d_lin'][:], vd_r[0:3, :],
                     start=True, stop=False)
    nc.tensor.matmul(phv[:], W['Wv_d_sin'][:], sind[:], start=False, stop=False)
    nc.tensor.matmul(phv[:], W['Wv_emb'][:], embT[:], start=False, stop=False)
    nc.tensor.matmul(phv[:], W['Wv_t_lin'][:], t_r[0:1, :],
                     start=False, stop=False)
    nc.tensor.matmul(phv[:], W['Wv_t_sin'][:], sint[:], start=False, stop=True)
    hvray = per.tile([128, 128], F32)
    nc.vector.tensor_scalar(hvray[:], phv[:], W['bveffcol'][:], None,
                            op0=OP.add)
    if debug:
        nc.sync.dma_start(dbg["d_hvray"][:], hvray[:])
    phvT = psC.tile([128, 128], F32, tag="pmc")
    nc.tensor.transpose(phvT[:], hvray[:], ident[:])
    hvrayT = per.tile([128, 128], F32R)
    nc.scalar.copy(hvrayT[:], phvT[:])
    hvb = dram.tile([128, 128], F32R, tag="hvb")
    nc.sync.dma_start(hvb[:], hvrayT[:])
    hvre = wpool.tile([4, 32, 128], F32R, tag="hvre")
    nc.sync.dma_start(hvre[:], hvb[:].rearrange("(t rl) m -> rl t m", rl=4))

    # coarse z edges
    zc = per.tile([R, S + 1], F32)
    nc.vector.tensor_scalar(zc[:], W['sgrid'][:], spanc[:], None, op0=OP.mult)
    nc.vector.tensor_scalar(zc[:], zc[:], nearc[:], None, op0=OP.add)
    midc = per.tile([R, S], F32)
    nc.vector.tensor_tensor(midc[:], zc[:, 0:S], zc[:, 1:S + 1], op=OP.add)
    nc.vector.tensor_scalar(midc[:], midc[:], 0.5, None, op0=OP.mult)

    # ======================= COARSE PASS =======================
    # midc bounce to DRAM once; per chunk DMA-replicate flat mids to 63 rows
    mc_dram = dram.tile([R, S], F32, tag="mcd", bufs=1)
    nc.scalar.dma_start(mc_dram[:], midc[:])
    sigcT = per.tile([R, S], F32, tag="sigcT")
    for ci in range(NCHUNK):
        r0 = ci * CHUNK_RAYS
        argc = big.tile([63, CN], F32, tag="arg")
        msrc = (mc_dram[r0:r0 + CHUNK_RAYS, :]
                .rearrange("p f -> (p f)").unsqueeze(0)
                .broadcast_to([63, CN]))
        nc.scalar.dma_start(argc[:], msrc)
        B3 = Bc[:, r0:r0 + CHUNK_RAYS].unsqueeze(2).broadcast_to(
            [63, CHUNK_RAYS, S])
        C3 = Cc[:, r0:r0 + CHUNK_RAYS].unsqueeze(2).broadcast_to(
            [63, CHUNK_RAYS, S])
        a3 = argc[:].rearrange("p (r s) -> p r s", r=CHUNK_RAYS)
        nc.vector.tensor_tensor(a3, a3, B3, op=OP.mult)
        nc.gpsimd.tensor_tensor(a3, a3, C3, op=OP.add)
        sc = big.tile([100, CN], F32, tag="mbcrr", bufs=1)
        nc.gpsimd.tensor_scalar(sc[0:60, :], argc[0:60, :], float(INV2PI),
                                float(MAGIC), op0=OP.mult, op1=OP.add)
        nc.gpsimd.tensor_scalar(sc[0:60, :], sc[0:60, :], float(MAGIC), None,
                                op0=OP.subtract)
        nc.vector.scalar_tensor_tensor(argc[0:60, :], sc[0:60, :], -float(C1),
                                       argc[0:60, :], op0=OP.mult, op1=OP.add)
        nc.vector.scalar_tensor_tensor(argc[0:60, :], sc[0:60, :], -float(C2),
                                       argc[0:60, :], op0=OP.mult, op1=OP.add)
        sb_ = dram.tile([1, CN], F32, tag="sigb")
        sigflat = pp2.tile([1, CN], F32, tag="sigflat", bufs=1)
        nc.scalar.activation(argc[0:60, :], argc[0:60, :], AF.Sin)
        # layer-major over tile pairs: PE works tile t+1 while relu(t) lands
        for tp in range(NTILE // 2):
            pair = (2 * tp, 2 * tp + 1)
            colsv = [slice(t * TILE_N, (t + 1) * TILE_N) for t in pair]
            hh = []
            for i, t in enumerate(pair):
                p1 = psA.tile([128, TILE_N], F32, tag="mmps")
                nc.tensor.matmul(p1[:], W['pW0my'][:], argc[:, colsv[i]],
                                 start=True, stop=True)
                h1 = hp.tile([128, TILE_N], F32, tag="ch", bufs=4)
                nc.scalar.activation(h1[:], p1[:], AF.Relu,
                                     bias=W['pb0col'][:])
                hh.append(h1)
            for i, t in enumerate(pair):
                p2 = psA.tile([128, TILE_N], F32, tag="mmps")
                nc.tensor.matmul(p2[:], W['pW1'][:], hh[i][:],
                                 start=True, stop=True)
                h2 = hp.tile([128, TILE_N], F32, tag="ch", bufs=4)
                if i == 0:
                    nc.vector.tensor_scalar(h2[:], p2[:], W['pb1col'][:], 0.0,
                                            op0=OP.add, op1=OP.max)
                else:
                    nc.scalar.activation(h2[:], p2[:], AF.Relu,
                                         bias=W['pb1col'][:])
                hh[i] = h2
            for i, t in enumerate(pair):
                p3 = psA.tile([128, TILE_N], F32, tag="mmps")
                nc.tensor.matmul(p3[:], W['pW2'][:], hh[i][:],
                                 start=True, stop=True)
                h3 = hp.tile([128, TILE_N], F32, tag="ch", bufs=4)
                if i == 0:
                    nc.scalar.activation(h3[:], p3[:], AF.Relu,
                                         bias=W['pb2col'][:])
                else:
                    nc.vector.tensor_scalar(h3[:], p3[:], W['pb2col'][:], 0.0,
                                            op0=OP.add, op1=OP.max)
                hh[i] = h3
            for i, t in enumerate(pair):
                ps_ = psS.tile([1, TILE_N], F32, tag="sigps")
                nc.tensor.matmul(ps_[:], W['pWo'][:], hh[i][:],
                                 start=True, stop=True)
                if t % 2 == 0:
                    nc.scalar.copy(sigflat[0:1, colsv[i]], ps_[:])
                else:
                    nc.vector.tensor_copy(sigflat[0:1, colsv[i]], ps_[:])
        nc.sync.dma_start(sb_[:], sigflat[:])
        nc.sync.dma_start(sigcT[r0:r0 + CHUNK_RAYS, :],
                          sb_[:].rearrange("a (p f) -> (a p) f", p=CHUNK_RAYS))

    if debug:
        nc.sync.dma_start(dbg["d_sigc"][:], sigcT[:])
    if stage < 2:
        ctx.close()
        return

    # ======================= raw2weights helper =======================
    def raw2w(sigT_ap, z_lo, z_hi, norm_ap, bias_f, nrows, tag):
        """w = alpha * exclusive-cumprod(1-alpha+1e-10); returns (w, dz)."""
        P = nrows
        dz = per.tile([P, S], F32, tag=tag + "dz")
        nc.vector.tensor_tensor(dz[:], z_hi, z_lo, op=OP.subtract)
        di = per.tile([P, S], F32, tag=tag + "di")
        nc.vector.tensor_scalar(di[:], dz[:], norm_ap, None, op0=OP.mult)
        s1 = per.tile([P, S], F32, tag=tag + "s1")
        nc.vector.tensor_scalar(s1[:], sigT_ap, bias_f, 0.0,
                                op0=OP.add, op1=OP.max)
        ea = per.tile([P, S], F32, tag=tag + "ea")
        nc.vector.tensor_tensor(ea[:], s1[:], di[:], op=OP.mult)
        e = per.tile([P, S], F32, tag=tag + "e")
        nc.scalar.activation(e[:], ea[:], AF.Exp, scale=-1.0)
        al = per.tile([P, S], F32, tag=tag + "al")
        nc.vector.tensor_scalar(al[:], e[:], -1.0, 1.0, op0=OP.mult, op1=OP.add)
        om = per.tile([P, S], F32, tag=tag + "om")
        nc.vector.tensor_scalar(om[:], e[:], 1e-10, None, op0=OP.add)
        tr = per.tile([P, S], F32, tag=tag + "tr")
        nc.vector.tensor_tensor_scan(tr[:], om[:], om[:], 1.0,
                                     op0=OP.mult, op1=OP.bypass)
        w = per.tile([P, S], F32, tag=tag + "w")
        nc.vector.tensor_copy(w[:, 0:1], al[:, 0:1])
        nc.vector.tensor_tensor(w[:, 1:S], al[:, 1:S], tr[:, 0:S - 1],
                                op=OP.mult)
        return w, dz

    zf = per.tile([R, S + 1], F32)
    wc, dzc = raw2w(sigcT[:], zc[:, 0:S], zc[:, 1:S + 1],
                    norm[:], pbo_f, R, "c")
    Wt = per.tile([R, S], F32, tag="Wt")
    nc.vector.tensor_scalar(Wt[:], wc[:], 1e-5, None, op0=OP.add)
    Sx = per.tile([R, S], F32, tag="Sx")
    nc.vector.memset(Sx[:, 0:1], 0.0)
    nc.vector.tensor_tensor_scan(Sx[:, 1:S], Wt[:, 0:S - 1],
                                 Wt[:, 0:S - 1], 0.0,
                                 op0=OP.add, op1=OP.bypass)
    Tt = per.tile([R, 1], F32, tag="Tt")
    nc.vector.tensor_tensor(Tt[:], Sx[:, S - 1:S], Wt[:, S - 1:S],
                            op=OP.add)
    P2 = per.tile([R, S], F32, tag="P2")
    nc.vector.reciprocal(P2[:], Wt[:])
    nc.vector.tensor_tensor(P2[:], P2[:], dzc[:], op=OP.mult)
    JB = 16
    Sx_b = Sx[:].unsqueeze(1).broadcast_to([R, JB, S])
    P2_b = P2[:].unsqueeze(1).broadcast_to([R, JB, S])
    dz_b = dzc[:].unsqueeze(1).broadcast_to([R, JB, S])
    for jb in range(0, S, JB):
        rs_ = pp2.tile([R, JB * S], F32, tag="rsx", name="rs_", bufs=2)
        x3 = rs_[:].rearrange("p (j s) -> p j s", j=JB)
        g_b = W['sgrid'][:, jb:jb + JB].unsqueeze(2).broadcast_to([R, JB, S])
        nc.vector.scalar_tensor_tensor(x3, g_b, Tt[:], Sx_b,
                                       op0=OP.mult, op1=OP.subtract)
        nc.vector.scalar_tensor_tensor(x3, x3, 0.0, P2_b,
                                       op0=OP.max, op1=OP.mult)
        nc.vector.tensor_tensor(x3, x3, dz_b, op=OP.min)
        nc.vector.tensor_reduce(zf[:, jb:jb + JB], x3,
                                axis=mybir.AxisListType.X, op=OP.add)
    # last edge j=S: all bins saturate -> sum(dz) == zc[:,S] - zc[:,0]
    nc.vector.tensor_tensor(zf[:, S:S + 1], zc[:, S:S + 1], zc[:, 0:1],
                            op=OP.subtract)
    nc.vector.tensor_scalar(zf[:], zf[:], zc[:, 0:1], None, op0=OP.add)
    if debug:
        nc.sync.dma_start(dbg["d_zf"][:], zf[:])
        nc.sync.dma_start(dbg["d_wc"][:], wc[:])
    if stage < 3:
        ctx.close()
        return

    midf = per.tile([R, S], F32)
    nc.vector.tensor_tensor(midf[:], zf[:, 0:S], zf[:, 1:S + 1], op=OP.add)
    nc.vector.tensor_scalar(midf[:], midf[:], 0.5, None, op0=OP.mult)

    # ======================= FINE PASS =======================
    rgbmT = per.tile([3, 128], F32)
    nc.vector.memset(rgbmT[:], 0.0)

    mf_dram = dram.tile([R, S], F32, tag="mfd", bufs=1)
    nc.scalar.dma_start(mf_dram[:], midf[:])
    for ci in range(NCHUNK):
        r0 = ci * CHUNK_RAYS
        argf = big.tile([106, CN], F32, tag="arg")
        msrc = (mf_dram[r0:r0 + CHUNK_RAYS, :]
                .rearrange("p f -> (p f)").unsqueeze(0)
                .broadcast_to([106, CN]))
        nc.scalar.dma_start(argf[:], msrc)
        b3 = Bf[:, r0:r0 + CHUNK_RAYS].unsqueeze(2).broadcast_to(
            [106, CHUNK_RAYS, S])
        c3 = Cf[:, r0:r0 + CHUNK_RAYS].unsqueeze(2).broadcast_to(
            [106, CHUNK_RAYS, S])
        a3 = argf[:].rearrange("p (r s) -> p r s", r=CHUNK_RAYS)
        nc.vector.tensor_tensor(a3, a3, b3, op=OP.mult)
        nc.gpsimd.tensor_tensor(a3, a3, c3, op=OP.add)
        sc = big.tile([100, CN], F32, tag="mbcrr", bufs=1)
        TWOPI = float(np.float32(2.0 * np.pi))
        for lo, hi in ((0, 60), (64, 100)):
            nc.gpsimd.tensor_scalar(sc[lo:hi, :], argf[lo:hi, :], float(INV2PI),
                                    float(MAGIC), op0=OP.mult, op1=OP.add)
            nc.gpsimd.tensor_scalar(sc[lo:hi, :], sc[lo:hi, :], float(MAGIC),
                                    None, op0=OP.subtract)
            nc.vector.scalar_tensor_tensor(argf[lo:hi, :], sc[lo:hi, :],
                                           -TWOPI, argf[lo:hi, :],
                                           op0=OP.mult, op1=OP.add)
        efa = big.tile([63, CN], F32R, tag="efa")
        efb = big.tile([39, CN], F32R, tag="efb")
        nc.scalar.activation(efa[0:60, :], argf[0:60, :], AF.Sin)
        nc.scalar.activation(efb[0:36, :], argf[64:100, :], AF.Sin)
        nc.sync.dma_start(efa[60:63, :], argf[100:103, :].bitcast(F32R))
        nc.sync.dma_start(efb[36:39, :], argf[103:106, :].bitcast(F32R))
        if debug and ci == 0:
            nc.sync.dma_start(dbg["d_efa"][:], efa[:].bitcast(F32))
            nc.sync.dma_start(dbg["d_efb"][:], efb[:].bitcast(F32))

        rgbS = big.tile([3, CN], F32, tag="rgbS")
        sb_ = dram.tile([1, CN], F32, tag="sigb")
        sigflat = pp2.tile([1, CN], F32, tag="sigflat", bufs=1)

        def relu2(pmm, bname, i):
            """bias+relu both halves; engines alternate per tile parity."""
            hout = hp.tile([128, 2 * TILE_N], F32R, tag="fh", bufs=4)
            if i == 0:
                nc.scalar.activation(hout[:, 0:TILE_N], pmm[0][:], AF.Relu,
                                     bias=W[bname][:, 0:1])
                nc.vector.tensor_scalar(hout[:, TILE_N:], pmm[1][:],
                                        W[bname][:, 1:2], 0.0,
                                        op0=OP.add, op1=OP.max)
            else:
                nc.vector.tensor_scalar(hout[:, 0:TILE_N], pmm[0][:],
                                        W[bname][:, 0:1], 0.0,
                                        op0=OP.add, op1=OP.max)
                nc.scalar.activation(hout[:, TILE_N:], pmm[1][:], AF.Relu,
                                     bias=W[bname][:, 1:2])
            return hout

        for tp in range(NTILE // 2):
            pair = (2 * tp, 2 * tp + 1)
            colsv = [slice(t * TILE_N, (t + 1) * TILE_N) for t in pair]
            hh = []
            for i, t in enumerate(pair):
                pm = [psA.tile([128, TILE_N], F32, tag="mmps",
                               name="pm%d" % _m) for _m in range(2)]
                for m in range(2):
                    nc.tensor.matmul(pm[m][:],
                                     W['fW0my'][:, m * 128:(m + 1) * 128],
                                     efa[:, colsv[i]], start=True, stop=True)
                hh.append(relu2(pm, 'fb0col', i))
            if debug and ci == 0:
                nc.sync.dma_start(dbg["d_h1"][:], hh[0][:].bitcast(F32))

            for wname, bname, skip in (
                    ('fWm0', 'fbm0col', False), ('fWm1', 'fbm1col', False),
                    ('fWm2', 'fbm2col', False), ('fWs_h', 'fbscol', True),
                    ('fWp0', 'fbp0col', False), ('fWp1', 'fbp1col', False),
                    ('fWp2', 'fbp2col', False)):
                for i, t in enumerate(pair):
                    hin = hh[i]
                    pmm = [psA.tile([128, TILE_N], F32, tag="mmps",
                                    name="pmm%d" % _m) for _m in range(2)]
                    for m in range(2):
                        nc.tensor.matmul(pmm[m][:], W[wname][:, m, :],
                                         hin[:, 0:TILE_N],
                                         start=True, stop=False)
                        nc.tensor.matmul(pmm[m][:], W[wname][:, 2 + m, :],
                                         hin[:, TILE_N:],
                                         start=False, stop=not skip)
                        if skip:
                            nc.tensor.matmul(
                                pmm[m][:],
                                W['fWs_e'][:, m * 128:(m + 1) * 128],
                                efa[:, colsv[i]], start=False, stop=True)
                    hh[i] = relu2(pmm, bname, i)

            for i, t in enumerate(pair):
                h = hh[i]
                gtile = ci * NTILE + t
                ps_ = psS.tile([1, TILE_N], F32, tag="sigps")
                nc.tensor.matmul(ps_[:], W['Wsig'][:, 0:1], h[:, 0:TILE_N],
                                 start=True, stop=False)
                nc.tensor.matmul(ps_[:], W['Wsig'][:, 1:2], h[:, TILE_N:],
                                 start=False, stop=True)
                if t % 2 == 0:
                    nc.scalar.copy(sigflat[0:1, colsv[i]], ps_[:])
                else:
                    nc.vector.tensor_copy(sigflat[0:1, colsv[i]], ps_[:])

                pv = psA.tile([128, TILE_N], F32, tag="mmps")
                nc.tensor.matmul(pv[:], W['Wfc'][:, 0, :], h[:, 0:TILE_N],
                                 start=True, stop=False)
                nc.tensor.matmul(pv[:], W['Wfc'][:, 1, :], h[:, TILE_N:],
                                 start=False, stop=False)
                nc.tensor.matmul(pv[:], W['Wv_app'][:], efb[:, colsv[i]],
                                 start=False, stop=False)
                nc.tensor.matmul(pv[:], hvre[:, gtile, :], W['Etile'][:],
                                 start=False, stop=True)
                hv = hp.tile([128, TILE_N], F32R, tag="fhv", bufs=2)
                nc.vector.tensor_scalar(hv[:], pv[:], 0.0, None, op0=OP.max)

                prgb = psR.tile([3, TILE_N], F32, tag="rgbps")
                nc.tensor.matmul(prgb[:], W['Wrgb'][:], hv[:],
                                 start=True, stop=True)
                nc.scalar.activation(rgbS[0:3, colsv[i]], prgb[:],
                                     AF.Sigmoid, bias=W['brgbcol'][:])

        nc.sync.dma_start(sb_[:], sigflat[:])
        sigch = pp2.tile([CHUNK_RAYS, S], F32, tag="sigch")
        nc.sync.dma_start(sigch[:],
                          sb_[:].rearrange("a (p f) -> (a p) f", p=CHUNK_RAYS))
        zfc = pp2.tile([CHUNK_RAYS, S + 1], F32, tag="zfc")
        nc.sync.dma_start(zfc[:], zf[r0:r0 + CHUNK_RAYS, :])
        normc = pp2.tile([CHUNK_RAYS, 1], F32, tag="normc")
        nc.sync.dma_start(normc[:], norm[r0:r0 + CHUNK_RAYS, :])

        wf, _dzf = raw2w(sigch[:], zfc[:, 0:S], zfc[:, 1:S + 1],
                         normc[:], bsig_f, CHUNK_RAYS, "f")
        if debug:
            nc.sync.dma_start(dbg["d_sigf"][r0:r0 + CHUNK_RAYS, :], sigch[:])
            nc.sync.dma_start(dbg["d_wf"][r0:r0 + CHUNK_RAYS, :], wf[:])

        wb = dram.tile([CHUNK_RAYS, S], F32, tag="wb")
        nc.sync.dma_start(wb[:], wf[:])
        wBC = pp2.tile([3, CN], F32, tag="wbc", bufs=1)
        nc.sync.dma_start(
            wBC[:],
            wb[:].rearrange("p f -> (p f)").unsqueeze(0).broadcast_to([3, CN]))
        nc.gpsimd.tensor_tensor(rgbS[0:3, :], rgbS[0:3, :], wBC[0:3, :],
                                op=OP.mult)
        nc.vector.tensor_reduce(
            rgbmT[0:3, r0:r0 + CHUNK_RAYS],
            rgbS[0:3, :].rearrange("p (r s) -> p r s", r=CHUNK_RAYS),
            axis=mybir.AxisListType.X, op=OP.add)

    # out: transpose [3,128] -> [128,3] via DRAM bounce
    rb = dram.tile([3, 128], F32, tag="rb")
    nc.sync.dma_start(rb[:], rgbmT[:])
    rgbout = per.tile([128, 3], F32)
    nc.sync.dma_start(rgbout[:], rb[:].rearrange("c r -> r c"))
    nc.sync.dma_start(OUT[:], rgbout[:])
    ctx.close()


# ---------------------------------------------------------------- entry
_CACHE = {}


def kernel(**inputs):
    inp = {k: np.asarray(v) for k, v in inputs.items()}
    consts, scal = host_prep(inp)
    key = (BUILD_STAGE, DEBUG_OUT, scal['pbo_f'], scal['bsig_f'])
    if key not in _CACHE:
        _CACHE[key] = build_nc(scal['pbo_f'], scal['bsig_f'],
                               stage=BUILD_STAGE, debug=DEBUG_OUT)
    nc = _CACHE[key]
    rays = np.asarray(inp['rays'], np.float32)
    in_maps = []
    for core in range(NCORES):
        m = {k: np.ascontiguousarray(v, dtype=np.float32)
             for k, v in consts.items()}
        m['rays'] = np.ascontiguousarray(rays[core * R:(core + 1) * R])
        in_maps.append(m)
    res = run_bass_kernel_spmd(nc, in_maps, core_ids=list(range(NCORES)))
    globals()['_LAST_RESULTS'] = res
    return np.concatenate([r['rgb_out'] for r in res.results], 0)



# revision 46
# speedup vs baseline: 1.0688x; 1.0301x over previous
"""NeRF-style render kernel for TRN2 (8 NeuronCores, data-parallel over rays).

Self-contained: hardcodes all shapes. Coarse proposal MLP runs in fp32
(resampling is precision-critical), fine MLP in float32r.
"""
import os
import sys

sys.path.insert(0, '/opt/trn_rl_repo')
import numpy as np
import concourse.bass as bass
import concourse.bacc as bacc
import concourse.tile as tile
import concourse.mybir as mybir
from concourse.bass_utils import run_bass_kernel_spmd

F32 = mybir.dt.float32
F32R = mybir.dt.float32r
AF = mybir.ActivationFunctionType
OP = mybir.AluOpType

NCORES = 8
R = 128          # rays per core
S = 128          # samples per pass
CHUNK_RAYS = 16  # rays per chunk
NCHUNK = R // CHUNK_RAYS          # 8
CN = CHUNK_RAYS * S               # 2048 cols per chunk
TILE_N = 512                      # matmul moving size
NTILE = CN // TILE_N              # 4 point-tiles per chunk

MAGIC = np.float32(12582912.0)    # 1.5 * 2^23 (round-to-int trick)
INV2PI = np.float32(1.0 / (2.0 * np.pi))
C1 = np.float32(6.28125)          # 2*pi split, k*C1 exact for k < 2^13
C2 = np.float32(2.0 * np.pi - 6.28125)

BUILD_STAGE = int(os.environ.get("KERNEL_STAGE", "3"))
DEBUG_OUT = os.environ.get("KERNEL_DEBUG", "0") == "1"


# ---------------------------------------------------------------- host prep
def _posenc_rows(nf, span=None, minp=None):
    """A3 [6*nf,3] / const [6*nf] for rows f-major: per f: 3 sin, 3 cos."""
    rows = 6 * nf
    A3 = np.zeros((rows, 3), np.float64)
    ph = np.zeros((rows,), np.float64)
    for f in range(nf):
        for k in range(6):
            r = 6 * f + k
            d = k % 3
            sc = 2.0 ** f
            if span is not None:
                A3[r, d] = sc / span[d]
                ph[r] = -sc * minp[d] / span[d]
            else:
                A3[r, d] = sc
            if k >= 3:
                ph[r] += np.pi / 2.0
    return A3, ph


def host_prep(inp):
    c = {}
    f32 = np.float32

    # coarse enc: per-ray rank-1 matrices  arg[i,(r,s)] = m*(B@d) + (C@[o;1])
    # cC4T [4,63]: cols 0:60 sin rows (A3s row + phase), cols 60:63 linear xyz
    A3s, phs = _posenc_rows(10)
    cC4 = np.zeros((4, 63), np.float64)
    cC4[0:3, 0:60] = A3s.T
    cC4[3, 0:60] = phs
    cC4[0:3, 60:63] = np.eye(3)
    c['cC4T'] = cC4.astype(f32).copy()                           # [4,63]

    # fine posenc rows: [sinx60, sinapp36, xyz3, appx3]
    minp = inp['min_point'].astype(np.float64)
    span = (inp['max_point'] - inp['min_point']).astype(np.float64)
    A3a, pha = _posenc_rows(6, span=span, minp=minp)
    pad4 = np.zeros((4, 3))
    fA3 = np.concatenate([A3s, pad4, A3a, np.eye(3), np.diag(1.0 / span)], 0)
    fph = np.concatenate([phs, np.zeros(4), pha, np.zeros(3), -minp / span], 0)
    c['fA3T'] = fA3.T.astype(f32).copy()                         # [3,106]
    c['fA4T'] = np.concatenate([fA3, fph[:, None]], 1).T.astype(f32).copy()

    # per-ray enc matrices (lhsT)
    Ad = np.zeros((24, 4), np.float64)
    for f in range(4):
        for k in range(6):
            r = 6 * f + k
            Ad[r, k % 3] = 2.0 ** f
            if k >= 3:
                Ad[r, 3] = np.pi / 2.0
    c['AdT'] = Ad.T.astype(f32).copy()                           # [4,24]
    At = np.zeros((12, 2), np.float64)
    for f in range(6):
        At[2 * f, 0] = 2.0 ** f
        At[2 * f + 1, 0] = 2.0 ** f
        At[2 * f + 1, 1] = np.pi / 2.0
    c['AtT'] = At.T.astype(f32).copy()                           # [2,12]

    perm63 = list(range(3, 63)) + [0, 1, 2]
    c['pW0my'] = np.ascontiguousarray(inp['pW0'][perm63])        # [63,128]
    c['pW1'] = inp['pW1'].copy()
    c['pW2'] = inp['pW2'].copy()
    c['pWo'] = inp['pWo'].copy()                                 # [128,1]
    c['pb0col'] = inp['pb0'].reshape(-1, 1).copy()
    c['pb1col'] = inp['pb1'].reshape(-1, 1).copy()
    c['pb2col'] = inp['pb2'].reshape(-1, 1).copy()

    c['fW0my'] = np.ascontiguousarray(inp['fW0'][perm63])        # [63,256]

    def pack_km(Wm):  # [256, 256] -> [128, 4, 128], slot 2k+m
        out = np.zeros((128, 4, 128), f32)
        for k in range(2):
            for m in range(2):
                out[:, 2 * k + m, :] = Wm[k * 128:(k + 1) * 128,
                                          m * 128:(m + 1) * 128]
        return out

    for i in range(3):
        c[f'fWm{i}'] = pack_km(inp['fWm'][i])
        c[f'fWp{i}'] = pack_km(inp['fWp'][i])
    c['fWs_h'] = pack_km(inp['fWs'][0:256])
    c['fWs_e'] = np.ascontiguousarray(inp['fWs'][256:][perm63])  # [63,256]
    c['fb0col'] = inp['fb0'].reshape(2, 128).T.copy()            # [128,2]
    for i in range(3):
        c[f'fbm{i}col'] = inp['fbm'][i].reshape(2, 128).T.copy()
        c[f'fbp{i}col'] = inp['fbp'][i].reshape(2, 128).T.copy()
    c['fbscol'] = inp['fbs'].reshape(2, 128).T.copy()

    # view head: fold Wfeat into Wview
    Wv = inp['Wview']
    Wv_d, Wv_emb, Wv_t, Wv_app = (Wv[256:283], Wv[283:331],
                                  Wv[331:344], Wv[344:383])
    Wfc = (inp['Wfeat'].astype(np.float64) @ Wv[0:256].astype(np.float64)
           ).astype(f32)
    out = np.zeros((128, 2, 128), f32)
    out[:, 0, :] = Wfc[0:128]
    out[:, 1, :] = Wfc[128:256]
    c['Wfc'] = out
    c['bveffcol'] = (inp['bfeat'].astype(np.float64)
                     @ Wv[0:256].astype(np.float64)
                     + inp['bview'].astype(np.float64)
                     ).astype(f32).reshape(-1, 1)
    perm39 = list(range(3, 39)) + [0, 1, 2]
    c['Wv_app'] = np.ascontiguousarray(Wv_app[perm39])           # [39,128]
    c['Wv_d_lin'] = np.ascontiguousarray(Wv_d[0:3])
    c['Wv_d_sin'] = np.ascontiguousarray(Wv_d[3:27])
    c['Wv_emb'] = np.ascontiguousarray(Wv_emb)
    c['Wv_t_lin'] = np.ascontiguousarray(Wv_t[0:1])
    c['Wv_t_sin'] = np.ascontiguousarray(Wv_t[1:13])
    c['Wsig'] = np.stack([inp['Wsig'][0:128, 0],
                          inp['Wsig'][128:256, 0]], 1).copy()    # [128,2]
    c['Wrgb'] = inp['Wrgb'].copy()                               # [128,3]
    c['brgbcol'] = inp['brgb'].reshape(-1, 1).copy()             # [3,1]
    c['brgbcol2'] = (0.5 * inp['brgb']).reshape(-1, 1).astype(f32)
    c['emb_table'] = inp['emb_table'].copy()

    c['sgrid'] = np.broadcast_to(
        np.arange(129, dtype=f32) / 128.0, (128, 129)).copy()
    c['identity'] = np.eye(128, dtype=f32)
    E = np.zeros((4, 512), f32)
    for rl in range(4):
        E[rl, rl * 128:(rl + 1) * 128] = 1.0
    c['Etile'] = E
    c['iotacol'] = np.arange(100, dtype=f32).reshape(-1, 1)
    scalars = dict(pbo_f=float(inp['pbo'][0]), bsig_f=float(inp['bsig'][0]))
    return c, scalars


INPUT_SHAPES = {
    'rays': (R, 12),
    'cC4T': (4, 63),
    'fA3T': (3, 106), 'fA4T': (4, 106),
    'AdT': (4, 24), 'AtT': (2, 12),
    'pW0my': (63, 128),
    'pW1': (128, 128), 'pW2': (128, 128), 'pWo': (128, 1),
    'pb0col': (128, 1), 'pb1col': (128, 1), 'pb2col': (128, 1),
    'fW0my': (63, 256), 'fWm0': (128, 4, 128), 'fWm1': (128, 4, 128),
    'fWm2': (128, 4, 128), 'fWp0': (128, 4, 128), 'fWp1': (128, 4, 128),
    'fWp2': (128, 4, 128), 'fWs_h': (128, 4, 128), 'fWs_e': (63, 256),
    'fb0col': (128, 2), 'fbm0col': (128, 2), 'fbm1col': (128, 2),
    'fbm2col': (128, 2), 'fbp0col': (128, 2), 'fbp1col': (128, 2),
    'fbp2col': (128, 2), 'fbscol': (128, 2),
    'Wfc': (128, 2, 128), 'bveffcol': (128, 1), 'Wv_app': (39, 128),
    'Wv_d_lin': (3, 128), 'Wv_d_sin': (24, 128), 'Wv_emb': (48, 128),
    'Wv_t_lin': (1, 128), 'Wv_t_sin': (12, 128),
    'Wsig': (128, 2), 'Wrgb': (128, 3), 'brgbcol': (3, 1),
    'brgbcol2': (3, 1),
    'emb_table': (100, 48),
    'sgrid': (128, 129), 'identity': (128, 128),
    'Etile': (4, 512), 'iotacol': (100, 1),
}
F32R_WEIGHTS = {'fW0my', 'fWm0', 'fWm1', 'fWm2', 'fWp0', 'fWp1', 'fWp2',
                'fWs_h', 'fWs_e', 'Wfc', 'Wv_app', 'Wv_d_lin', 'Wv_d_sin',
                'Wv_emb', 'Wv_t_lin', 'Wv_t_sin', 'Wsig', 'Wrgb',
                'emb_table', 'Etile'}


# ---------------------------------------------------------------- bass build
def build_nc(pbo_f, bsig_f, stage=3, debug=False):
    nc = bacc.Bacc("TRN2", target_bir_lowering=False)
    D = {k: nc.dram_tensor(k, list(v), F32, kind="ExternalInput")
         for k, v in INPUT_SHAPES.items()}
    OUT = nc.dram_tensor("rgb_out", [R, 3], F32, kind="ExternalOutput")
    dbg = {}
    if debug:
        for nm, shp in [("d_sigc", (R, S)), ("d_zf", (R, S + 1)),
                        ("d_wc", (R, S)), ("d_sigf", (R, S)),
                        ("d_wf", (R, S)), ("d_hvray", (128, R)),
                        ("d_ec", (63, CN)), ("d_efa", (63, CN)),
                        ("d_efb", (39, CN)), ("d_h1", (128, 2 * TILE_N))]:
            dbg[nm] = nc.dram_tensor(nm, list(shp), F32, kind="ExternalOutput")
    with tile.TileContext(nc) as tc:
        _body(nc, tc, D, OUT, dbg, pbo_f, bsig_f, stage, debug)
    nc.compile()
    return nc


def _body(nc, tc, D, OUT, dbg, pbo_f, bsig_f, stage, debug):
    from contextlib import ExitStack
    ctx = ExitStack()
    wpool = ctx.enter_context(tc.tile_pool(name="w", bufs=1))
    per = ctx.enter_context(tc.tile_pool(name="per", bufs=1))
    pp2 = ctx.enter_context(tc.tile_pool(name="pp2", bufs=2))
    big = ctx.enter_context(tc.tile_pool(name="big", bufs=2))
    hp = ctx.enter_context(tc.tile_pool(name="h", bufs=3))
    dram = ctx.enter_context(tc.tile_pool(name="dr", bufs=2, space="DRAM"))
    psA = ctx.enter_context(tc.tile_pool(name="psA", bufs=4, space="PSUM"))
    psS = ctx.enter_context(tc.tile_pool(name="psS", bufs=1, space="PSUM"))
    psR = ctx.enter_context(tc.tile_pool(name="psR", bufs=1, space="PSUM"))
    psC = ctx.enter_context(tc.tile_pool(name="psC", bufs=1, space="PSUM"))

    W = {}
    for k, t in D.items():
        if k == 'rays':
            continue
        dt = F32R if k in F32R_WEIGHTS else F32
        tl = wpool.tile(list(t.shape), dt, tag="w_" + k)
        nc.sync.dma_start(tl[:], t[:].bitcast(F32R) if dt == F32R else t[:])
        W[k] = tl
    rays = wpool.tile([R, 12], F32, tag="w_rays")
    nc.sync.dma_start(rays[:], D['rays'][:])
    ident = W['identity']

    # ---------------- phase 0: per-ray prep (ray-major layout)
    nearc = per.tile([R, 1], F32)
    nc.vector.tensor_scalar(nearc[:], rays[:, 6:7], 1e-8, None, op0=OP.max)
    spanc = per.tile([R, 1], F32)
    nc.vector.tensor_tensor(spanc[:], rays[:, 7:8], nearc[:], op=OP.subtract)

    dsq = per.tile([R, 3], F32)
    nc.vector.tensor_tensor(dsq[:], rays[:, 3:6], rays[:, 3:6], op=OP.mult)
    ssum = per.tile([R, 1], F32)
    nc.vector.reduce_sum(ssum[:], dsq[:], axis=mybir.AxisListType.X)
    norm = per.tile([R, 1], F32)
    nc.scalar.activation(norm[:], ssum[:], AF.Sqrt)
    for it in range(2):
        t1 = per.tile([R, 1], F32, tag="nwt")
        nc.vector.reciprocal(t1[:], norm[:])
        nc.vector.scalar_tensor_tensor(t1[:], ssum[:], 1.0, t1[:],
                                       op0=OP.mult, op1=OP.mult)
        nc.vector.tensor_tensor(t1[:], t1[:], norm[:], op=OP.add)
        nc.vector.tensor_scalar(norm[:], t1[:], 0.5, None, op0=OP.mult)
    invn = per.tile([R, 1], F32)
    nc.vector.reciprocal(invn[:], norm[:])

    # bundle: 0:3 oc, 3 ones | 4:7 dc | 8:11 o, 11 ones | 12:15 dir |
    #         16:19 viewdir, 19 ones | 20 t, 21 ones | 22 embid
    bundle = per.tile([R, 28], F32)
    nc.gpsimd.memset(bundle[:], 0.0)
    nc.vector.scalar_tensor_tensor(bundle[:, 0:3], rays[:, 3:6], nearc[:],
                                   rays[:, 0:3], op0=OP.mult, op1=OP.add)
    nc.vector.memset(bundle[:, 3:4], 1.0)
    nc.vector.tensor_scalar(bundle[:, 4:7], rays[:, 3:6], spanc[:], None,
                            op0=OP.mult)
    nc.vector.tensor_copy(bundle[:, 8:11], rays[:, 0:3])
    nc.vector.memset(bundle[:, 11:12], 1.0)
    nc.vector.tensor_copy(bundle[:, 12:15], rays[:, 3:6])
    nc.vector.tensor_scalar(bundle[:, 16:19], rays[:, 3:6], invn[:], None,
                            op0=OP.mult)
    nc.vector.memset(bundle[:, 19:20], 1.0)
    nc.vector.tensor_copy(bundle[:, 20:21], rays[:, 8:9])
    nc.vector.memset(bundle[:, 21:22], 1.0)
    nc.vector.tensor_copy(bundle[:, 22:23], rays[:, 9:10])

    def transp(col):
        p = psC.tile([4, 128], F32, tag="ptp")
        nc.tensor.transpose(p[:], bundle[:, col:col + 4], ident[:])
        sb = per.tile([4, 128], F32, tag="tp%d" % col)
        nc.scalar.copy(sb[:], p[:])
        return sb

    ocT = transp(0)      # [ocT;ones]
    dcT = transp(4)      # [dcT;..]
    oT = transp(8)       # [oT;ones]
    dirT = transp(12)
    vdT = transp(16)     # [viewdirT;ones]
    tT = transp(20)      # [t;ones;embid]
    eiT = transp(22)     # row0 = embid (base 0 for partition_broadcast)

    def mm_copy(lhsT, rhs, shape, nm, dst_dtype=F32):
        p = psC.tile(shape, F32, tag="pmc")
        nc.tensor.matmul(p[:], lhsT, rhs, start=True, stop=True)
        sb = per.tile(shape, dst_dtype, tag="mc_" + nm)
        nc.scalar.copy(sb[:], p[:])
        return sb

    Bf = mm_copy(W['fA3T'][:], dirT[0:3, :], [106, 128], "Bf")
    Cf = mm_copy(W['fA4T'][:], oT[:], [106, 128], "Cf")
    Bc = mm_copy(W['cC4T'][0:3, :], dirT[0:3, :], [63, 128], "Bc")
    Cc = mm_copy(W['cC4T'][:], oT[:], [63, 128], "Cc")

    def rangered_v(ap, shape, tag):
        sc = per.tile(shape, F32, tag=tag)
        nc.vector.tensor_scalar(sc[:], ap, float(INV2PI), float(MAGIC),
                                op0=OP.mult, op1=OP.add)
        nc.vector.tensor_scalar(sc[:], sc[:], float(MAGIC), None,
                                op0=OP.subtract)
        nc.vector.scalar_tensor_tensor(ap, sc[:], -float(C1), ap,
                                       op0=OP.mult, op1=OP.add)
        nc.vector.scalar_tensor_tensor(ap, sc[:], -float(C2), ap,
                                       op0=OP.mult, op1=OP.add)

    # per-ray view features
    argd = mm_copy(W['AdT'][:], vdT[:], [24, 128], 'argd')
    rangered_v(argd[:], [24, 128], "rrd")
    sind = per.tile([24, 128], F32R)
    nc.scalar.activation(sind[:], argd[:], AF.Sin)
    vd_r = per.tile([4, 128], F32R)
    nc.vector.tensor_copy(vd_r[:], vdT[:])

    argt = mm_copy(W['AtT'][:], tT[0:2, :], [12, 128], 'argt')
    rangered_v(argt[:], [12, 128], "rrt")
    sint = per.tile([12, 128], F32R)
    nc.scalar.activation(sint[:], argt[:], AF.Sin)
    t_r = per.tile([4, 128], F32R)
    nc.vector.tensor_copy(t_r[:], tT[:])

    embBC = per.tile([100, 128], F32)
    nc.gpsimd.partition_broadcast(embBC[:], eiT[0:1, :], channels=100)
    onehot = per.tile([100, 128], F32R)
    nc.vector.tensor_scalar(onehot[:], embBC[:], W['iotacol'][:], None,
                            op0=OP.is_equal)
    embT = mm_copy(W['emb_table'][:], onehot[:], [48, 128], 'embT', dst_dtype=F32R)

    phv = psC.tile([128, 128], F32, tag="pmc")
    nc.tensor.matmul(phv[:], W['Wv_d_lin'][:], vd_r[0:3, :],
                     start=True, stop=False)
    nc.tensor.matmul(phv[:], W['Wv_d_sin'][:], sind[:], start=False, stop=False)
    nc.tensor.matmul(phv[:], W['Wv_emb'][:], embT[:], start=False, stop=False)
    nc.tensor.matmul(phv[:], W['Wv_t_lin'][:], t_r[0:1, :],
                     start=False, stop=False)
    nc.tensor.matmul(phv[:], W['Wv_t_sin'][:], sint[:], start=False, stop=True)
    hvray = per.tile([128, 128], F32)
    nc.vector.tensor_scalar(hvray[:], phv[:], W['bveffcol'][:], None,
                            op0=OP.add)
    if debug:
        nc.sync.dma_start(dbg["d_hvray"][:], hvray[:])
    phvT = psC.tile([128, 128], F32, tag="pmc")
    nc.tensor.transpose(phvT[:], hvray[:], ident[:])
    hvrayT = per.tile([128, 128], F32R)
    nc.scalar.copy(hvrayT[:], phvT[:])
    hvb = dram.tile([128, 128], F32R, tag="hvb")
    nc.sync.dma_start(hvb[:], hvrayT[:])
    hvre = wpool.tile([4, 32, 128], F32R, tag="hvre")
    nc.sync.dma_start(hvre[:], hvb[:].rearrange("(t rl) m -> rl t m", rl=4))

    # coarse z edges
    zc = per.tile([R, S + 1], F32)
    nc.vector.tensor_scalar(zc[:], W['sgrid'][:], spanc[:], None, op0=OP.mult)
    nc.vector.tensor_scalar(zc[:], zc[:], nearc[:], None, op0=OP.add)
    midc = per.tile([R, S], F32)
    nc.vector.tensor_tensor(midc[:], zc[:, 0:S], zc[:, 1:S + 1], op=OP.add)
    nc.vector.tensor_scalar(midc[:], midc[:], 0.5, None, op0=OP.mult)

    # ======================= COARSE PASS =======================
    # midc bounce to DRAM once; per chunk DMA-replicate flat mids to 63 rows
    mc_dram = dram.tile([R, S], F32, tag="mcd", bufs=1)
    nc.scalar.dma_start(mc_dram[:], midc[:])
    sigcT = per.tile([R, S], F32, tag="sigcT")

    def issue_argc(ci):
        r0 = ci * CHUNK_RAYS
        t = big.tile([63, CN], F32, tag="arg")
        msrc = (mc_dram[r0:r0 + CHUNK_RAYS, :]
                .rearrange("p f -> (p f)").unsqueeze(0)
                .broadcast_to([63, CN]))
        nc.sync.dma_start(t[:], msrc)
        return t

    argc_pre = {0: issue_argc(0)}
    for ci in range(NCHUNK):
        r0 = ci * CHUNK_RAYS
        if ci + 1 < NCHUNK:
            argc_pre[ci + 1] = issue_argc(ci + 1)
        argc = argc_pre.pop(ci)
        B3 = Bc[:, r0:r0 + CHUNK_RAYS].unsqueeze(2).broadcast_to(
            [63, CHUNK_RAYS, S])
        C3 = Cc[:, r0:r0 + CHUNK_RAYS].unsqueeze(2).broadcast_to(
            [63, CHUNK_RAYS, S])
        a3 = argc[:].rearrange("p (r s) -> p r s", r=CHUNK_RAYS)
        nc.vector.tensor_tensor(a3, a3, B3, op=OP.mult)
        nc.gpsimd.tensor_tensor(a3, a3, C3, op=OP.add)
        sc = big.tile([100, CN], F32, tag="mbcrr", bufs=1)
        nc.gpsimd.tensor_scalar(sc[0:60, :], argc[0:60, :], float(INV2PI),
                                float(MAGIC), op0=OP.mult, op1=OP.add)
        nc.gpsimd.tensor_scalar(sc[0:60, :], sc[0:60, :], float(MAGIC), None,
                                op0=OP.subtract)
        nc.vector.scalar_tensor_tensor(argc[0:60, :], sc[0:60, :], -float(C1),
                                       argc[0:60, :], op0=OP.mult, op1=OP.add)
        nc.vector.scalar_tensor_tensor(argc[0:60, :], sc[0:60, :], -float(C2),
                                       argc[0:60, :], op0=OP.mult, op1=OP.add)
        sb_ = dram.tile([1, CN], F32, tag="sigb")
        sigflat = pp2.tile([1, CN], F32, tag="sigflat", bufs=1)
        nc.scalar.activation(argc[0:60, :], argc[0:60, :], AF.Sin)
        # layer-major over tile pairs: PE works tile t+1 while relu(t) lands
        for tp in range(NTILE // 2):
            pair = (2 * tp, 2 * tp + 1)
            colsv = [slice(t * TILE_N, (t + 1) * TILE_N) for t in pair]
            hh = []
            for i, t in enumerate(pair):
                p1 = psA.tile([128, TILE_N], F32, tag="mmps")
                nc.tensor.matmul(p1[:], W['pW0my'][:], argc[:, colsv[i]],
                                 start=True, stop=True)
                h1 = hp.tile([128, TILE_N], F32, tag="ch", bufs=4)
                nc.scalar.activation(h1[:], p1[:], AF.Relu,
                                     bias=W['pb0col'][:])
                hh.append(h1)
            for i, t in enumerate(pair):
                p2 = psA.tile([128, TILE_N], F32, tag="mmps")
                nc.tensor.matmul(p2[:], W['pW1'][:], hh[i][:],
                                 start=True, stop=True)
                h2 = hp.tile([128, TILE_N], F32, tag="ch", bufs=4)
                if i == 0:
                    nc.vector.tensor_scalar(h2[:], p2[:], W['pb1col'][:], 0.0,
                                            op0=OP.add, op1=OP.max)
                else:
                    nc.scalar.activation(h2[:], p2[:], AF.Relu,
                                         bias=W['pb1col'][:])
                hh[i] = h2
            for i, t in enumerate(pair):
                p3 = psA.tile([128, TILE_N], F32, tag="mmps")
                nc.tensor.matmul(p3[:], W['pW2'][:], hh[i][:],
                                 start=True, stop=True)
                h3 = hp.tile([128, TILE_N], F32, tag="ch", bufs=4)
                if i == 0:
                    nc.scalar.activation(h3[:], p3[:], AF.Relu,
                                         bias=W['pb2col'][:])
                else:
                    nc.vector.tensor_scalar(h3[:], p3[:], W['pb2col'][:], 0.0,
                                            op0=OP.add, op1=OP.max)
                hh[i] = h3
            for i, t in enumerate(pair):
                ps_ = psS.tile([1, TILE_N], F32, tag="sigps")
                nc.tensor.matmul(ps_[:], W['pWo'][:], hh[i][:],
                                 start=True, stop=True)
                if t % 2 == 0:
                    nc.scalar.copy(sigflat[0:1, colsv[i]], ps_[:])
                else:
                    nc.vector.tensor_copy(sigflat[0:1, colsv[i]], ps_[:])
        nc.sync.dma_start(sb_[:], sigflat[:])
        nc.sync.dma_start(sigcT[r0:r0 + CHUNK_RAYS, :],
                          sb_[:].rearrange("a (p f) -> (a p) f", p=CHUNK_RAYS))

    if debug:
        nc.sync.dma_start(dbg["d_sigc"][:], sigcT[:])
    if stage < 2:
        ctx.close()
        return

    # ======================= raw2weights helper =======================
    def raw2w(sigT_ap, z_lo, z_hi, norm_ap, bias_f, nrows, tag):
        """w = alpha * exclusive-cumprod(1-alpha+1e-10); returns (w, dz)."""
        P = nrows
        dz = per.tile([P, S], F32, tag=tag + "dz")
        nc.vector.tensor_tensor(dz[:], z_hi, z_lo, op=OP.subtract)
        di = per.tile([P, S], F32, tag=tag + "di")
        nc.vector.tensor_scalar(di[:], dz[:], norm_ap, None, op0=OP.mult)
        s1 = per.tile([P, S], F32, tag=tag + "s1")
        nc.vector.tensor_scalar(s1[:], sigT_ap, bias_f, 0.0,
                                op0=OP.add, op1=OP.max)
        ea = per.tile([P, S], F32, tag=tag + "ea")
        nc.vector.tensor_tensor(ea[:], s1[:], di[:], op=OP.mult)
        e = per.tile([P, S], F32, tag=tag + "e")
        nc.scalar.activation(e[:], ea[:], AF.Exp, scale=-1.0)
        al = per.tile([P, S], F32, tag=tag + "al")
        nc.vector.tensor_scalar(al[:], e[:], -1.0, 1.0, op0=OP.mult, op1=OP.add)
        om = per.tile([P, S], F32, tag=tag + "om")
        nc.vector.tensor_scalar(om[:], e[:], 1e-10, None, op0=OP.add)
        tr = per.tile([P, S], F32, tag=tag + "tr")
        nc.vector.tensor_tensor_scan(tr[:], om[:], om[:], 1.0,
                                     op0=OP.mult, op1=OP.bypass)
        w = per.tile([P, S], F32, tag=tag + "w")
        nc.vector.tensor_copy(w[:, 0:1], al[:, 0:1])
        nc.vector.tensor_tensor(w[:, 1:S], al[:, 1:S], tr[:, 0:S - 1],
                                op=OP.mult)
        return w, dz

    zf = per.tile([R, S + 1], F32)
    wc, dzc = raw2w(sigcT[:], zc[:, 0:S], zc[:, 1:S + 1],
                    norm[:], pbo_f, R, "c")
    Wt = per.tile([R, S], F32, tag="Wt")
    nc.vector.tensor_scalar(Wt[:], wc[:], 1e-5, None, op0=OP.add)
    Sx = per.tile([R, S], F32, tag="Sx")
    nc.vector.memset(Sx[:, 0:1], 0.0)
    nc.vector.tensor_tensor_scan(Sx[:, 1:S], Wt[:, 0:S - 1],
                                 Wt[:, 0:S - 1], 0.0,
                                 op0=OP.add, op1=OP.bypass)
    Tt = per.tile([R, 1], F32, tag="Tt")
    nc.vector.tensor_tensor(Tt[:], Sx[:, S - 1:S], Wt[:, S - 1:S],
                            op=OP.add)
    P2 = per.tile([R, S], F32, tag="P2")
    nc.vector.reciprocal(P2[:], Wt[:])
    nc.vector.tensor_tensor(P2[:], P2[:], dzc[:], op=OP.mult)
    JB = 16
    Sx_b = Sx[:].unsqueeze(1).broadcast_to([R, JB, S])
    P2_b = P2[:].unsqueeze(1).broadcast_to([R, JB, S])
    dz_b = dzc[:].unsqueeze(1).broadcast_to([R, JB, S])
    for jb in range(0, S, JB):
        rs_ = pp2.tile([R, JB * S], F32, tag="rsx", name="rs_", bufs=2)
        x3 = rs_[:].rearrange("p (j s) -> p j s", j=JB)
        g_b = W['sgrid'][:, jb:jb + JB].unsqueeze(2).broadcast_to([R, JB, S])
        nc.vector.scalar_tensor_tensor(x3, g_b, Tt[:], Sx_b,
                                       op0=OP.mult, op1=OP.subtract)
        nc.vector.scalar_tensor_tensor(x3, x3, 0.0, P2_b,
                                       op0=OP.max, op1=OP.mult)
        nc.vector.tensor_tensor(x3, x3, dz_b, op=OP.min)
        nc.vector.tensor_reduce(zf[:, jb:jb + JB], x3,
                                axis=mybir.AxisListType.X, op=OP.add)
    # last edge j=S: all bins saturate -> sum(dz) == zc[:,S] - zc[:,0]
    nc.vector.tensor_tensor(zf[:, S:S + 1], zc[:, S:S + 1], zc[:, 0:1],
                            op=OP.subtract)
    nc.vector.tensor_scalar(zf[:], zf[:], zc[:, 0:1], None, op0=OP.add)
    if debug:
        nc.sync.dma_start(dbg["d_zf"][:], zf[:])
        nc.sync.dma_start(dbg["d_wc"][:], wc[:])
    if stage < 3:
        ctx.close()
        return

    midf = per.tile([R, S], F32)
    nc.vector.tensor_tensor(midf[:], zf[:, 0:S], zf[:, 1:S + 1], op=OP.add)
    nc.vector.tensor_scalar(midf[:], midf[:], 0.5, None, op0=OP.mult)

    # ======================= FINE PASS =======================
    rgbmT = per.tile([3, 128], F32)
    nc.vector.memset(rgbmT[:], 0.0)

    mf_dram = dram.tile([R, S], F32, tag="mfd", bufs=1)
    nc.scalar.dma_start(mf_dram[:], midf[:])

    def issue_argf(ci):
        r0 = ci * CHUNK_RAYS
        t = big.tile([106, CN], F32, tag="arg")
        msrc = (mf_dram[r0:r0 + CHUNK_RAYS, :]
                .rearrange("p f -> (p f)").unsqueeze(0)
                .broadcast_to([106, CN]))
        nc.sync.dma_start(t[:], msrc)
        return t

    argf_pre = {0: issue_argf(0)}
    for ci in range(NCHUNK):
        r0 = ci * CHUNK_RAYS
        if ci + 1 < NCHUNK:
            argf_pre[ci + 1] = issue_argf(ci + 1)
        argf = argf_pre.pop(ci)
        b3 = Bf[:, r0:r0 + CHUNK_RAYS].unsqueeze(2).broadcast_to(
            [106, CHUNK_RAYS, S])
        c3 = Cf[:, r0:r0 + CHUNK_RAYS].unsqueeze(2).broadcast_to(
            [106, CHUNK_RAYS, S])
        a3 = argf[:].rearrange("p (r s) -> p r s", r=CHUNK_RAYS)
        nc.vector.tensor_tensor(a3, a3, b3, op=OP.mult)
        nc.gpsimd.tensor_tensor(a3, a3, c3, op=OP.add)
        sc = big.tile([100, CN], F32, tag="mbcrr", bufs=1)
        TWOPI = float(np.float32(2.0 * np.pi))
        for lo, hi in ((0, 60), (64, 100)):
            nc.gpsimd.tensor_scalar(sc[lo:hi, :], argf[lo:hi, :], float(INV2PI),
                                    float(MAGIC), op0=OP.mult, op1=OP.add)
            nc.gpsimd.tensor_scalar(sc[lo:hi, :], sc[lo:hi, :], float(MAGIC),
                                    None, op0=OP.subtract)
            nc.vector.scalar_tensor_tensor(argf[lo:hi, :], sc[lo:hi, :],
                                           -TWOPI, argf[lo:hi, :],
                                           op0=OP.mult, op1=OP.add)
        efa = big.tile([63, CN], F32R, tag="efa")
        efb = big.tile([39, CN], F32R, tag="efb")
        nc.scalar.activation(efa[0:60, :], argf[0:60, :], AF.Sin)
        nc.scalar.activation(efb[0:36, :], argf[64:100, :], AF.Sin)
        nc.sync.dma_start(efa[60:63, :], argf[100:103, :].bitcast(F32R))
        nc.sync.dma_start(efb[36:39, :], argf[103:106, :].bitcast(F32R))
        if debug and ci == 0:
            nc.sync.dma_start(dbg["d_efa"][:], efa[:].bitcast(F32))
            nc.sync.dma_start(dbg["d_efb"][:], efb[:].bitcast(F32))

        rgbS = big.tile([3, CN], F32, tag="rgbS")
        sb_ = dram.tile([1, CN], F32, tag="sigb")
        sigflat = pp2.tile([1, CN], F32, tag="sigflat", bufs=1)

        def relu2(pmm, bname, i):
            """bias+relu both halves; engines alternate per tile parity."""
            hout = hp.tile([128, 2 * TILE_N], F32R, tag="fh", bufs=4)
            if i == 0:
                nc.scalar.activation(hout[:, 0:TILE_N], pmm[0][:], AF.Relu,
                                     bias=W[bname][:, 0:1])
                nc.vector.tensor_scalar(hout[:, TILE_N:], pmm[1][:],
                                        W[bname][:, 1:2], 0.0,
                                        op0=OP.add, op1=OP.max)
            else:
                nc.vector.tensor_scalar(hout[:, 0:TILE_N], pmm[0][:],
                                        W[bname][:, 0:1], 0.0,
                                        op0=OP.add, op1=OP.max)
                nc.scalar.activation(hout[:, TILE_N:], pmm[1][:], AF.Relu,
                                     bias=W[bname][:, 1:2])
            return hout

        for tp in range(NTILE // 2):
            pair = (2 * tp, 2 * tp + 1)
            colsv = [slice(t * TILE_N, (t + 1) * TILE_N) for t in pair]
            hh = []
            for i, t in enumerate(pair):
                pm = [psA.tile([128, TILE_N], F32, tag="mmps",
                               name="pm%d" % _m) for _m in range(2)]
                for m in range(2):
                    nc.tensor.matmul(pm[m][:],
                                     W['fW0my'][:, m * 128:(m + 1) * 128],
                                     efa[:, colsv[i]], start=True, stop=True)
                hh.append(relu2(pm, 'fb0col', i))
            if debug and ci == 0:
                nc.sync.dma_start(dbg["d_h1"][:], hh[0][:].bitcast(F32))

            for wname, bname, skip in (
                    ('fWm0', 'fbm0col', False), ('fWm1', 'fbm1col', False),
                    ('fWm2', 'fbm2col', False), ('fWs_h', 'fbscol', True),
                    ('fWp0', 'fbp0col', False), ('fWp1', 'fbp1col', False),
                    ('fWp2', 'fbp2col', False)):
                for i, t in enumerate(pair):
                    hin = hh[i]
                    pmm = [psA.tile([128, TILE_N], F32, tag="mmps",
                                    name="pmm%d" % _m) for _m in range(2)]
                    for m in range(2):
                        nc.tensor.matmul(pmm[m][:], W[wname][:, m, :],
                                         hin[:, 0:TILE_N],
                                         start=True, stop=False)
                        nc.tensor.matmul(pmm[m][:], W[wname][:, 2 + m, :],
                                         hin[:, TILE_N:],
                                         start=False, stop=not skip)
                        if skip:
                            nc.tensor.matmul(
                                pmm[m][:],
                                W['fWs_e'][:, m * 128:(m + 1) * 128],
                                efa[:, colsv[i]], start=False, stop=True)
                    hh[i] = relu2(pmm, bname, i)

            for i, t in enumerate(pair):
                h = hh[i]
                gtile = ci * NTILE + t
                ps_ = psS.tile([1, TILE_N], F32, tag="sigps")
                nc.tensor.matmul(ps_[:], W['Wsig'][:, 0:1], h[:, 0:TILE_N],
                                 start=True, stop=False)
                nc.tensor.matmul(ps_[:], W['Wsig'][:, 1:2], h[:, TILE_N:],
                                 start=False, stop=True)
                if t % 2 == 0:
                    nc.scalar.copy(sigflat[0:1, colsv[i]], ps_[:])
                else:
                    nc.vector.tensor_copy(sigflat[0:1, colsv[i]], ps_[:])

                pv = psA.tile([128, TILE_N], F32, tag="mmps")
                nc.tensor.matmul(pv[:], W['Wfc'][:, 0, :], h[:, 0:TILE_N],
                                 start=True, stop=False)
                nc.tensor.matmul(pv[:], W['Wfc'][:, 1, :], h[:, TILE_N:],
                                 start=False, stop=False)
                nc.tensor.matmul(pv[:], W['Wv_app'][:], efb[:, colsv[i]],
                                 start=False, stop=False)
                nc.tensor.matmul(pv[:], hvre[:, gtile, :], W['Etile'][:],
                                 start=False, stop=True)
                hv = hp.tile([128, TILE_N], F32R, tag="fhv", bufs=2)
                nc.vector.tensor_scalar(hv[:], pv[:], 0.0, None, op0=OP.max)

                prgb = psR.tile([3, TILE_N], F32, tag="rgbps")
                nc.tensor.matmul(prgb[:], W['Wrgb'][:], hv[:],
                                 start=True, stop=True)
                nc.scalar.activation(rgbS[0:3, colsv[i]], prgb[:],
                                     AF.Sigmoid, bias=W['brgbcol'][:])

        nc.sync.dma_start(sb_[:], sigflat[:])
        sigch = pp2.tile([CHUNK_RAYS, S], F32, tag="sigch")
        nc.sync.dma_start(sigch[:],
                          sb_[:].rearrange("a (p f) -> (a p) f", p=CHUNK_RAYS))
        zfc = pp2.tile([CHUNK_RAYS, S + 1], F32, tag="zfc")
        nc.sync.dma_start(zfc[:], zf[r0:r0 + CHUNK_RAYS, :])
        normc = pp2.tile([CHUNK_RAYS, 1], F32, tag="normc")
        nc.sync.dma_start(normc[:], norm[r0:r0 + CHUNK_RAYS, :])

        wf, _dzf = raw2w(sigch[:], zfc[:, 0:S], zfc[:, 1:S + 1],
                         normc[:], bsig_f, CHUNK_RAYS, "f")
        if debug:
            nc.sync.dma_start(dbg["d_sigf"][r0:r0 + CHUNK_RAYS, :], sigch[:])
            nc.sync.dma_start(dbg["d_wf"][r0:r0 + CHUNK_RAYS, :], wf[:])

        wb = dram.tile([CHUNK_RAYS, S], F32, tag="wb")
        nc.sync.dma_start(wb[:], wf[:])
        wBC = pp2.tile([3, CN], F32, tag="wbc", bufs=1)
        nc.sync.dma_start(
            wBC[:],
            wb[:].rearrange("p f -> (p f)").unsqueeze(0).broadcast_to([3, CN]))
        nc.gpsimd.tensor_tensor(rgbS[0:3, :], rgbS[0:3, :], wBC[0:3, :],
                                op=OP.mult)
        nc.vector.tensor_reduce(
            rgbmT[0:3, r0:r0 + CHUNK_RAYS],
            rgbS[0:3, :].rearrange("p (r s) -> p r s", r=CHUNK_RAYS),
            axis=mybir.AxisListType.X, op=OP.add)

    # out: transpose [3,128] -> [128,3] via DRAM bounce
    rb = dram.tile([3, 128], F32, tag="rb")
    nc.sync.dma_start(rb[:], rgbmT[:])
    rgbout = per.tile([128, 3], F32)
    nc.sync.dma_start(rgbout[:], rb[:].rearrange("c r -> r c"))
    nc.sync.dma_start(OUT[:], rgbout[:])
    ctx.close()


# ---------------------------------------------------------------- entry
_CACHE = {}


def kernel(**inputs):
    inp = {k: np.asarray(v) for k, v in inputs.items()}
    consts, scal = host_prep(inp)
    key = (BUILD_STAGE, DEBUG_OUT, scal['pbo_f'], scal['bsig_f'])
    if key not in _CACHE:
        _CACHE[key] = build_nc(scal['pbo_f'], scal['bsig_f'],
                               stage=BUILD_STAGE, debug=DEBUG_OUT)
    nc = _CACHE[key]
    rays = np.asarray(inp['rays'], np.float32)
    in_maps = []
    for core in range(NCORES):
        m = {k: np.ascontiguousarray(v, dtype=np.float32)
             for k, v in consts.items()}
        m['rays'] = np.ascontiguousarray(rays[core * R:(core + 1) * R])
        in_maps.append(m)
    res = run_bass_kernel_spmd(nc, in_maps, core_ids=list(range(NCORES)))
    globals()['_LAST_RESULTS'] = res
    return np.concatenate([r['rgb_out'] for r in res.results], 0)



# revision 49
# speedup vs baseline: 1.0945x; 1.0240x over previous
"""NeRF-style render kernel for TRN2 (8 NeuronCores, data-parallel over rays).

Self-contained: hardcodes all shapes. Coarse proposal MLP runs in fp32
(resampling is precision-critical), fine MLP in float32r.
"""
import os
import sys

sys.path.insert(0, '/opt/trn_rl_repo')
import numpy as np
import concourse.bass as bass
import concourse.bacc as bacc
import concourse.tile as tile
import concourse.mybir as mybir
from concourse.bass_utils import run_bass_kernel_spmd

F32 = mybir.dt.float32
F32R = mybir.dt.float32r
AF = mybir.ActivationFunctionType
OP = mybir.AluOpType

NCORES = 8
R = 128          # rays per core
S = 128          # samples per pass
CHUNK_RAYS = 16  # rays per chunk
NCHUNK = R // CHUNK_RAYS          # 8
CN = CHUNK_RAYS * S               # 2048 cols per chunk
TILE_N = 512                      # matmul moving size
NTILE = CN // TILE_N              # 4 point-tiles per chunk

MAGIC = np.float32(12582912.0)    # 1.5 * 2^23 (round-to-int trick)
INV2PI = np.float32(1.0 / (2.0 * np.pi))
C1 = np.float32(6.28125)          # 2*pi split, k*C1 exact for k < 2^13
C2 = np.float32(2.0 * np.pi - 6.28125)

BUILD_STAGE = int(os.environ.get("KERNEL_STAGE", "3"))
DEBUG_OUT = os.environ.get("KERNEL_DEBUG", "0") == "1"


# ---------------------------------------------------------------- host prep
def _posenc_rows(nf, span=None, minp=None):
    """A3 [6*nf,3] / const [6*nf] for rows f-major: per f: 3 sin, 3 cos."""
    rows = 6 * nf
    A3 = np.zeros((rows, 3), np.float64)
    ph = np.zeros((rows,), np.float64)
    for f in range(nf):
        for k in range(6):
            r = 6 * f + k
            d = k % 3
            sc = 2.0 ** f
            if span is not None:
                A3[r, d] = sc / span[d]
                ph[r] = -sc * minp[d] / span[d]
            else:
                A3[r, d] = sc
            if k >= 3:
                ph[r] += np.pi / 2.0
    return A3, ph


def host_prep(inp):
    c = {}
    f32 = np.float32

    # coarse enc: per-ray rank-1 matrices  arg[i,(r,s)] = m*(B@d) + (C@[o;1])
    # cC4T [4,63]: cols 0:60 sin rows (A3s row + phase), cols 60:63 linear xyz
    A3s, phs = _posenc_rows(10)
    cC4 = np.zeros((4, 63), np.float64)
    cC4[0:3, 0:60] = A3s.T
    cC4[3, 0:60] = phs
    cC4[0:3, 60:63] = np.eye(3)
    c['cC4T'] = cC4.astype(f32).copy()                           # [4,63]

    # fine posenc rows: [sinx60, sinapp36, xyz3, appx3]
    minp = inp['min_point'].astype(np.float64)
    span = (inp['max_point'] - inp['min_point']).astype(np.float64)
    A3a, pha = _posenc_rows(6, span=span, minp=minp)
    pad4 = np.zeros((4, 3))
    fA3 = np.concatenate([A3s, pad4, A3a, np.eye(3), np.diag(1.0 / span)], 0)
    fph = np.concatenate([phs, np.zeros(4), pha, np.zeros(3), -minp / span], 0)
    c['fA3T'] = fA3.T.astype(f32).copy()                         # [3,106]
    c['fA4T'] = np.concatenate([fA3, fph[:, None]], 1).T.astype(f32).copy()

    # per-ray enc matrices (lhsT)
    Ad = np.zeros((24, 4), np.float64)
    for f in range(4):
        for k in range(6):
            r = 6 * f + k
            Ad[r, k % 3] = 2.0 ** f
            if k >= 3:
                Ad[r, 3] = np.pi / 2.0
    c['AdT'] = Ad.T.astype(f32).copy()                           # [4,24]
    At = np.zeros((12, 2), np.float64)
    for f in range(6):
        At[2 * f, 0] = 2.0 ** f
        At[2 * f + 1, 0] = 2.0 ** f
        At[2 * f + 1, 1] = np.pi / 2.0
    c['AtT'] = At.T.astype(f32).copy()                           # [2,12]

    perm63 = list(range(3, 63)) + [0, 1, 2]
    c['pW0my'] = np.ascontiguousarray(inp['pW0'][perm63])        # [63,128]
    c['pW1'] = inp['pW1'].copy()
    c['pW2'] = inp['pW2'].copy()
    c['pWo'] = inp['pWo'].copy()                                 # [128,1]
    c['pb0col'] = inp['pb0'].reshape(-1, 1).copy()
    c['pb1col'] = inp['pb1'].reshape(-1, 1).copy()
    c['pb2col'] = inp['pb2'].reshape(-1, 1).copy()

    c['fW0my'] = np.ascontiguousarray(inp['fW0'][perm63])        # [63,256]

    def pack_km(Wm):  # [256, 256] -> [128, 4, 128], slot 2k+m
        out = np.zeros((128, 4, 128), f32)
        for k in range(2):
            for m in range(2):
                out[:, 2 * k + m, :] = Wm[k * 128:(k + 1) * 128,
                                          m * 128:(m + 1) * 128]
        return out

    for i in range(3):
        c[f'fWm{i}'] = pack_km(inp['fWm'][i])
        c[f'fWp{i}'] = pack_km(inp['fWp'][i])
    c['fWs_h'] = pack_km(inp['fWs'][0:256])
    c['fWs_e'] = np.ascontiguousarray(inp['fWs'][256:][perm63])  # [63,256]
    c['fb0col'] = inp['fb0'].reshape(2, 128).T.copy()            # [128,2]
    for i in range(3):
        c[f'fbm{i}col'] = inp['fbm'][i].reshape(2, 128).T.copy()
        c[f'fbp{i}col'] = inp['fbp'][i].reshape(2, 128).T.copy()
    c['fbscol'] = inp['fbs'].reshape(2, 128).T.copy()

    # view head: fold Wfeat into Wview
    Wv = inp['Wview']
    Wv_d, Wv_emb, Wv_t, Wv_app = (Wv[256:283], Wv[283:331],
                                  Wv[331:344], Wv[344:383])
    Wfc = (inp['Wfeat'].astype(np.float64) @ Wv[0:256].astype(np.float64)
           ).astype(f32)
    out = np.zeros((128, 2, 128), f32)
    out[:, 0, :] = Wfc[0:128]
    out[:, 1, :] = Wfc[128:256]
    c['Wfc'] = out
    c['bveffcol'] = (inp['bfeat'].astype(np.float64)
                     @ Wv[0:256].astype(np.float64)
                     + inp['bview'].astype(np.float64)
                     ).astype(f32).reshape(-1, 1)
    perm39 = list(range(3, 39)) + [0, 1, 2]
    c['Wv_app'] = np.ascontiguousarray(Wv_app[perm39])           # [39,128]
    c['Wv_d_lin'] = np.ascontiguousarray(Wv_d[0:3])
    c['Wv_d_sin'] = np.ascontiguousarray(Wv_d[3:27])
    c['Wv_emb'] = np.ascontiguousarray(Wv_emb)
    c['Wv_t_lin'] = np.ascontiguousarray(Wv_t[0:1])
    c['Wv_t_sin'] = np.ascontiguousarray(Wv_t[1:13])
    c['Wsig'] = np.stack([inp['Wsig'][0:128, 0],
                          inp['Wsig'][128:256, 0]], 1).copy()    # [128,2]
    c['Wrgb'] = inp['Wrgb'].copy()                               # [128,3]
    c['brgbcol'] = inp['brgb'].reshape(-1, 1).copy()             # [3,1]
    c['brgbcol2'] = (0.5 * inp['brgb']).reshape(-1, 1).astype(f32)
    c['emb_table'] = inp['emb_table'].copy()

    c['sgrid'] = np.broadcast_to(
        np.arange(129, dtype=f32) / 128.0, (128, 129)).copy()
    c['identity'] = np.eye(128, dtype=f32)
    E = np.zeros((4, 512), f32)
    for rl in range(4):
        E[rl, rl * 128:(rl + 1) * 128] = 1.0
    c['Etile'] = E
    c['iotacol'] = np.arange(100, dtype=f32).reshape(-1, 1)
    scalars = dict(pbo_f=float(inp['pbo'][0]), bsig_f=float(inp['bsig'][0]))
    return c, scalars


INPUT_SHAPES = {
    'rays': (R, 12),
    'cC4T': (4, 63),
    'fA3T': (3, 106), 'fA4T': (4, 106),
    'AdT': (4, 24), 'AtT': (2, 12),
    'pW0my': (63, 128),
    'pW1': (128, 128), 'pW2': (128, 128), 'pWo': (128, 1),
    'pb0col': (128, 1), 'pb1col': (128, 1), 'pb2col': (128, 1),
    'fW0my': (63, 256), 'fWm0': (128, 4, 128), 'fWm1': (128, 4, 128),
    'fWm2': (128, 4, 128), 'fWp0': (128, 4, 128), 'fWp1': (128, 4, 128),
    'fWp2': (128, 4, 128), 'fWs_h': (128, 4, 128), 'fWs_e': (63, 256),
    'fb0col': (128, 2), 'fbm0col': (128, 2), 'fbm1col': (128, 2),
    'fbm2col': (128, 2), 'fbp0col': (128, 2), 'fbp1col': (128, 2),
    'fbp2col': (128, 2), 'fbscol': (128, 2),
    'Wfc': (128, 2, 128), 'bveffcol': (128, 1), 'Wv_app': (39, 128),
    'Wv_d_lin': (3, 128), 'Wv_d_sin': (24, 128), 'Wv_emb': (48, 128),
    'Wv_t_lin': (1, 128), 'Wv_t_sin': (12, 128),
    'Wsig': (128, 2), 'Wrgb': (128, 3), 'brgbcol': (3, 1),
    'brgbcol2': (3, 1),
    'emb_table': (100, 48),
    'sgrid': (128, 129), 'identity': (128, 128),
    'Etile': (4, 512), 'iotacol': (100, 1),
}
F32R_WEIGHTS = {'fW0my', 'fWm0', 'fWm1', 'fWm2', 'fWp0', 'fWp1', 'fWp2',
                'fWs_h', 'fWs_e', 'Wfc', 'Wv_app', 'Wv_d_lin', 'Wv_d_sin',
                'Wv_emb', 'Wv_t_lin', 'Wv_t_sin', 'Wsig', 'Wrgb',
                'emb_table', 'Etile'}


def _pack_layout():
    """Pack all consts into two [128, X] tensors (one per dtype)."""
    lay = {}
    offs = {'R': 0, 'F': 0}
    for k, shp in INPUT_SHAPES.items():
        if k == 'rays':
            continue
        which = 'R' if k in F32R_WEIGHTS else 'F'
        rows, cols = shp[0], int(np.prod(shp[1:]))
        lay[k] = (which, offs[which], rows, cols, shp)
        offs[which] += cols
    return lay, offs['R'], offs['F']


PACK_LAYOUT, PACK_NR, PACK_NF = _pack_layout()


def pack_consts(c):
    pR = np.zeros((128, PACK_NR), np.float32)
    pF = np.zeros((128, PACK_NF), np.float32)
    for k, (which, off, rows, cols, shp) in PACK_LAYOUT.items():
        dst = pR if which == 'R' else pF
        dst[0:rows, off:off + cols] = np.asarray(c[k], np.float32).reshape(
            rows, cols)
    return pR, pF


# ---------------------------------------------------------------- bass build
def build_nc(pbo_f, bsig_f, stage=3, debug=False):
    nc = bacc.Bacc("TRN2", target_bir_lowering=False)
    D = {'rays': nc.dram_tensor('rays', [R, 12], F32, kind="ExternalInput"),
         'packR': nc.dram_tensor('packR', [128, PACK_NR], F32,
                                 kind="ExternalInput"),
         'packF': nc.dram_tensor('packF', [128, PACK_NF], F32,
                                 kind="ExternalInput")}
    OUT = nc.dram_tensor("rgb_out", [R, 3], F32, kind="ExternalOutput")
    dbg = {}
    if debug:
        for nm, shp in [("d_sigc", (R, S)), ("d_zf", (R, S + 1)),
                        ("d_wc", (R, S)), ("d_sigf", (R, S)),
                        ("d_wf", (R, S)), ("d_hvray", (128, R)),
                        ("d_ec", (63, CN)), ("d_efa", (63, CN)),
                        ("d_efb", (39, CN)), ("d_h1", (128, 2 * TILE_N))]:
            dbg[nm] = nc.dram_tensor(nm, list(shp), F32, kind="ExternalOutput")
    with tile.TileContext(nc) as tc:
        _body(nc, tc, D, OUT, dbg, pbo_f, bsig_f, stage, debug)
    nc.compile()
    return nc


def _body(nc, tc, D, OUT, dbg, pbo_f, bsig_f, stage, debug):
    from contextlib import ExitStack
    ctx = ExitStack()
    wpool = ctx.enter_context(tc.tile_pool(name="w", bufs=1))
    per = ctx.enter_context(tc.tile_pool(name="per", bufs=1))
    pp2 = ctx.enter_context(tc.tile_pool(name="pp2", bufs=2))
    big = ctx.enter_context(tc.tile_pool(name="big", bufs=2))
    hp = ctx.enter_context(tc.tile_pool(name="h", bufs=3))
    dram = ctx.enter_context(tc.tile_pool(name="dr", bufs=2, space="DRAM"))
    psA = ctx.enter_context(tc.tile_pool(name="psA", bufs=4, space="PSUM"))
    psS = ctx.enter_context(tc.tile_pool(name="psS", bufs=1, space="PSUM"))
    psR = ctx.enter_context(tc.tile_pool(name="psR", bufs=1, space="PSUM"))
    psC = ctx.enter_context(tc.tile_pool(name="psC", bufs=1, space="PSUM"))

    tR = wpool.tile([128, PACK_NR], F32R, tag="w_packR")
    nc.sync.dma_start(tR[:], D['packR'][:].bitcast(F32R))
    tF = wpool.tile([128, PACK_NF], F32, tag="w_packF")
    nc.sync.dma_start(tF[:], D['packF'][:])
    W = {}
    for k, (which, off, rows, cols, shp) in PACK_LAYOUT.items():
        ap = (tR if which == 'R' else tF)[0:rows, off:off + cols]
        if len(shp) == 3:
            ap = ap.rearrange("p (a b) -> p a b", a=shp[1])
        W[k] = ap
    rays = wpool.tile([R, 12], F32, tag="w_rays")
    nc.sync.dma_start(rays[:], D['rays'][:])
    ident = W['identity']

    # ---------------- phase 0: per-ray prep (ray-major layout)
    nearc = per.tile([R, 1], F32)
    nc.vector.tensor_scalar(nearc[:], rays[:, 6:7], 1e-8, None, op0=OP.max)
    spanc = per.tile([R, 1], F32)
    nc.vector.tensor_tensor(spanc[:], rays[:, 7:8], nearc[:], op=OP.subtract)

    dsq = per.tile([R, 3], F32)
    nc.vector.tensor_tensor(dsq[:], rays[:, 3:6], rays[:, 3:6], op=OP.mult)
    ssum = per.tile([R, 1], F32)
    nc.vector.reduce_sum(ssum[:], dsq[:], axis=mybir.AxisListType.X)
    norm = per.tile([R, 1], F32)
    nc.scalar.activation(norm[:], ssum[:], AF.Sqrt)
    for it in range(2):
        t1 = per.tile([R, 1], F32, tag="nwt")
        nc.vector.reciprocal(t1[:], norm[:])
        nc.vector.scalar_tensor_tensor(t1[:], ssum[:], 1.0, t1[:],
                                       op0=OP.mult, op1=OP.mult)
        nc.vector.tensor_tensor(t1[:], t1[:], norm[:], op=OP.add)
        nc.vector.tensor_scalar(norm[:], t1[:], 0.5, None, op0=OP.mult)
    invn = per.tile([R, 1], F32)
    nc.vector.reciprocal(invn[:], norm[:])

    # bundle: 0:3 oc, 3 ones | 4:7 dc | 8:11 o, 11 ones | 12:15 dir |
    #         16:19 viewdir, 19 ones | 20 t, 21 ones | 22 embid
    bundle = per.tile([R, 28], F32)
    nc.gpsimd.memset(bundle[:], 0.0)
    nc.vector.scalar_tensor_tensor(bundle[:, 0:3], rays[:, 3:6], nearc[:],
                                   rays[:, 0:3], op0=OP.mult, op1=OP.add)
    nc.vector.memset(bundle[:, 3:4], 1.0)
    nc.vector.tensor_scalar(bundle[:, 4:7], rays[:, 3:6], spanc[:], None,
                            op0=OP.mult)
    nc.vector.tensor_copy(bundle[:, 8:11], rays[:, 0:3])
    nc.vector.memset(bundle[:, 11:12], 1.0)
    nc.vector.tensor_copy(bundle[:, 12:15], rays[:, 3:6])
    nc.vector.tensor_scalar(bundle[:, 16:19], rays[:, 3:6], invn[:], None,
                            op0=OP.mult)
    nc.vector.memset(bundle[:, 19:20], 1.0)
    nc.vector.tensor_copy(bundle[:, 20:21], rays[:, 8:9])
    nc.vector.memset(bundle[:, 21:22], 1.0)
    nc.vector.tensor_copy(bundle[:, 22:23], rays[:, 9:10])

    def transp(col):
        p = psC.tile([4, 128], F32, tag="ptp")
        nc.tensor.transpose(p[:], bundle[:, col:col + 4], ident[:])
        sb = per.tile([4, 128], F32, tag="tp%d" % col)
        nc.scalar.copy(sb[:], p[:])
        return sb

    ocT = transp(0)      # [ocT;ones]
    dcT = transp(4)      # [dcT;..]
    oT = transp(8)       # [oT;ones]
    dirT = transp(12)
    vdT = transp(16)     # [viewdirT;ones]
    tT = transp(20)      # [t;ones;embid]
    eiT = transp(22)     # row0 = embid (base 0 for partition_broadcast)

    def mm_copy(lhsT, rhs, shape, nm, dst_dtype=F32):
        p = psC.tile(shape, F32, tag="pmc")
        nc.tensor.matmul(p[:], lhsT, rhs, start=True, stop=True)
        sb = per.tile(shape, dst_dtype, tag="mc_" + nm)
        nc.scalar.copy(sb[:], p[:])
        return sb

    Bf = mm_copy(W['fA3T'][:], dirT[0:3, :], [106, 128], "Bf")
    Cf = mm_copy(W['fA4T'][:], oT[:], [106, 128], "Cf")
    Bc = mm_copy(W['cC4T'][0:3, :], dirT[0:3, :], [63, 128], "Bc")
    Cc = mm_copy(W['cC4T'][:], oT[:], [63, 128], "Cc")

    def rangered_v(ap, shape, tag):
        sc = per.tile(shape, F32, tag=tag)
        nc.vector.tensor_scalar(sc[:], ap, float(INV2PI), float(MAGIC),
                                op0=OP.mult, op1=OP.add)
        nc.vector.tensor_scalar(sc[:], sc[:], float(MAGIC), None,
                                op0=OP.subtract)
        nc.vector.scalar_tensor_tensor(ap, sc[:], -float(C1), ap,
                                       op0=OP.mult, op1=OP.add)
        nc.vector.scalar_tensor_tensor(ap, sc[:], -float(C2), ap,
                                       op0=OP.mult, op1=OP.add)

    # per-ray view features
    argd = mm_copy(W['AdT'][:], vdT[:], [24, 128], 'argd')
    rangered_v(argd[:], [24, 128], "rrd")
    sind = per.tile([24, 128], F32R)
    nc.scalar.activation(sind[:], argd[:], AF.Sin)
    vd_r = per.tile([4, 128], F32R)
    nc.vector.tensor_copy(vd_r[:], vdT[:])

    argt = mm_copy(W['AtT'][:], tT[0:2, :], [12, 128], 'argt')
    rangered_v(argt[:], [12, 128], "rrt")
    sint = per.tile([12, 128], F32R)
    nc.scalar.activation(sint[:], argt[:], AF.Sin)
    t_r = per.tile([4, 128], F32R)
    nc.vector.tensor_copy(t_r[:], tT[:])

    embBC = per.tile([100, 128], F32)
    nc.gpsimd.partition_broadcast(embBC[:], eiT[0:1, :], channels=100)
    onehot = per.tile([100, 128], F32R)
    nc.vector.tensor_scalar(onehot[:], embBC[:], W['iotacol'][:], None,
                            op0=OP.is_equal)
    embT = mm_copy(W['emb_table'][:], onehot[:], [48, 128], 'embT', dst_dtype=F32R)

    phv = psC.tile([128, 128], F32, tag="pmc")
    nc.tensor.matmul(phv[:], W['Wv_d_lin'][:], vd_r[0:3, :],
                     start=True, stop=False)
    nc.tensor.matmul(phv[:], W['Wv_d_sin'][:], sind[:], start=False, stop=False)
    nc.tensor.matmul(phv[:], W['Wv_emb'][:], embT[:], start=False, stop=False)
    nc.tensor.matmul(phv[:], W['Wv_t_lin'][:], t_r[0:1, :],
                     start=False, stop=False)
    nc.tensor.matmul(phv[:], W['Wv_t_sin'][:], sint[:], start=False, stop=True)
    hvray = per.tile([128, 128], F32)
    nc.vector.tensor_scalar(hvray[:], phv[:], W['bveffcol'][:], None,
                            op0=OP.add)
    if debug:
        nc.sync.dma_start(dbg["d_hvray"][:], hvray[:])
    phvT = psC.tile([128, 128], F32, tag="pmc")
    nc.tensor.transpose(phvT[:], hvray[:], ident[:])
    hvrayT = per.tile([128, 128], F32R)
    nc.scalar.copy(hvrayT[:], phvT[:])
    hvb = dram.tile([128, 128], F32R, tag="hvb")
    nc.sync.dma_start(hvb[:], hvrayT[:])
    hvre = wpool.tile([4, 32, 128], F32R, tag="hvre")
    nc.sync.dma_start(hvre[:], hvb[:].rearrange("(t rl) m -> rl t m", rl=4))

    # coarse z edges
    zc = per.tile([R, S + 1], F32)
    nc.vector.tensor_scalar(zc[:], W['sgrid'][:], spanc[:], None, op0=OP.mult)
    nc.vector.tensor_scalar(zc[:], zc[:], nearc[:], None, op0=OP.add)
    midc = per.tile([R, S], F32)
    nc.vector.tensor_tensor(midc[:], zc[:, 0:S], zc[:, 1:S + 1], op=OP.add)
    nc.vector.tensor_scalar(midc[:], midc[:], 0.5, None, op0=OP.mult)

    # ======================= COARSE PASS =======================
    # midc bounce to DRAM once; per chunk DMA-replicate flat mids to 63 rows
    mc_dram = dram.tile([R, S], F32, tag="mcd", bufs=1)
    nc.scalar.dma_start(mc_dram[:], midc[:])
    sigcT = per.tile([R, S], F32, tag="sigcT")

    def issue_argc(ci):
        r0 = ci * CHUNK_RAYS
        t = big.tile([63, CN], F32, tag="arg")
        msrc = (mc_dram[r0:r0 + CHUNK_RAYS, :]
                .rearrange("p f -> (p f)").unsqueeze(0)
                .broadcast_to([63, CN]))
        nc.sync.dma_start(t[:], msrc)
        return t

    argc_pre = {0: issue_argc(0)}
    for ci in range(NCHUNK):
        r0 = ci * CHUNK_RAYS
        if ci + 1 < NCHUNK:
            argc_pre[ci + 1] = issue_argc(ci + 1)
        argc = argc_pre.pop(ci)
        B3 = Bc[:, r0:r0 + CHUNK_RAYS].unsqueeze(2).broadcast_to(
            [63, CHUNK_RAYS, S])
        C3 = Cc[:, r0:r0 + CHUNK_RAYS].unsqueeze(2).broadcast_to(
            [63, CHUNK_RAYS, S])
        a3 = argc[:].rearrange("p (r s) -> p r s", r=CHUNK_RAYS)
        nc.vector.tensor_tensor(a3, a3, B3, op=OP.mult)
        nc.gpsimd.tensor_tensor(a3, a3, C3, op=OP.add)
        sc = big.tile([100, CN], F32, tag="mbcrr", bufs=1)
        nc.gpsimd.tensor_scalar(sc[0:60, :], argc[0:60, :], float(INV2PI),
                                float(MAGIC), op0=OP.mult, op1=OP.add)
        nc.gpsimd.tensor_scalar(sc[0:60, :], sc[0:60, :], float(MAGIC), None,
                                op0=OP.subtract)
        nc.vector.scalar_tensor_tensor(argc[0:60, :], sc[0:60, :], -float(C1),
                                       argc[0:60, :], op0=OP.mult, op1=OP.add)
        nc.vector.scalar_tensor_tensor(argc[0:60, :], sc[0:60, :], -float(C2),
                                       argc[0:60, :], op0=OP.mult, op1=OP.add)
        sb_ = dram.tile([1, CN], F32, tag="sigb")
        sigflat = pp2.tile([1, CN], F32, tag="sigflat", bufs=1)
        nc.scalar.activation(argc[0:60, :], argc[0:60, :], AF.Sin)
        # layer-major over tile pairs: PE works tile t+1 while relu(t) lands
        for tp in range(NTILE // 2):
            pair = (2 * tp, 2 * tp + 1)
            colsv = [slice(t * TILE_N, (t + 1) * TILE_N) for t in pair]
            hh = []
            for i, t in enumerate(pair):
                p1 = psA.tile([128, TILE_N], F32, tag="mmps")
                nc.tensor.matmul(p1[:], W['pW0my'][:], argc[:, colsv[i]],
                                 start=True, stop=True)
                h1 = hp.tile([128, TILE_N], F32, tag="ch", bufs=4)
                nc.scalar.activation(h1[:], p1[:], AF.Relu,
                                     bias=W['pb0col'][:])
                hh.append(h1)
            for i, t in enumerate(pair):
                p2 = psA.tile([128, TILE_N], F32, tag="mmps")
                nc.tensor.matmul(p2[:], W['pW1'][:], hh[i][:],
                                 start=True, stop=True)
                h2 = hp.tile([128, TILE_N], F32, tag="ch", bufs=4)
                if i == 0:
                    nc.vector.tensor_scalar(h2[:], p2[:], W['pb1col'][:], 0.0,
                                            op0=OP.add, op1=OP.max)
                else:
                    nc.scalar.activation(h2[:], p2[:], AF.Relu,
                                         bias=W['pb1col'][:])
                hh[i] = h2
            for i, t in enumerate(pair):
                p3 = psA.tile([128, TILE_N], F32, tag="mmps")
                nc.tensor.matmul(p3[:], W['pW2'][:], hh[i][:],
                                 start=True, stop=True)
                h3 = hp.tile([128, TILE_N], F32, tag="ch", bufs=4)
                if i == 0:
                    nc.scalar.activation(h3[:], p3[:], AF.Relu,
                                         bias=W['pb2col'][:])
                else:
                    nc.vector.tensor_scalar(h3[:], p3[:], W['pb2col'][:], 0.0,
                                            op0=OP.add, op1=OP.max)
                hh[i] = h3
            for i, t in enumerate(pair):
                ps_ = psS.tile([1, TILE_N], F32, tag="sigps")
                nc.tensor.matmul(ps_[:], W['pWo'][:], hh[i][:],
                                 start=True, stop=True)
                if t % 2 == 0:
                    nc.scalar.copy(sigflat[0:1, colsv[i]], ps_[:])
                else:
                    nc.vector.tensor_copy(sigflat[0:1, colsv[i]], ps_[:])
        nc.sync.dma_start(sb_[:], sigflat[:])
        nc.sync.dma_start(sigcT[r0:r0 + CHUNK_RAYS, :],
                          sb_[:].rearrange("a (p f) -> (a p) f", p=CHUNK_RAYS))

    if debug:
        nc.sync.dma_start(dbg["d_sigc"][:], sigcT[:])
    if stage < 2:
        ctx.close()
        return

    # ======================= raw2weights helper =======================
    def raw2w(sigT_ap, z_lo, z_hi, norm_ap, bias_f, nrows, tag):
        """w = alpha * exclusive-cumprod(1-alpha+1e-10); returns (w, dz)."""
        P = nrows
        dz = per.tile([P, S], F32, tag=tag + "dz")
        nc.vector.tensor_tensor(dz[:], z_hi, z_lo, op=OP.subtract)
        di = per.tile([P, S], F32, tag=tag + "di")
        nc.vector.tensor_scalar(di[:], dz[:], norm_ap, None, op0=OP.mult)
        s1 = per.tile([P, S], F32, tag=tag + "s1")
        nc.vector.tensor_scalar(s1[:], sigT_ap, bias_f, 0.0,
                                op0=OP.add, op1=OP.max)
        ea = per.tile([P, S], F32, tag=tag + "ea")
        nc.vector.tensor_tensor(ea[:], s1[:], di[:], op=OP.mult)
        e = per.tile([P, S], F32, tag=tag + "e")
        nc.scalar.activation(e[:], ea[:], AF.Exp, scale=-1.0)
        al = per.tile([P, S], F32, tag=tag + "al")
        nc.vector.tensor_scalar(al[:], e[:], -1.0, 1.0, op0=OP.mult, op1=OP.add)
        om = per.tile([P, S], F32, tag=tag + "om")
        nc.vector.tensor_scalar(om[:], e[:], 1e-10, None, op0=OP.add)
        tr = per.tile([P, S], F32, tag=tag + "tr")
        nc.vector.tensor_tensor_scan(tr[:], om[:], om[:], 1.0,
                                     op0=OP.mult, op1=OP.bypass)
        w = per.tile([P, S], F32, tag=tag + "w")
        nc.vector.tensor_copy(w[:, 0:1], al[:, 0:1])
        nc.vector.tensor_tensor(w[:, 1:S], al[:, 1:S], tr[:, 0:S - 1],
                                op=OP.mult)
        return w, dz

    zf = per.tile([R, S + 1], F32)
    wc, dzc = raw2w(sigcT[:], zc[:, 0:S], zc[:, 1:S + 1],
                    norm[:], pbo_f, R, "c")
    Wt = per.tile([R, S], F32, tag="Wt")
    nc.vector.tensor_scalar(Wt[:], wc[:], 1e-5, None, op0=OP.add)
    Sx = per.tile([R, S], F32, tag="Sx")
    nc.vector.memset(Sx[:, 0:1], 0.0)
    nc.vector.tensor_tensor_scan(Sx[:, 1:S], Wt[:, 0:S - 1],
                                 Wt[:, 0:S - 1], 0.0,
                                 op0=OP.add, op1=OP.bypass)
    Tt = per.tile([R, 1], F32, tag="Tt")
    nc.vector.tensor_tensor(Tt[:], Sx[:, S - 1:S], Wt[:, S - 1:S],
                            op=OP.add)
    P2 = per.tile([R, S], F32, tag="P2")
    nc.vector.reciprocal(P2[:], Wt[:])
    nc.vector.tensor_tensor(P2[:], P2[:], dzc[:], op=OP.mult)
    JB = 16
    Sx_b = Sx[:].unsqueeze(1).broadcast_to([R, JB, S])
    P2_b = P2[:].unsqueeze(1).broadcast_to([R, JB, S])
    dz_b = dzc[:].unsqueeze(1).broadcast_to([R, JB, S])
    for jb in range(0, S, JB):
        rs_ = pp2.tile([R, JB * S], F32, tag="rsx", name="rs_", bufs=2)
        x3 = rs_[:].rearrange("p (j s) -> p j s", j=JB)
        g_b = W['sgrid'][:, jb:jb + JB].unsqueeze(2).broadcast_to([R, JB, S])
        nc.vector.scalar_tensor_tensor(x3, g_b, Tt[:], Sx_b,
                                       op0=OP.mult, op1=OP.subtract)
        nc.vector.scalar_tensor_tensor(x3, x3, 0.0, P2_b,
                                       op0=OP.max, op1=OP.mult)
        nc.vector.tensor_tensor(x3, x3, dz_b, op=OP.min)
        nc.vector.tensor_reduce(zf[:, jb:jb + JB], x3,
                                axis=mybir.AxisListType.X, op=OP.add)
    # last edge j=S: all bins saturate -> sum(dz) == zc[:,S] - zc[:,0]
    nc.vector.tensor_tensor(zf[:, S:S + 1], zc[:, S:S + 1], zc[:, 0:1],
                            op=OP.subtract)
    nc.vector.tensor_scalar(zf[:], zf[:], zc[:, 0:1], None, op0=OP.add)
    if debug:
        nc.sync.dma_start(dbg["d_zf"][:], zf[:])
        nc.sync.dma_start(dbg["d_wc"][:], wc[:])
    if stage < 3:
        ctx.close()
        return

    midf = per.tile([R, S], F32)
    nc.vector.tensor_tensor(midf[:], zf[:, 0:S], zf[:, 1:S + 1], op=OP.add)
    nc.vector.tensor_scalar(midf[:], midf[:], 0.5, None, op0=OP.mult)

    # ======================= FINE PASS =======================
    rgbmT = per.tile([3, 128], F32)
    nc.vector.memset(rgbmT[:], 0.0)

    mf_dram = dram.tile([R, S], F32, tag="mfd", bufs=1)
    nc.scalar.dma_start(mf_dram[:], midf[:])

    def issue_argf(ci):
        r0 = ci * CHUNK_RAYS
        t = big.tile([106, CN], F32, tag="arg")
        msrc = (mf_dram[r0:r0 + CHUNK_RAYS, :]
                .rearrange("p f -> (p f)").unsqueeze(0)
                .broadcast_to([106, CN]))
        nc.sync.dma_start(t[:], msrc)
        return t

    argf_pre = {0: issue_argf(0)}
    for ci in range(NCHUNK):
        r0 = ci * CHUNK_RAYS
        if ci + 1 < NCHUNK:
            argf_pre[ci + 1] = issue_argf(ci + 1)
        argf = argf_pre.pop(ci)
        b3 = Bf[:, r0:r0 + CHUNK_RAYS].unsqueeze(2).broadcast_to(
            [106, CHUNK_RAYS, S])
        c3 = Cf[:, r0:r0 + CHUNK_RAYS].unsqueeze(2).broadcast_to(
            [106, CHUNK_RAYS, S])
        a3 = argf[:].rearrange("p (r s) -> p r s", r=CHUNK_RAYS)
        nc.vector.tensor_tensor(a3, a3, b3, op=OP.mult)
        nc.gpsimd.tensor_tensor(a3, a3, c3, op=OP.add)
        sc = big.tile([100, CN], F32, tag="mbcrr", bufs=1)
        TWOPI = float(np.float32(2.0 * np.pi))
        for lo, hi in ((0, 60), (64, 100)):
            nc.gpsimd.tensor_scalar(sc[lo:hi, :], argf[lo:hi, :], float(INV2PI),
                                    float(MAGIC), op0=OP.mult, op1=OP.add)
            nc.gpsimd.tensor_scalar(sc[lo:hi, :], sc[lo:hi, :], float(MAGIC),
                                    None, op0=OP.subtract)
            nc.vector.scalar_tensor_tensor(argf[lo:hi, :], sc[lo:hi, :],
                                           -TWOPI, argf[lo:hi, :],
                                           op0=OP.mult, op1=OP.add)
        efa = big.tile([63, CN], F32R, tag="efa")
        efb = big.tile([39, CN], F32R, tag="efb")
        nc.scalar.activation(efa[0:60, :], argf[0:60, :], AF.Sin)
        nc.scalar.activation(efb[0:36, :], argf[64:100, :], AF.Sin)
        nc.sync.dma_start(efa[60:63, :], argf[100:103, :].bitcast(F32R))
        nc.sync.dma_start(efb[36:39, :], argf[103:106, :].bitcast(F32R))
        if debug and ci == 0:
            nc.sync.dma_start(dbg["d_efa"][:], efa[:].bitcast(F32))
            nc.sync.dma_start(dbg["d_efb"][:], efb[:].bitcast(F32))

        rgbS = big.tile([3, CN], F32, tag="rgbS")
        sb_ = dram.tile([1, CN], F32, tag="sigb")
        sigflat = pp2.tile([1, CN], F32, tag="sigflat", bufs=1)

        def relu2(pmm, bname, i):
            """bias+relu both halves; engines alternate per tile parity."""
            hout = hp.tile([128, 2 * TILE_N], F32R, tag="fh", bufs=4)
            if i == 0:
                nc.scalar.activation(hout[:, 0:TILE_N], pmm[0][:], AF.Relu,
                                     bias=W[bname][:, 0:1])
                nc.vector.tensor_scalar(hout[:, TILE_N:], pmm[1][:],
                                        W[bname][:, 1:2], 0.0,
                                        op0=OP.add, op1=OP.max)
            else:
                nc.vector.tensor_scalar(hout[:, 0:TILE_N], pmm[0][:],
                                        W[bname][:, 0:1], 0.0,
                                        op0=OP.add, op1=OP.max)
                nc.scalar.activation(hout[:, TILE_N:], pmm[1][:], AF.Relu,
                                     bias=W[bname][:, 1:2])
            return hout

        for tp in range(NTILE // 2):
            pair = (2 * tp, 2 * tp + 1)
            colsv = [slice(t * TILE_N, (t + 1) * TILE_N) for t in pair]
            hh = []
            for i, t in enumerate(pair):
                pm = [psA.tile([128, TILE_N], F32, tag="mmps",
                               name="pm%d" % _m) for _m in range(2)]
                for m in range(2):
                    nc.tensor.matmul(pm[m][:],
                                     W['fW0my'][:, m * 128:(m + 1) * 128],
                                     efa[:, colsv[i]], start=True, stop=True)
                hh.append(relu2(pm, 'fb0col', i))
            if debug and ci == 0:
                nc.sync.dma_start(dbg["d_h1"][:], hh[0][:].bitcast(F32))

            for wname, bname, skip in (
                    ('fWm0', 'fbm0col', False), ('fWm1', 'fbm1col', False),
                    ('fWm2', 'fbm2col', False), ('fWs_h', 'fbscol', True),
                    ('fWp0', 'fbp0col', False), ('fWp1', 'fbp1col', False),
                    ('fWp2', 'fbp2col', False)):
                for i, t in enumerate(pair):
                    hin = hh[i]
                    pmm = [psA.tile([128, TILE_N], F32, tag="mmps",
                                    name="pmm%d" % _m) for _m in range(2)]
                    for m in range(2):
                        nc.tensor.matmul(pmm[m][:], W[wname][:, m, :],
                                         hin[:, 0:TILE_N],
                                         start=True, stop=False)
                        nc.tensor.matmul(pmm[m][:], W[wname][:, 2 + m, :],
                                         hin[:, TILE_N:],
                                         start=False, stop=not skip)
                        if skip:
                            nc.tensor.matmul(
                                pmm[m][:],
                                W['fWs_e'][:, m * 128:(m + 1) * 128],
                                efa[:, colsv[i]], start=False, stop=True)
                    hh[i] = relu2(pmm, bname, i)

            for i, t in enumerate(pair):
                h = hh[i]
                gtile = ci * NTILE + t
                ps_ = psS.tile([1, TILE_N], F32, tag="sigps")
                nc.tensor.matmul(ps_[:], W['Wsig'][:, 0:1], h[:, 0:TILE_N],
                                 start=True, stop=False)
                nc.tensor.matmul(ps_[:], W['Wsig'][:, 1:2], h[:, TILE_N:],
                                 start=False, stop=True)
                if t % 2 == 0:
                    nc.scalar.copy(sigflat[0:1, colsv[i]], ps_[:])
                else:
                    nc.vector.tensor_copy(sigflat[0:1, colsv[i]], ps_[:])

                pv = psA.tile([128, TILE_N], F32, tag="mmps")
                nc.tensor.matmul(pv[:], W['Wfc'][:, 0, :], h[:, 0:TILE_N],
                                 start=True, stop=False)
                nc.tensor.matmul(pv[:], W['Wfc'][:, 1, :], h[:, TILE_N:],
                                 start=False, stop=False)
                nc.tensor.matmul(pv[:], W['Wv_app'][:], efb[:, colsv[i]],
                                 start=False, stop=False)
                nc.tensor.matmul(pv[:], hvre[:, gtile, :], W['Etile'][:],
                                 start=False, stop=True)
                hv = hp.tile([128, TILE_N], F32R, tag="fhv", bufs=2)
                nc.vector.tensor_scalar(hv[:], pv[:], 0.0, None, op0=OP.max)

                prgb = psR.tile([3, TILE_N], F32, tag="rgbps")
                nc.tensor.matmul(prgb[:], W['Wrgb'][:], hv[:],
                                 start=True, stop=True)
                nc.scalar.activation(rgbS[0:3, colsv[i]], prgb[:],
                                     AF.Sigmoid, bias=W['brgbcol'][:])

        nc.sync.dma_start(sb_[:], sigflat[:])
        sigch = pp2.tile([CHUNK_RAYS, S], F32, tag="sigch")
        nc.sync.dma_start(sigch[:],
                          sb_[:].rearrange("a (p f) -> (a p) f", p=CHUNK_RAYS))
        zfc = pp2.tile([CHUNK_RAYS, S + 1], F32, tag="zfc")
        nc.sync.dma_start(zfc[:], zf[r0:r0 + CHUNK_RAYS, :])
        normc = pp2.tile([CHUNK_RAYS, 1], F32, tag="normc")
        nc.sync.dma_start(normc[:], norm[r0:r0 + CHUNK_RAYS, :])

        wf, _dzf = raw2w(sigch[:], zfc[:, 0:S], zfc[:, 1:S + 1],
                         normc[:], bsig_f, CHUNK_RAYS, "f")
        if debug:
            nc.sync.dma_start(dbg["d_sigf"][r0:r0 + CHUNK_RAYS, :], sigch[:])
            nc.sync.dma_start(dbg["d_wf"][r0:r0 + CHUNK_RAYS, :], wf[:])

        wb = dram.tile([CHUNK_RAYS, S], F32, tag="wb")
        nc.sync.dma_start(wb[:], wf[:])
        wBC = pp2.tile([3, CN], F32, tag="wbc", bufs=1)
        nc.sync.dma_start(
            wBC[:],
            wb[:].rearrange("p f -> (p f)").unsqueeze(0).broadcast_to([3, CN]))
        nc.gpsimd.tensor_tensor(rgbS[0:3, :], rgbS[0:3, :], wBC[0:3, :],
                                op=OP.mult)
        nc.vector.tensor_reduce(
            rgbmT[0:3, r0:r0 + CHUNK_RAYS],
            rgbS[0:3, :].rearrange("p (r s) -> p r s", r=CHUNK_RAYS),
            axis=mybir.AxisListType.X, op=OP.add)

    # out: transpose [3,128] -> [128,3] via DRAM bounce
    rb = dram.tile([3, 128], F32, tag="rb")
    nc.sync.dma_start(rb[:], rgbmT[:])
    rgbout = per.tile([128, 3], F32)
    nc.sync.dma_start(rgbout[:], rb[:].rearrange("c r -> r c"))
    nc.sync.dma_start(OUT[:], rgbout[:])
    ctx.close()


# ---------------------------------------------------------------- entry
_CACHE = {}


def kernel(**inputs):
    inp = {k: np.asarray(v) for k, v in inputs.items()}
    consts, scal = host_prep(inp)
    key = (BUILD_STAGE, DEBUG_OUT, scal['pbo_f'], scal['bsig_f'])
    if key not in _CACHE:
        _CACHE[key] = build_nc(scal['pbo_f'], scal['bsig_f'],
                               stage=BUILD_STAGE, debug=DEBUG_OUT)
    nc = _CACHE[key]
    rays = np.asarray(inp['rays'], np.float32)
    pR, pF = pack_consts(consts)
    in_maps = []
    for core in range(NCORES):
        m = {'packR': pR, 'packF': pF,
             'rays': np.ascontiguousarray(rays[core * R:(core + 1) * R])}
        in_maps.append(m)
    res = run_bass_kernel_spmd(nc, in_maps, core_ids=list(range(NCORES)))
    globals()['_LAST_RESULTS'] = res
    return np.concatenate([r['rgb_out'] for r in res.results], 0)

